# revision 1
# baseline (speedup 1.0000x reference)
"""AuthorGroupAttention Trainium2 kernel.

Data-parallel over batch: 8 samples -> 8 NeuronCores, one sample per core.
Per-sample routing (reader_token) is resolved on the host by gathering the
routed per-group weights into per-core combined projection weights.

Device-side layout is fully transposed ([feature, token]) so every matmul
contracts with the contraction dim on partitions:
  - Q/K projections per head h produce [128=(d_gen|d_rdr), T] tiles from
    host-combined weights [Wq.T[:,h*64:] | RWq[g].T[:,h*64:]].
  - scores^T[s,t] via row-packed K=64 matmuls (gen rows 0-63, rdr 64-127).
  - exp on ScalarE directly from PSUM with scale=D**-0.5 folded in.
  - attention: stationary operand is a 192-wide per-head-pair block
    [v_even(64) | 1_e | 0*31 | 1_o | 0*31 | v_odd(64)]; even heads read
    cols 0:128 so attn lands in PSUM partitions 0-63 with the softmax
    denominator Z at row 64, odd heads read cols 64:192 so attn lands in
    partitions 64-127 with Z at row 32 (32-aligned as PSUM access needs).
    Every PSUM drain is then partition-aligned for DVE.
  - normalize+combine (0.45/Zg + 0.05/Zr) on VectorE: 1/Z computed by DVE
    reciprocal straight off the PSUM row, shifted to partition 0 by DMA,
    then broadcast to all partitions with the GPSIMD partition_broadcast
    custom op; output projection streams Wo.T per o-tile with the v-bias
    folded into the output bias on the host (probs rows sum to 0.5, so
    attention over (v + bv) contributes exactly 0.5*bv per row).
"""

import os
import sys

for _p in ("/opt/trn_rl_repo",):
    if os.path.isdir(_p) and _p not in sys.path:
        sys.path.insert(0, _p)

import numpy as np

import concourse.bass as bass
import concourse.mybir as mybir
from concourse import bacc
from concourse.tile import TileContext
from concourse.bass_utils import run_bass_kernel_spmd

B, T, E, H, G = 8, 1024, 1024, 16, 4
D = E // H  # 64
SCALING = float(D) ** -0.5
W_G = 0.9 / 2.0  # generic path weight after the /2
W_R = 0.1 / 2.0  # reader path weight after the /2

F32 = mybir.dt.float32
F32R = mybir.dt.float32r
EO = E // 128  # 8 e-tiles
SO = T // 128  # 8 s-tiles
OO = E // 128  # 8 o-tiles
VB = 192  # v block width per head pair


def build_nc():
    nc = bacc.Bacc(name="author_group_attention")

    hsT = nc.dram_tensor("hsT", [E, T], F32R, kind="ExternalInput")
    wq = nc.dram_tensor("wq", [E, H, 128], F32R, kind="ExternalInput")
    wk = nc.dram_tensor("wk", [E, H, 128], F32R, kind="ExternalInput")
    wv = nc.dram_tensor("wv", [E, E], F32R, kind="ExternalInput")
    wo = nc.dram_tensor("wo", [E, E], F32R, kind="ExternalInput")
    bqk = nc.dram_tensor("bqk", [128, 2 * H], F32, kind="ExternalInput")
    wcol = nc.dram_tensor("wcol", [128, 4], F32, kind="ExternalInput")
    bo = nc.dram_tensor("bo", [128, OO], F32, kind="ExternalInput")
    outT = nc.dram_tensor("outT", [E, T], F32, kind="ExternalOutput")

    with TileContext(nc) as tc:
        from contextlib import ExitStack

        with ExitStack() as stack:
            const = stack.enter_context(tc.tile_pool(name="const", bufs=1))
            ppsum = stack.enter_context(
                tc.tile_pool(name="ppsum", bufs=1, space="PSUM")
            )

            hsT_sb = const.tile([128, EO, T], F32R, tag="hsT")
            hsT_r = hsT.rearrange("(eo ep) t -> ep eo t", ep=128)
            # v blocks: [s_p, s_o, pair, 192] = [v_even |1| 0*62 |1| v_odd]
            v_sb = const.tile([128, SO, H // 2, VB], F32R, tag="v")
            U32 = mybir.dt.uint32
            ONE_F32_BITS = 0x3F800000
            nc.vector.memset(v_sb[:].bitcast(U32), 0)
            nc.vector.memset(v_sb[:, :, :, D].bitcast(U32), ONE_F32_BITS)
            nc.vector.memset(v_sb[:, :, :, 96].bitcast(U32), ONE_F32_BITS)
            comb_tiles = [
                const.tile([128, T], F32R, tag=f"comb{eo}", name=f"comb{eo}") for eo in range(EO)
            ]
            bqk_sb = const.tile([128, 2 * H], F32, tag="bqk")
            wcol_sb = const.tile([128, 4], F32, tag="wcol")
            bo_sb = const.tile([128, OO], F32, tag="bo")

            wpool = stack.enter_context(tc.tile_pool(name="wqk", bufs=3))
            qkpool = stack.enter_context(tc.tile_pool(name="qk", bufs=2))

            def proj_steps(h, which):
                """Projection of combined Q or K for head h as a list of
                emission steps, so the PE work can be interleaved into other
                loops. The weight DMA fires now."""
                wt = wpool.tile([128, EO, 128], F32R, tag="w")
                srcw = wq if which == "q" else wk
                nc.sync.dma_start(
                    wt[:], srcw[:, h, :].rearrange("(eo ep) m -> ep eo m", ep=128)
                )
                dst = qkpool.tile([128, T], F32R, tag=which)
                bias_col = 2 * h if which == "q" else 2 * h + 1
                state = {}

                def mk_mm(nh, eo):
                    def step():
                        if nh == 0 and eo == 0:
                            state[0] = ppsum.tile([128, T], F32, tag="proj", name="pq")
                        nc.tensor.matmul(
                            state[0][:, nh * 512 : (nh + 1) * 512],
                            wt[:, eo, :],
                            hsT_sb[:, eo, nh * 512 : (nh + 1) * 512],
                            start=(eo == 0),
                            stop=(eo == EO - 1),
                        )
                        if eo == EO - 1:
                            nc.vector.tensor_scalar_add(
                                dst[:, nh * 512 : (nh + 1) * 512],
                                state[0][:, nh * 512 : (nh + 1) * 512],
                                bqk_sb[:, bias_col : bias_col + 1],
                            )
                    return step

                return dst, [mk_mm(nh, eo) for nh in range(2) for eo in range(EO)]

            # ---------------- v projection (natural layout [s, o]) ---------
            with tc.tile_pool(name="wvp", bufs=1) as wvp, tc.tile_pool(
                name="vpsum", bufs=3, space="PSUM"
            ) as vpsum:
                wv_sb = wvp.tile([128, EO, E], F32R, tag="wv")
                wv_r = wv.rearrange("(eo ep) o -> ep eo o", ep=128)
                nc.sync.dma_start(bqk_sb[:], bqk[:])
                Qh, steps_q0 = proj_steps(0, "q")
                Kh, steps_k0 = proj_steps(0, "k")
                qk0_pump = steps_q0 + steps_k0
                for eo in range(EO):
                    nc.sync.dma_start(hsT_sb[:, eo], hsT_r[:, eo])
                    nc.sync.dma_start(wv_sb[:, eo], wv_r[:, eo])
                nc.sync.dma_start(wcol_sb[:], wcol[:])
                nc.sync.dma_start(bo_sb[:], bo[:])
                # s-tiles in groups of 3 with eo-inner emission: each arriving
                # (hsT, wv) chunk pair immediately feeds the whole group, which
                # keeps PE fed while the first chunks stream in
                for g0 in range(0, SO, 3):
                    grp = list(range(g0, min(g0 + 3, SO)))
                    pvs = {}
                    for so in grp:
                        pvs[so] = vpsum.tile([128, T], F32, tag="vproj",
                                             name=f"pv{so}")
                    for eo in range(EO):
                        for so in grp:
                            for nh in range(2):
                                nc.tensor.matmul(
                                    pvs[so][:, nh * 512 : (nh + 1) * 512],
                                    hsT_sb[:, eo, so * 128 : (so + 1) * 128],
                                    wv_sb[:, eo, nh * 512 : (nh + 1) * 512],
                                    start=(eo == 0),
                                    stop=(eo == EO - 1),
                                )
                        for _ in range(2 if g0 >= 6 else 1):
                            if qk0_pump:
                                qk0_pump.pop(0)()
                    for so in grp:
                        pv4 = pvs[so].rearrange("p (m two d) -> p m two d", two=2, d=D)
                        nc.vector.tensor_copy(v_sb[:, so, :, 0:D], pv4[:, :, 0, :])
                        nc.vector.tensor_copy(
                            v_sb[:, so, :, 128 : 128 + D], pv4[:, :, 1, :]
                        )

                while qk0_pump:
                    qk0_pump.pop(0)()

            # ---------------- attention main loop ---------------------------
            with ExitStack() as attn_stack:
                expp = attn_stack.enter_context(tc.tile_pool(name="exp", bufs=4))
                rawp = attn_stack.enter_context(tc.tile_pool(name="raw", bufs=2))
                zp = attn_stack.enter_context(tc.tile_pool(name="z", bufs=1))
                bcp = attn_stack.enter_context(tc.tile_pool(name="bc", bufs=2))
                spsum = attn_stack.enter_context(
                    tc.tile_pool(name="spsum", bufs=2, space="PSUM")
                )
                apsum = attn_stack.enter_context(
                    tc.tile_pool(name="apsum", bufs=1, space="PSUM")
                )

                for h in range(H):
                    par_odd = h % 2  # 0 -> attn rows 0:64, Z row 64
                    abase = 64 * par_odd
                    zrow = 64 if par_odd == 0 else 32
                    voff = 64 * par_odd  # v block col offset
                    rawg = rawp.tile([128, T], F32, tag="rg")
                    rawr = rawp.tile([128, T], F32, tag="rr")
                    # zrec holds 1/Z rows (on partition zrow): cols 0:T gen,
                    # T:2T rdr
                    zrec = zp.tile([128, 2 * T], F32, tag="zrec")
                    nextQ = nextK = None
                    pump = []
                    if h + 1 < H:
                        nextQ, steps_q = proj_steps(h + 1, "q")
                        nextK, steps_k = proj_steps(h + 1, "k")
                        pump = steps_q + steps_k

                    for th in range(2):
                        tsl = slice(th * 512, (th + 1) * 512)
                        pag = apsum.tile([128, 512], F32, tag="ag")
                        par_ = apsum.tile([128, 512], F32, tag="ar")
                        for s in range(SO):
                            ps = spsum.tile([128, T], F32, tag="sc")
                            ssl = slice(s * 128, (s + 1) * 128)
                            nc.tensor.matmul(
                                ps[:, 0:512],
                                Kh[0:64, ssl],
                                Qh[0:64, tsl],
                                start=True,
                                stop=True,
                            )
                            nc.tensor.matmul(
                                ps[:, 512:1024],
                                Kh[64:128, ssl],
                                Qh[64:128, tsl],
                                start=True,
                                stop=True,
                            )
                            ex = expp.tile([128, T], F32R, tag="ex")
                            nc.scalar.activation(
                                ex[:],
                                ps[:],
                                mybir.ActivationFunctionType.Exp,
                                scale=SCALING,
                            )
                            vblk = v_sb[:, s, h // 2, voff : voff + 128]
                            nc.tensor.matmul(
                                pag[:],
                                vblk,
                                ex[:, 0:512],
                                start=(s == 0),
                                stop=(s == SO - 1),
                            )
                            nc.tensor.matmul(
                                par_[:],
                                vblk,
                                ex[:, 512:1024],
                                start=(s == 0),
                                stop=(s == SO - 1),
                            )
                            for _ in range(2):
                                if pump:
                                    pump.pop(0)()
                        # drain attention rows + Z row: wcol applies W^2 to
                        # attn rows and W to the Z row, so raw*(1/(W*Z))
                        # recovers W*attn/Z in the combine. Even heads merge
                        # both into one [0:65] op; odd heads need two ops
                        # (spans starting at partition 32 are limited to 32).
                        if par_odd == 0:
                            nc.vector.tensor_scalar_mul(
                                rawg[0:65, tsl], pag[0:65, :], wcol_sb[0:65, 0:1]
                            )
                            nc.vector.tensor_scalar_mul(
                                rawr[0:65, tsl], par_[0:65, :], wcol_sb[0:65, 1:2]
                            )
                        else:
                            nc.vector.tensor_scalar_mul(
                                rawg[64:128, tsl], pag[64:128, :], W_G * W_G
                            )
                            nc.vector.tensor_scalar_mul(
                                rawg[32:33, tsl], pag[32:33, :], W_G
                            )
                            nc.vector.tensor_scalar_mul(
                                rawr[64:128, tsl], par_[64:128, :], W_R * W_R
                            )
                            nc.vector.tensor_scalar_mul(
                                rawr[32:33, tsl], par_[32:33, :], W_R
                            )
                        zsl = slice(zrow, zrow + 1)
                        nc.vector.reciprocal(
                            zrec[zsl, th * 512 : th * 512 + 512], rawg[zsl, tsl]
                        )
                        nc.vector.reciprocal(
                            zrec[zsl, T + th * 512 : T + th * 512 + 512],
                            rawr[zsl, tsl],
                        )
                        # shift 1/Z rows to partition 0 (DMA can cross
                        # partitions), broadcast on GPSIMD, combine this half
                        nc.sync.dma_start(
                            zrec[0:1, th * 512 : th * 512 + 512],
                            zrec[zrow : zrow + 1, th * 512 : th * 512 + 512],
                        )
                        nc.sync.dma_start(
                            zrec[0:1, T + th * 512 : T + th * 512 + 512],
                            zrec[zrow : zrow + 1, T + th * 512 : T + th * 512 + 512],
                        )
                        bcg = bcp.tile([128, 512], F32, tag="bg")
                        bcr = bcp.tile([128, 512], F32, tag="br")
                        nc.gpsimd.partition_broadcast(
                            bcg[:], zrec[0:1, th * 512 : th * 512 + 512]
                        )
                        nc.gpsimd.partition_broadcast(
                            bcr[:], zrec[0:1, T + th * 512 : T + th * 512 + 512]
                        )
                        asl2 = slice(abase, abase + 64)
                        nc.vector.tensor_mul(
                            rawg[asl2, tsl], rawg[asl2, tsl], bcg[asl2, :]
                        )
                        nc.vector.tensor_mul(
                            rawr[asl2, tsl], rawr[asl2, tsl], bcr[asl2, :]
                        )
                        nc.vector.tensor_add(
                            comb_tiles[h // 2][asl2, tsl],
                            rawg[asl2, tsl],
                            rawr[asl2, tsl],
                        )
                    while pump:
                        pump.pop(0)()
                    if h + 1 < H:
                        Qh, Kh = nextQ, nextK

            # ---------------- output projection -----------------------------
            with tc.tile_pool(name="tail", bufs=3) as tailp, tc.tile_pool(
                name="outsb", bufs=2
            ) as outp, tc.tile_pool(name="opsum", bufs=2, space="PSUM") as opsum:
                wo_r = wo.rearrange("(eo ep) (oo m) -> oo ep eo m", ep=128, m=128)
                for j in range(OO):
                    wt = tailp.tile([128, EO, 128], F32R, tag="wo")
                    nc.sync.dma_start(wt[:], wo_r[j])
                    po = opsum.tile([128, T], F32, tag="oproj")
                    ot = outp.tile([128, T], F32, tag="ot")
                    for nh in range(2):
                        for eo in range(EO):
                            nc.tensor.matmul(
                                po[:, nh * 512 : (nh + 1) * 512],
                                wt[:, eo, :],
                                comb_tiles[eo][:, nh * 512 : (nh + 1) * 512],
                                start=(eo == 0),
                                stop=(eo == EO - 1),
                            )
                        nc.vector.tensor_scalar_add(
                            ot[:, nh * 512 : (nh + 1) * 512],
                            po[:, nh * 512 : (nh + 1) * 512],
                            bo_sb[:, j : j + 1],
                        )
                        nc.sync.dma_start(
                            outT[j * 128 : (j + 1) * 128, nh * 512 : (nh + 1) * 512],
                            ot[:, nh * 512 : (nh + 1) * 512],
                        )

    nc.finalize()
    return nc


_NC_CACHE = {}


def get_nc():
    if "nc" not in _NC_CACHE:
        _NC_CACHE["nc"] = build_nc()
    return _NC_CACHE["nc"]


def _host_prep(hidden_states, reader_token, Wq, bq, Wk, bk, Wv, bv, Wo, bo,
               RWq, Rbq, RWk, Rbk, RWv, Rbv):
    """Build the 8 per-core input maps (numpy only)."""
    f = np.float32
    hs = np.asarray(hidden_states, f)
    tok = np.asarray(reader_token).astype(np.int64)
    WqT = np.ascontiguousarray(np.asarray(Wq, f).T)  # [e, o]
    WkT = np.ascontiguousarray(np.asarray(Wk, f).T)
    WvT = np.ascontiguousarray(np.asarray(Wv, f).T)
    WoT = np.ascontiguousarray(np.asarray(Wo, f).T)
    RWqT = np.transpose(np.asarray(RWq, f), (0, 2, 1))  # [g, e, o]
    RWkT = np.transpose(np.asarray(RWk, f), (0, 2, 1))
    bq = np.asarray(bq, f); bk = np.asarray(bk, f)
    bv = np.asarray(bv, f); bo_ = np.asarray(bo, f)
    Rbq = np.asarray(Rbq, f); Rbk = np.asarray(Rbk, f)

    # v-bias folds into the output bias: probs rows sum to 0.5, so attention
    # over (v + bv) adds 0.5*bv to every attn row -> out += 0.5 * bv @ Wo.T
    bo_eff = bo_ + 0.5 * (np.asarray(Wo, f) @ bv)
    bo_t = np.ascontiguousarray(bo_eff.reshape(OO, 128).T)  # [128, oo]

    # shared [e, h, 64] views of the generic weights
    WqT_h = WqT.reshape(E, H, D)
    WkT_h = WkT.reshape(E, H, D)

    wcol_t = np.zeros((128, 4), f)
    wcol_t[0:64, 0] = W_G * W_G
    wcol_t[64, 0] = W_G
    wcol_t[0:64, 1] = W_R * W_R
    wcol_t[64, 1] = W_R
    wcol_t[64:128, 2] = W_G * W_G
    wcol_t[32, 2] = W_G
    wcol_t[64:128, 3] = W_R * W_R
    wcol_t[32, 3] = W_R

    in_maps = []
    percore = {}
    for b in range(B):
        g = int(tok[b])
        if g not in percore:
            wqc = np.empty((E, H, 128), f)
            wqc[:, :, :D] = WqT_h
            wqc[:, :, D:] = RWqT[g].reshape(E, H, D)
            wkc = np.empty((E, H, 128), f)
            wkc[:, :, :D] = WkT_h
            wkc[:, :, D:] = RWkT[g].reshape(E, H, D)
            # per-head combined biases: col 2h = [bq_h|Rbq_h], col 2h+1 = k
            bqk_t = np.empty((128, 2 * H), f)
            bqk_t[:D, 0::2] = bq.reshape(H, D).T
            bqk_t[D:, 0::2] = Rbq[g].reshape(H, D).T
            bqk_t[:D, 1::2] = bk.reshape(H, D).T
            bqk_t[D:, 1::2] = Rbk[g].reshape(H, D).T
            percore[g] = (wqc, wkc, bqk_t)
        wqc, wkc, bqk_t = percore[g]
        in_maps.append(
            {
                "hsT": np.ascontiguousarray(hs[b].T),
                "wq": wqc,
                "wk": wkc,
                "wv": WvT,
                "wo": WoT,
                "bqk": bqk_t,
                "wcol": wcol_t,
                "bo": bo_t,
            }
        )
    return in_maps


def kernel(**inputs) -> np.ndarray:
    in_maps = _host_prep(**inputs)
    nc = get_nc()
    res = run_bass_kernel_spmd(nc, in_maps, list(range(B)))
    out = np.stack([res.results[c]["outT"].T for c in range(B)], axis=0)
    return np.ascontiguousarray(out.astype(np.float32))


if __name__ == "__main__":
    rng = np.random.default_rng(0)
    ins = {
        "hidden_states": rng.standard_normal((B, T, E), dtype=np.float32),
        "reader_token": rng.integers(0, G, size=(B,)).astype(np.int32),
        "Wq": rng.standard_normal((E, E), dtype=np.float32) * 0.02,
        "bq": np.zeros(E, np.float32),
        "Wk": rng.standard_normal((E, E), dtype=np.float32) * 0.02,
        "bk": np.zeros(E, np.float32),
        "Wv": rng.standard_normal((E, E), dtype=np.float32) * 0.02,
        "bv": np.zeros(E, np.float32),
        "Wo": rng.standard_normal((E, E), dtype=np.float32) * 0.02,
        "bo": np.zeros(E, np.float32),
        "RWq": rng.standard_normal((G, E, E), dtype=np.float32) * 0.02,
        "Rbq": np.zeros((G, E), np.float32),
        "RWk": rng.standard_normal((G, E, E), dtype=np.float32) * 0.02,
        "Rbk": np.zeros((G, E), np.float32),
        "RWv": rng.standard_normal((G, E, E), dtype=np.float32) * 0.02,
        "Rbv": np.zeros((G, E), np.float32),
    }
    out = kernel(**ins)
    print("out", out.shape, out.dtype, float(np.abs(out).max()))



# revision 2
# speedup vs baseline: 1.0798x; 1.0798x over previous
"""AuthorGroupAttention Trainium2 kernel, v2.

Data-parallel over batch: 8 samples -> 8 NeuronCores, one sample per core.
Routing resolved on host (per-core reader-group weights gathered).

Precision plan (validated numerically, rel err ~5e-3 vs 2e-2 gate):
  - generic path: fp16 operands everywhere (matmul accum fp32 in PSUM)
  - reader path (0.1 weight in the combine): fp8e4 operands with DoubleRow
    matmuls, exp via Schraudolph bit-trick on DVE writing e4m3 bits directly
Layouts:
  - q/k in [d, t] (d on partitions) for scoresT = K^T-block stationary vs
    Q moving -> scores [s, t] in PSUM, exp'd per s-block to SBUF
  - attention transposed: stationary = exp-block [s,t], moving = [v | 1/w]
    -> attn [t, d] in PSUM with the softmax denominator Z (pre-scaled by
    1/w) landing in the extra column; per-partition 1/Z scale on ACT/DVE,
    no cross-partition broadcast needed
  - attn [t, e] tiles are DMA-transposed (xbar) to [e, t] for the output
    projection
"""

import os
import sys

for _p in ("/opt/trn_rl_repo",):
    if os.path.isdir(_p) and _p not in sys.path:
        sys.path.insert(0, _p)

import numpy as np

import concourse.bass as bass
import concourse.mybir as mybir
from concourse import bacc
from concourse.tile import TileContext
from concourse.bass_utils import run_bass_kernel_spmd

B, T, E, H, G = 8, 1024, 1024, 16, 4
D = E // H  # 64
SCALING = float(D) ** -0.5
W_G = 0.9 / 2.0
W_R = 0.1 / 2.0
EO = 8
SO = 8
TB = 8
NP = 8  # head pairs
NQ = 4  # head quads

F32 = mybir.dt.float32
F16 = mybir.dt.float16
F8 = mybir.dt.float8e4
U8 = mybir.dt.uint8
DRM = mybir.MatmulPerfMode.DoubleRow
EXP = mybir.ActivationFunctionType.Exp
MULT = mybir.AluOpType.mult
ADD = mybir.AluOpType.add

# Schraudolph constants for e4m3 bits: bits = round(score*A + Bc)
SCH_A = SCALING * 8.0 / float(np.log(2.0))
SCH_B = 56.0 - 0.8

USE_RDR_SCORE_DR = True  # DoubleRow with 32-row subtiles for reader scores


def build_nc():
    nc = bacc.Bacc(name="author_group_attention_v2")

    hsT16 = nc.dram_tensor("hsT16", [E, T], F16, kind="ExternalInput")
    hsT8 = nc.dram_tensor("hsT8", [E, T], F8, kind="ExternalInput")
    wg = nc.dram_tensor("wg", [2, NP, 128, EO, 128], F16, kind="ExternalInput")
    w8 = nc.dram_tensor("w8", [2, NQ, 2, 128, EO, 128], F8, kind="ExternalInput")
    wv = nc.dram_tensor("wv", [128, EO, E], F16, kind="ExternalInput")
    wo = nc.dram_tensor("wo", [TB, 128, NP, 128], F16, kind="ExternalInput")
    gbias = nc.dram_tensor("gbias", [128, 2, NP], F32, kind="ExternalInput")
    rbias = nc.dram_tensor("rbias", [128, 2, NQ, 2], F32, kind="ExternalInput")
    bo = nc.dram_tensor("bo", [128, TB], F32, kind="ExternalInput")
    outT = nc.dram_tensor("outT", [E, T], F16, kind="ExternalOutput")

    hsT16_r = hsT16.rearrange("(eo p) t -> p eo t", p=128)
    hsT8_r = hsT8.rearrange("(eo p) t -> p eo t", p=128)

    with TileContext(nc) as tc:
        from contextlib import ExitStack

        with ExitStack() as stack:
            const = stack.enter_context(tc.tile_pool(name="const", bufs=1))

            hsT16_sb = const.tile([128, EO, T], F16, tag="hsT16")
            hs8_sb = const.tile([128, EO, T], F8, tag="hs8")
            v16_sb = const.tile([128, SO, H, 66], F16, tag="v16")
            v8_sb = const.tile([128, SO, H, 66], F8, tag="v8")
            gbias_sb = const.tile([128, 2, NP], F32, tag="gbias")
            rbias_sb = const.tile([128, 2, NQ, 2], F32, tag="rbias")
            bo_sb = const.tile([128, TB], F32, tag="bo")
            comb_tiles = [
                const.tile([128, E], F16, tag=f"comb{tb}", name=f"comb{tb}")
                for tb in range(TB)
            ]
            attnT_tiles = [
                const.tile([128, T], F16, tag=f"attnT{p}", name=f"attnT{p}")
                for p in range(NP)
            ]

            # ones columns pre-scaled by 1/w so reciprocal gives w/Z
            nc.vector.memset(v16_sb[:, :, :, 64:65], 1.0 / W_G)
            nc.vector.memset(v8_sb[:, :, :, 64:65], 1.0 / W_R)

            nc.sync.dma_start(gbias_sb[:], gbias[:])
            nc.sync.dma_start(rbias_sb[:], rbias[:])
            nc.sync.dma_start(bo_sb[:], bo[:])

            # persistent pools used across prologue + main
            qk16p = stack.enter_context(tc.tile_pool(name="qk16", bufs=2))
            qk8p = stack.enter_context(tc.tile_pool(name="qk8", bufs=2))
            wgp = stack.enter_context(tc.tile_pool(name="wgp", bufs=3))
            w8p = stack.enter_context(tc.tile_pool(name="w8p", bufs=4))

            def gen_proj_steps(pair, pp2_pool):
                """Generic q/k projection for a head pair -> list of step
                closures (PE matmuls + ACT drains). Weight DMAs fire now."""
                steps = []
                outs = {}
                for qk in range(2):
                    wt = wgp.tile([128, EO, 128], F16, tag="wg")
                    nc.sync.dma_start(wt[:], wg[qk, pair])
                    dst = qk16p.tile([128, T], F16, tag=f"qk16_{qk}")
                    outs[qk] = dst
                    state = {}

                    def mk(qk, wt, dst, state, th, eo):
                        def step():
                            if eo == 0:
                                state[th] = pp2_pool.tile(
                                    [128, 512], F32, tag="sm", name=f"gp{pair}{qk}{th}"
                                )
                            nc.tensor.matmul(
                                state[th][:],
                                wt[:, eo, :],
                                hsT16_sb[:, eo, th * 512 : (th + 1) * 512],
                                start=(eo == 0),
                                stop=(eo == EO - 1),
                            )
                            if eo == EO - 1:
                                nc.scalar.activation(
                                    dst[:, th * 512 : (th + 1) * 512],
                                    state[th][:],
                                    mybir.ActivationFunctionType.Identity,
                                    bias=gbias_sb[:, qk, pair : pair + 1],
                                )
                        return step

                    for th in range(2):
                        for eo in range(EO):
                            steps.append(mk(qk, wt, dst, state, th, eo))
                return outs, steps

            def rdr_proj_steps(quad, pp2_pool):
                """Reader q/k projection for a head quad (fp8 DoubleRow).
                Outputs supertiles [128(=4h x 32d), 2(d-half), T] fp8."""
                steps = []
                outs = {}
                for qk in range(2):
                    dst = qk8p.tile([128, 2, T], F8, tag=f"qk8_{qk}")
                    outs[qk] = dst
                    for ab in range(2):
                        wt = w8p.tile([128, EO, 128], F8, tag="w8")
                        nc.sync.dma_start(wt[:], w8[qk, quad, ab])
                        state = {}

                        def mk(qk, ab, wt, dst, state, th, a):
                            def step():
                                if a == 0:
                                    state[th] = pp2_pool.tile(
                                        [128, 512], F32, tag="sm",
                                        name=f"rp{quad}{qk}{ab}{th}",
                                    )
                                nc.tensor.matmul(
                                    state[th][:],
                                    wt[:, 2 * a : 2 * a + 2, :],
                                    hs8_sb[:, 2 * a : 2 * a + 2,
                                           th * 512 : (th + 1) * 512],
                                    start=(a == 0),
                                    stop=(a == 3),
                                    perf_mode=DRM,
                                )
                                if a == 3:
                                    nc.vector.tensor_scalar_add(
                                        dst[:, ab, th * 512 : (th + 1) * 512],
                                        state[th][:],
                                        rbias_sb[:, qk, quad, ab : ab + 1],
                                    )
                            return step

                        for th in range(2):
                            for a in range(4):
                                steps.append(mk(qk, ab, wt, dst, state, th, a))
                return outs, steps

            # ---------------- prologue: v proj + first projections ----------
            pump = []
            with tc.tile_pool(name="wvp", bufs=1) as wvp, tc.tile_pool(
                name="vps", bufs=2, space="PSUM"
            ) as vps, tc.tile_pool(name="pps", bufs=2, space="PSUM") as pps:
                wv_sb = wvp.tile([128, EO, E], F16, tag="wv")
                for eo in range(EO):
                    nc.sync.dma_start(hsT16_sb[:, eo], hsT16_r[:, eo])
                    nc.sync.dma_start(wv_sb[:, eo], wv[:, eo])
                for eo in range(EO):
                    nc.sync.dma_start(hs8_sb[:, eo], hsT8_r[:, eo])

                qk0, steps_g0 = gen_proj_steps(0, pps)
                r0, steps_r0 = rdr_proj_steps(0, pps)
                pro_pump = steps_g0 + steps_r0

                for sb in range(SO):
                    pv = vps.tile([128, E], F32, tag="pv", name=f"pv{sb}")
                    for eo in range(EO):
                        for vh in range(2):
                            nc.tensor.matmul(
                                pv[:, vh * 512 : (vh + 1) * 512],
                                hsT16_sb[:, eo, sb * 128 : (sb + 1) * 128],
                                wv_sb[:, eo, vh * 512 : (vh + 1) * 512],
                                start=(eo == 0),
                                stop=(eo == EO - 1),
                            )
                        for _ in range(2 if sb >= 4 else 1):
                            if pro_pump:
                                pro_pump.pop(0)()
                    pv_r = pv.rearrange("p (hh dd) -> p hh dd", dd=64)
                    nc.scalar.copy(v16_sb[:, sb, :, 0:64], pv_r)
                    nc.vector.tensor_copy(v8_sb[:, sb, :, 0:64], pv_r)
                while pro_pump:
                    pro_pump.pop(0)()

            # ---------------- main attention loop ---------------------------
            # Software pipeline: the AV/combine work of head h-1 is emitted
            # interleaved into the scores/exp loop of head h, so the PE has
            # filler work while ACT/DVE drain the score tiles.
            wo_sb = const.tile([128, TB, NP, 128], F16, tag="wo_all")
            for j in range(TB):
                nc.sync.dma_start(wo_sb[:, j], wo[j])

            with ExitStack() as mstack:
                scp = mstack.enter_context(
                    tc.tile_pool(name="scp", bufs=2, space="PSUM")
                )
                smallp = mstack.enter_context(
                    tc.tile_pool(name="smallp", bufs=4, space="PSUM")
                )
                ex16p = mstack.enter_context(tc.tile_pool(name="ex16", bufs=2))
                ex8p = mstack.enter_context(tc.tile_pool(name="ex8", bufs=2))
                zp = mstack.enter_context(tc.tile_pool(name="zp", bufs=8))
                tmpp = mstack.enter_context(tc.tile_pool(name="tmpp", bufs=8))

                def av_steps(h, ex16, ex8):
                    """AV + combine for head h as a list of step closures.
                    Each tb yields 3 steps: gen-av mms, rdr-av mms, combine."""
                    pair, hp = h // 2, h % 2
                    vg = v16_sb[:, :, h, :]
                    v8h = v8_sb[:, :, h, :]
                    steps = []
                    state = {}

                    def mk_gen(tb):
                        def step():
                            tsl = slice(tb * 128, (tb + 1) * 128)
                            av = smallp.tile([128, 512], F32, tag="sm",
                                             name=f"av{h}_{tb}")
                            state[tb] = av
                            for a in range(SO):
                                nc.tensor.matmul(
                                    av[:, 0:65],
                                    ex16[:, a, tsl],
                                    vg[:, a, 0:65],
                                    start=(a == 0),
                                    stop=(a == SO - 1),
                                )
                        return step

                    def mk_rdr(tb):
                        def step():
                            tsl = slice(tb * 128, (tb + 1) * 128)
                            av = state[tb]
                            for a in range(4):
                                nc.tensor.matmul(
                                    av[:, 68:133],
                                    ex8[:, 2 * a : 2 * a + 2, tsl],
                                    v8h[:, 2 * a : 2 * a + 2, 0:65],
                                    start=(a == 0),
                                    stop=(a == 3),
                                    perf_mode=DRM,
                                )
                        return step

                    def mk_comb(tb):
                        def step():
                            av = state.pop(tb)
                            zr = zp.tile([128, 2], F32, tag="zr")
                            nc.vector.reciprocal(zr[:], av[:, 64:133:68])
                            csl = slice(h * 64, h * 64 + 64)
                            nc.scalar.mul(
                                comb_tiles[tb][:, csl], av[:, 0:64], zr[:, 0:1]
                            )
                            tmp = tmpp.tile([128, 64], F16, tag="tmp")
                            nc.scalar.mul(tmp[:], av[:, 68:132], zr[:, 1:2])
                            nc.gpsimd.tensor_tensor(
                                comb_tiles[tb][:, csl],
                                comb_tiles[tb][:, csl],
                                tmp[:],
                                ADD,
                            )
                            if hp == 1 and tb == TB - 1:
                                for tb2 in range(TB):
                                    nc.sync.dma_start_transpose(
                                        attnT_tiles[pair][
                                            :, tb2 * 128 : (tb2 + 1) * 128
                                        ],
                                        comb_tiles[tb2][
                                            :, pair * 128 : (pair + 1) * 128
                                        ],
                                    )
                        return step

                    for tb in range(TB):
                        steps.append(mk_gen(tb))
                        steps.append(mk_rdr(tb))
                        steps.append(mk_comb(tb))
                    return steps

                Qg = Kg = Q8 = K8 = None
                av_q = []  # pending av steps of the previous head

                def fill(n):
                    """Emit up to n units of filler: av steps take priority
                    (they unblock comb tiles), then proj pump steps."""
                    for _ in range(n):
                        if av_q:
                            av_q.pop(0)()
                        elif pump:
                            pump.pop(0)()

                for h in range(H):
                    pair, quad = h // 2, h // 4
                    hp, hq = h % 2, h % 4
                    if h == 0:
                        Qg, Kg = qk0[0], qk0[1]
                        Q8, K8 = r0[0], r0[1]
                        nxt_g = nxt_r = None
                    if hp == 0 and pair + 1 < NP:
                        nxt_g, s = gen_proj_steps(pair + 1, smallp)
                        pump.extend(s)
                    if hq == 0 and quad + 1 < NQ:
                        nxt_r, s = rdr_proj_steps(quad + 1, smallp)
                        pump.extend(s)

                    ex16 = ex16p.tile([128, SO, T], F16, tag="ex16")
                    ex8 = ex8p.tile([128, SO, T], F8, tag="ex8")

                    grow = slice(64 * hp, 64 * hp + 64)
                    rrow = slice(32 * hq, 32 * hq + 32)

                    for sb in range(SO):
                        ssl = slice(sb * 128, (sb + 1) * 128)
                        sc = scp.tile([128, T], F32, tag="sc", name=f"sc{h}_{sb}")
                        for th in range(2):
                            nc.tensor.matmul(
                                sc[:, th * 512 : (th + 1) * 512],
                                Kg[grow, ssl],
                                Qg[grow, th * 512 : (th + 1) * 512],
                                start=True, stop=True,
                            )
                        nc.scalar.activation(
                            ex16[:, sb, :], sc[:], EXP, scale=SCALING
                        )
                        fill(2)
                        for th in range(2):
                            rc = smallp.tile([128, 512], F32, tag="sm",
                                             name=f"rc{h}_{sb}_{th}")
                            if USE_RDR_SCORE_DR:
                                nc.tensor.matmul(
                                    rc[:],
                                    K8[rrow, :, ssl],
                                    Q8[rrow, :, th * 512 : (th + 1) * 512],
                                    start=True, stop=True,
                                    perf_mode=DRM,
                                    tile_position=(32 * hq, 0),
                                )
                            else:
                                for ab in range(2):
                                    nc.tensor.matmul(
                                        rc[:],
                                        K8[rrow, ab, ssl],
                                        Q8[rrow, ab, th * 512 : (th + 1) * 512],
                                        start=(ab == 0), stop=(ab == 1),
                                        tile_position=(32 * hq, 0),
                                    )
                            nc.vector.tensor_scalar(
                                ex8[:, sb, th * 512 : (th + 1) * 512].bitcast(U8),
                                rc[:],
                                SCH_A, SCH_B, MULT, ADD,
                            )
                            fill(1)
                        fill(1)

                    # queue this head's av work; emitted during later heads'
                    # scores loops via fill()
                    av_q.extend(av_steps(h, ex16, ex8))

                    if hp == 1:
                        while pump:
                            pump.pop(0)()
                        if nxt_g is not None:
                            Qg, Kg = nxt_g[0], nxt_g[1]
                            nxt_g = None
                        if hq == 3 and nxt_r is not None:
                            Q8, K8 = nxt_r[0], nxt_r[1]
                            nxt_r = None

                while av_q:
                    av_q.pop(0)()

            # ---------------- output projection -----------------------------
            with tc.tile_pool(name="ops", bufs=2, space="PSUM") as ops, \
                 tc.tile_pool(name="o16p", bufs=2) as o16p:
                for j in range(TB):
                    po = ops.tile([128, T], F32, tag="po", name=f"po{j}")
                    for th in range(2):
                        for pr in range(NP):
                            nc.tensor.matmul(
                                po[:, th * 512 : (th + 1) * 512],
                                wo_sb[:, j, pr, :],
                                attnT_tiles[pr][:, th * 512 : (th + 1) * 512],
                                start=(pr == 0),
                                stop=(pr == NP - 1),
                            )
                    o16 = o16p.tile([128, T], F16, tag="o16")
                    nc.vector.tensor_scalar_add(o16[:], po[:], bo_sb[:, j : j + 1])
                    nc.sync.dma_start(outT[j * 128 : (j + 1) * 128, :], o16[:])

    nc.finalize()
    return nc


_NC_CACHE = {}


def get_nc():
    if "nc" not in _NC_CACHE:
        _NC_CACHE["nc"] = build_nc()
    return _NC_CACHE["nc"]


def _host_prep(hidden_states, reader_token, Wq, bq, Wk, bk, Wv, bv, Wo, bo,
               RWq, Rbq, RWk, Rbk, RWv, Rbv):
    f = np.float32
    np16 = mybir.dt.np(F16)
    np8 = mybir.dt.np(F8)
    hs = np.asarray(hidden_states, f)
    tok = np.asarray(reader_token).astype(np.int64)

    WqT = np.asarray(Wq, f).T  # [e, o]
    WkT = np.asarray(Wk, f).T
    WvT = np.asarray(Wv, f).T
    WoT = np.asarray(Wo, f).T
    bq = np.asarray(bq, f); bk = np.asarray(bk, f)
    bv = np.asarray(bv, f); bo_ = np.asarray(bo, f)
    Rbq = np.asarray(Rbq, f); Rbk = np.asarray(Rbk, f)

    # gen weights [2, NP, 128, EO, 128]
    wg_arr = np.empty((2, NP, 128, EO, 128), np16)
    for qk, WT in enumerate((WqT, WkT)):
        r = WT.reshape(EO, 128, NP, 128)  # (eo, p, pair, m)
        wg_arr[qk] = r.transpose(2, 1, 0, 3).astype(np16)
    # gen biases [128, 2, NP]
    gb = np.empty((128, 2, NP), f)
    for qk, bb in enumerate((bq, bk)):
        gb[:, qk, :] = bb.reshape(NP, 128).T

    # v-bias folds into output bias (probs rows sum to 0.5)
    bo_eff = bo_ + 0.5 * (np.asarray(Wo, f) @ bv)
    bo_t = np.ascontiguousarray(bo_eff.reshape(TB, 128).T)

    # wv [128, EO, E]
    wv_arr = np.ascontiguousarray(
        WvT.reshape(EO, 128, E).transpose(1, 0, 2)
    ).astype(np16)
    # wo [TB, 128, NP, 128]
    wo_arr = np.ascontiguousarray(
        WoT.reshape(NP, 128, TB, 128).transpose(2, 1, 0, 3)
    ).astype(np16)

    percore = {}
    in_maps = []
    for b in range(B):
        g = int(tok[b])
        if g not in percore:
            RWqT = np.asarray(RWq[g], f).T  # [e, o]
            RWkT = np.asarray(RWk[g], f).T
            w8_arr = np.empty((2, NQ, 2, 128, EO, 128), np8)
            for qk, WT in enumerate((RWqT, RWkT)):
                # o = (quad*4 + m//32)*64 + ab*32 + m%32
                r = WT.reshape(EO, 128, NQ, 4, 2, 32)  # (eo,p,quad,hin,ab,dd)
                # -> (qk, quad, ab, p, eo, m=(hin,dd))
                w8_arr[qk] = r.transpose(2, 4, 1, 0, 3, 5).reshape(
                    NQ, 2, 128, EO, 128
                ).astype(np8)
            rb = np.empty((128, 2, NQ, 2), f)
            for qk, bb in enumerate((Rbq[g], Rbk[g])):
                # p = hin*32 + dd ; value = b[(quad*4+hin)*64 + ab*32 + dd]
                r = bb.reshape(NQ, 4, 2, 32)  # (quad, hin, ab, dd)
                rb[:, qk, :, :] = r.transpose(1, 3, 0, 2).reshape(128, NQ, 2)
            percore[g] = (w8_arr, rb)
        w8_arr, rb = percore[g]
        hsT = np.ascontiguousarray(hs[b].T)
        in_maps.append(
            {
                "hsT16": hsT.astype(np16),
                "hsT8": hsT.astype(np8),
                "wg": wg_arr,
                "w8": w8_arr,
                "wv": wv_arr,
                "wo": wo_arr,
                "gbias": gb,
                "rbias": rb,
                "bo": bo_t,
            }
        )
    return in_maps


def kernel(**inputs) -> np.ndarray:
    in_maps = _host_prep(**inputs)
    nc = get_nc()
    res = run_bass_kernel_spmd(nc, in_maps, list(range(B)))
    out = np.stack(
        [np.asarray(res.results[c]["outT"]).astype(np.float32).T for c in range(B)],
        axis=0,
    )
    return np.ascontiguousarray(out)


# revision 4
# speedup vs baseline: 1.1016x; 1.0202x over previous
"""AuthorGroupAttention Trainium2 kernel.

Data-parallel over batch: 8 samples -> 8 NeuronCores, one sample per core.
Routing resolved on host (per-core reader-group weights gathered, cast and
laid out per-engine-friendly in _host_prep).

Precision (validated vs reference: rel err ~5e-3 against the 2e-2 gate):
  - generic path: fp16 matmul operands everywhere, fp32 PSUM accumulation
  - reader path (weight 0.1 in the prob combine): fp8e4 operands with
    DoubleRow matmuls (0.5 cyc/row); its exp is a Schraudolph bit-trick on
    DVE (scores*1/ln2 + 55.2 rounded to uint8 = e4m3 bits of exp(scores/8)),
    consistent numerator/denominator so the approximation bias cancels in
    the softmax normalization.

Structure:
  - q/k produced in [d, t] layout (d on partitions): per head-pair (gen,
    fp16) / head-quad (rdr, fp8 DR with 32-row subtiles at tile_position)
    projection chains, interleaved into the attention loop as PE filler
    ("pump") with per-boundary forced drains.
  - scores per (head, s-block): gen [128,1024] psum tiles (2-deep pool),
    rdr th-split [128,512] tiles in a shared 4-deep "small" pool that also
    carries av accumulators, projection chains, and deferred v-proj.
  - attention transposed: stationary = exp tile [s,t], moving = [v | 1/w]
    so attn lands [t, d] with the softmax denominator Z/w in the extra
    column; one ACT copy bounces the accumulator to SBUF, gpsimd
    normalize_recip applies w/Z per path, gpsimd add combines gen+rdr.
  - attn [t, e] tiles are DMA-transposed (xbar) per (pair, t-block) into
    [e, t] for the fp16 output projection; v-bias is folded into the output
    bias on host (combined prob rows sum to 0.5).
  - AV/combine of head h-1 is software-pipelined into head h's score loop;
    v-projection for pairs 6-7 is deferred into the pump as late filler.
"""

import os
import sys

for _p in ("/opt/trn_rl_repo",):
    if os.path.isdir(_p) and _p not in sys.path:
        sys.path.insert(0, _p)

import numpy as np

import concourse.bass as bass
import concourse.mybir as mybir
from concourse import bacc
from concourse.tile import TileContext
from concourse.bass_utils import run_bass_kernel_spmd

B, T, E, H, G = 8, 1024, 1024, 16, 4
D = E // H  # 64
SCALING = float(D) ** -0.5
W_G = 0.9 / 2.0
W_R = 0.1 / 2.0
EO = 8
SO = 8
TB = 8
NP = 8  # head pairs
NQ = 4  # head quads

F32 = mybir.dt.float32
F16 = mybir.dt.float16
F8 = mybir.dt.float8e4
U8 = mybir.dt.uint8
DRM = mybir.MatmulPerfMode.DoubleRow
EXP = mybir.ActivationFunctionType.Exp
MULT = mybir.AluOpType.mult
ADD = mybir.AluOpType.add

# Schraudolph constants for e4m3 bits: bits = round(score*A + Bc)
SCH_A = SCALING * 8.0 / float(np.log(2.0))
SCH_B = 56.0 - 0.8

USE_RDR_SCORE_DR = True  # DoubleRow with 32-row subtiles for reader scores


def build_nc():
    nc = bacc.Bacc(name="author_group_attention_v2")

    hsT16 = nc.dram_tensor("hsT16", [E, T], F16, kind="ExternalInput")
    hsT8 = nc.dram_tensor("hsT8", [E, T], F8, kind="ExternalInput")
    wg = nc.dram_tensor("wg", [NP, 128, 2, EO, 128], F16, kind="ExternalInput")
    w8 = nc.dram_tensor("w8", [NQ, 128, 2, 2, EO, 128], F8, kind="ExternalInput")
    wv = nc.dram_tensor("wv", [128, EO, E], F16, kind="ExternalInput")
    wo = nc.dram_tensor("wo", [TB, 128, NP, 128], F16, kind="ExternalInput")
    gbias = nc.dram_tensor("gbias", [128, 2, NP], F32, kind="ExternalInput")
    rbias = nc.dram_tensor("rbias", [128, 2, NQ, 2], F32, kind="ExternalInput")
    bo = nc.dram_tensor("bo", [128, TB], F32, kind="ExternalInput")
    outT = nc.dram_tensor("outT", [E, T], F16, kind="ExternalOutput")

    hsT16_r = hsT16.rearrange("(eo p) t -> p eo t", p=128)
    hsT8_r = hsT8.rearrange("(eo p) t -> p eo t", p=128)


    with TileContext(nc) as tc:
        from contextlib import ExitStack

        with ExitStack() as stack:
            const = stack.enter_context(tc.tile_pool(name="const", bufs=1))

            hsT16_sb = const.tile([128, EO, T], F16, tag="hsT16")
            hs8_sb = const.tile([128, EO, T], F8, tag="hs8")
            v16_sb = const.tile([128, SO, H, 66], F16, tag="v16")
            wv_sb = const.tile([128, EO, E], F16, tag="wv")
            v8_sb = const.tile([128, SO, H, 66], F8, tag="v8")
            gbias_sb = const.tile([128, 2, NP], F32, tag="gbias")
            rbias_sb = const.tile([128, 2, NQ, 2], F32, tag="rbias")
            bo_sb = const.tile([128, TB], F32, tag="bo")
            comb_tiles = [
                const.tile([128, E], F16, tag=f"comb{tb}", name=f"comb{tb}")
                for tb in range(TB)
            ]
            attnT_tiles = [
                const.tile([128, T], F16, tag=f"attnT{p}", name=f"attnT{p}")
                for p in range(NP)
            ]

            # ones columns pre-scaled by 1/w so reciprocal gives w/Z
            nc.vector.memset(v16_sb[:, :, :, 64:65], 1.0 / W_G)
            nc.vector.memset(v8_sb[:, :, :, 64:65], 1.0 / W_R)


            # persistent pools used across prologue + main
            qk16p = stack.enter_context(tc.tile_pool(name="qk16", bufs=2))
            qk8p = stack.enter_context(tc.tile_pool(name="qk8", bufs=2))
            wgp = stack.enter_context(tc.tile_pool(name="wgp", bufs=2))
            w8p = stack.enter_context(tc.tile_pool(name="w8p", bufs=2))

            def gen_proj_steps(pair, pp2_pool):
                """Generic q/k projection for a head pair -> list of step
                closures (PE matmuls + ACT drains). Weight DMAs fire now."""
                steps = []
                outs = {}
                wt2 = wgp.tile([128, 2, EO, 128], F16, tag="wg")
                nc.sync.dma_start(wt2[:], wg[pair])
                for qk in range(2):
                    wt = wt2[:, qk]
                    dst = qk16p.tile([128, T], F16, tag=f"qk16_{qk}")
                    outs[qk] = dst
                    state = {}

                    def mk(qk, wt, dst, state, th, eo):
                        def step():
                            if eo == 0:
                                state[th] = pp2_pool.tile(
                                    [128, 512], F32, tag="sm", name=f"gp{pair}{qk}{th}"
                                )
                            nc.tensor.matmul(
                                state[th][:],
                                wt[:, eo, :],
                                hsT16_sb[:, eo, th * 512 : (th + 1) * 512],
                                start=(eo == 0),
                                stop=(eo == EO - 1),
                            )
                            if eo == EO - 1:
                                if th == 0:
                                    nc.scalar.activation(
                                        dst[:, 0:512],
                                        state[th][:],
                                        mybir.ActivationFunctionType.Identity,
                                        bias=gbias_sb[:, qk, pair : pair + 1],
                                    )
                                else:
                                    nc.vector.tensor_scalar_add(
                                        dst[:, 512:1024],
                                        state[th][:],
                                        gbias_sb[:, qk, pair : pair + 1],
                                    )
                        return step

                    for th in range(2):
                        for eo in range(EO):
                            steps.append(mk(qk, wt, dst, state, th, eo))
                return outs, steps

            def vproj_steps(pair, pool, tag):
                steps = []
                state = {}
                ocols = slice(pair * 128, (pair + 1) * 128)

                def mk(sb, eo):
                    def step():
                        if eo == 0:
                            state[sb] = pool.tile(
                                [128, 512], F32, tag=tag, name=f"pv{pair}_{sb}"
                            )
                        nc.tensor.matmul(
                            state[sb][:, 0:128],
                            hsT16_sb[:, eo, sb * 128 : (sb + 1) * 128],
                            wv_sb[:, eo, ocols],
                            start=(eo == 0),
                            stop=(eo == EO - 1),
                        )
                        if eo == EO - 1:
                            pv_r = state[sb][:, 0:128].rearrange(
                                "p (hh dd) -> p hh dd", dd=64
                            )
                            vsl = v16_sb[:, sb, 2 * pair : 2 * pair + 2, 0:64]
                            nc.scalar.copy(vsl, pv_r)
                            nc.gpsimd.tensor_copy(
                                v8_sb[:, sb, 2 * pair : 2 * pair + 2, 0:64], vsl
                            )
                    return step

                for sb in range(SO):
                    for eo in range(EO):
                        steps.append(mk(sb, eo))
                return steps

            def rdr_proj_steps(quad, pp2_pool):
                """Reader q/k projection for a head quad (fp8 DoubleRow).
                Outputs supertiles [128(=4h x 32d), 2(d-half), T] fp8."""
                steps = []
                outs = {}
                wt4 = w8p.tile([128, 2, 2, EO, 128], F8, tag="w8")
                nc.sync.dma_start(wt4[:], w8[quad])
                for qk in range(2):
                    dst = qk8p.tile([128, 2, T], F8, tag=f"qk8_{qk}")
                    outs[qk] = dst
                    for ab in range(2):
                        wt = wt4[:, qk, ab]
                        state = {}

                        def mk(qk, ab, wt, dst, state, th, a):
                            def step():
                                if a == 0:
                                    state[th] = pp2_pool.tile(
                                        [128, 512], F32, tag="sm",
                                        name=f"rp{quad}{qk}{ab}{th}",
                                    )
                                nc.tensor.matmul(
                                    state[th][:],
                                    wt[:, 2 * a : 2 * a + 2, :],
                                    hs8_sb[:, 2 * a : 2 * a + 2,
                                           th * 512 : (th + 1) * 512],
                                    start=(a == 0),
                                    stop=(a == 3),
                                    perf_mode=DRM,
                                )
                                if a == 3:
                                    if th == 0:
                                        nc.vector.tensor_scalar_add(
                                            dst[:, ab, 0:512],
                                            state[th][:],
                                            rbias_sb[:, qk, quad, ab : ab + 1],
                                        )
                                    else:
                                        nc.scalar.activation(
                                            dst[:, ab, 512:1024],
                                            state[th][:],
                                            mybir.ActivationFunctionType.Identity,
                                            bias=rbias_sb[:, qk, quad, ab : ab + 1],
                                        )
                            return step

                        for th in range(2):
                            for a in range(4):
                                steps.append(mk(qk, ab, wt, dst, state, th, a))
                return outs, steps

            # ---------------- prologue: v proj + first projections ----------
            pump = []
            with tc.tile_pool(
                name="vps", bufs=2, space="PSUM"
            ) as vps, tc.tile_pool(name="pps", bufs=2, space="PSUM") as pps:
                # weight DMAs first: tiny vs the 5MB input stream, and the
                # prologue proj pump stalls without them
                qk0, steps_g0 = gen_proj_steps(0, pps)
                r0, steps_r0 = rdr_proj_steps(0, pps)
                for a, b in ((0, 1), (1, 2), (2, 4), (4, 6), (6, 8)):
                    nc.sync.dma_start(hsT16_sb[:, a:b], hsT16_r[:, a:b])
                    nc.sync.dma_start(wv_sb[:, a:b], wv[:, a:b])
                    if a == 2:
                        nc.sync.dma_start(gbias_sb[:], gbias[:])
                        nc.sync.dma_start(rbias_sb[:], rbias[:])
                        nc.sync.dma_start(bo_sb[:], bo[:])
                for a, b in ((0, 4), (4, 8)):
                    nc.sync.dma_start(hs8_sb[:, a:b], hsT8_r[:, a:b])
                pro_pump = steps_g0 + steps_r0

                for sb in range(SO):
                    pv = vps.tile([128, E], F32, tag="pv", name=f"pv{sb}")
                    for eo in range(EO):
                        for vh, w0, w1 in ((0, 0, 512), (1, 512, 768)):
                            nc.tensor.matmul(
                                pv[:, w0:w1],
                                hsT16_sb[:, eo, sb * 128 : (sb + 1) * 128],
                                wv_sb[:, eo, w0:w1],
                                start=(eo == 0),
                                stop=(eo == EO - 1),
                            )
                        for _ in range(2 if sb >= 4 else 1):
                            if pro_pump:
                                pro_pump.pop(0)()
                    pv_r = pv[:, 0:768].rearrange("p (hh dd) -> p hh dd", dd=64)
                    nc.scalar.copy(v16_sb[:, sb, 0:12, 0:64], pv_r)
                    nc.vector.tensor_copy(v8_sb[:, sb, 0:12, 0:64], pv_r)
                while pro_pump:
                    pro_pump.pop(0)()

            # ---------------- main attention loop ---------------------------
            # Software pipeline: the AV/combine work of head h-1 is emitted
            # interleaved into the scores/exp loop of head h, so the PE has
            # filler work while ACT/DVE drain the score tiles.
            wo_sb = const.tile([128, TB, NP, 128], F16, tag="wo_all")
            for j in range(TB):
                nc.sync.dma_start(wo_sb[:, j], wo[j])

            with ExitStack() as mstack:
                scp = mstack.enter_context(
                    tc.tile_pool(name="scp", bufs=2, space="PSUM")
                )
                smallp = mstack.enter_context(
                    tc.tile_pool(name="smallp", bufs=4, space="PSUM")
                )
                ex16p = mstack.enter_context(tc.tile_pool(name="ex16", bufs=2))
                ex8p = mstack.enter_context(tc.tile_pool(name="ex8", bufs=2))
                tmpp = mstack.enter_context(tc.tile_pool(name="tmpp", bufs=8))
                avsp = mstack.enter_context(tc.tile_pool(name="avsp", bufs=6))

                def av_steps(h, ex16, ex8):
                    """AV + combine for head h as a list of step closures.
                    Each tb yields 3 steps: gen-av mms, rdr-av mms, combine."""
                    pair, hp = h // 2, h % 2
                    vg = v16_sb[:, :, h, :]
                    v8h = v8_sb[:, :, h, :]
                    steps = []
                    state = {}

                    def mk_gen(tb):
                        def step():
                            tsl = slice(tb * 128, (tb + 1) * 128)
                            av = smallp.tile([128, 512], F32, tag="sm",
                                             name=f"av{h}_{tb}")
                            state[tb] = av
                            for a in range(SO):
                                nc.tensor.matmul(
                                    av[:, 0:65],
                                    ex16[:, a, tsl],
                                    vg[:, a, 0:65],
                                    start=(a == 0),
                                    stop=(a == SO - 1),
                                )
                        return step

                    def mk_rdr(tb):
                        def step():
                            tsl = slice(tb * 128, (tb + 1) * 128)
                            av = state[tb]
                            for a in range(4):
                                nc.tensor.matmul(
                                    av[:, 68:133],
                                    ex8[:, 2 * a : 2 * a + 2, tsl],
                                    v8h[:, 2 * a : 2 * a + 2, 0:65],
                                    start=(a == 0),
                                    stop=(a == 3),
                                    perf_mode=DRM,
                                )
                        return step

                    def mk_comb(tb):
                        def step():
                            av = state.pop(tb)
                            avs = avsp.tile([128, 133], F32, tag="avs")
                            nc.scalar.copy(avs[:], av[:, 0:133])
                            csl = slice(h * 64, h * 64 + 64)
                            tmp = tmpp.tile([128, 64], F16, tag="tmp")
                            nc.gpsimd.normalize_recip(
                                comb_tiles[tb][:, csl], avs[:, 0:64],
                                avs[:, 64:65],
                            )
                            nc.gpsimd.normalize_recip(
                                tmp[:], avs[:, 68:132], avs[:, 132:133],
                            )
                            nc.gpsimd.tensor_tensor(
                                comb_tiles[tb][:, csl],
                                comb_tiles[tb][:, csl],
                                tmp[:],
                                ADD,
                            )
                            if hp == 1:
                                nc.sync.dma_start_transpose(
                                    attnT_tiles[pair][
                                        :, tb * 128 : (tb + 1) * 128
                                    ],
                                    comb_tiles[tb][
                                        :, pair * 128 : (pair + 1) * 128
                                    ],
                                )
                        return step

                    for tb in range(TB):
                        steps.append(mk_gen(tb))
                        steps.append(mk_rdr(tb))
                        steps.append(mk_comb(tb))
                    return steps

                Qg = Kg = Q8 = K8 = None
                av_q = []  # pending av steps of the previous head

                def fill(n):
                    """Emit up to n units of filler: av steps take priority
                    (they unblock comb tiles), then proj pump steps."""
                    for _ in range(n):
                        if av_q:
                            av_q.pop(0)()
                        elif pump:
                            pump.pop(0)()

                for h in range(H):
                    pair, quad = h // 2, h // 4
                    hp, hq = h % 2, h % 4
                    if h == 0:
                        Qg, Kg = qk0[0], qk0[1]
                        Q8, K8 = r0[0], r0[1]
                        nxt_g = nxt_r = None
                    if h == 9:
                        pump.extend(vproj_steps(6, smallp, "sm"))
                    if h == 11:
                        pump.extend(vproj_steps(7, smallp, "sm"))
                    if hp == 0 and pair + 1 < NP:
                        nxt_g, s = gen_proj_steps(pair + 1, smallp)
                        pump.extend(s)
                    if hq == 0 and quad + 1 < NQ:
                        nxt_r, s = rdr_proj_steps(quad + 1, smallp)
                        pump.extend(s)

                    ex16 = ex16p.tile([128, SO, T], F16, tag="ex16")
                    ex8 = ex8p.tile([128, SO, T], F8, tag="ex8")

                    grow = slice(64 * hp, 64 * hp + 64)
                    rrow = slice(32 * hq, 32 * hq + 32)

                    for sb in range(SO):
                        ssl = slice(sb * 128, (sb + 1) * 128)
                        sc = scp.tile([128, T], F32, tag="sc", name=f"sc{h}_{sb}")
                        for th in range(2):
                            nc.tensor.matmul(
                                sc[:, th * 512 : (th + 1) * 512],
                                Kg[grow, ssl],
                                Qg[grow, th * 512 : (th + 1) * 512],
                                start=True, stop=True,
                            )
                        nc.scalar.activation(
                            ex16[:, sb, :], sc[:], EXP, scale=SCALING
                        )
                        fill(2)
                        for th in range(2):
                            rc = smallp.tile([128, 512], F32, tag="sm",
                                             name=f"rc{h}_{sb}_{th}")
                            if USE_RDR_SCORE_DR:
                                nc.tensor.matmul(
                                    rc[:],
                                    K8[rrow, :, ssl],
                                    Q8[rrow, :, th * 512 : (th + 1) * 512],
                                    start=True, stop=True,
                                    perf_mode=DRM,
                                    tile_position=(32 * hq, 0),
                                )
                            else:
                                for ab in range(2):
                                    nc.tensor.matmul(
                                        rc[:],
                                        K8[rrow, ab, ssl],
                                        Q8[rrow, ab, th * 512 : (th + 1) * 512],
                                        start=(ab == 0), stop=(ab == 1),
                                        tile_position=(32 * hq, 0),
                                    )
                            nc.vector.tensor_scalar(
                                ex8[:, sb, th * 512 : (th + 1) * 512].bitcast(U8),
                                rc[:],
                                SCH_A, SCH_B, MULT, ADD,
                            )
                            fill(1)
                        fill(1)

                    # queue this head's av work; emitted during later heads'
                    # scores loops via fill()
                    av_q.extend(av_steps(h, ex16, ex8))

                    if hp == 1:
                        while pump:
                            pump.pop(0)()
                        if nxt_g is not None:
                            Qg, Kg = nxt_g[0], nxt_g[1]
                            nxt_g = None
                        if hq == 3 and nxt_r is not None:
                            Q8, K8 = nxt_r[0], nxt_r[1]
                            nxt_r = None

                while av_q:
                    av_q.pop(0)()

            # ---------------- output projection -----------------------------
            with tc.tile_pool(name="ops", bufs=4, space="PSUM") as ops, \
                 tc.tile_pool(name="o16p", bufs=4) as o16p:
                for j in range(TB):
                    for th in range(2):
                        tsl = slice(th * 512, (th + 1) * 512)
                        po = ops.tile([128, 512], F32, tag="po",
                                      name=f"po{j}_{th}")
                        for pr in range(NP):
                            nc.tensor.matmul(
                                po[:],
                                wo_sb[:, j, pr, :],
                                attnT_tiles[pr][:, tsl],
                                start=(pr == 0),
                                stop=(pr == NP - 1),
                            )
                        o16 = o16p.tile([128, 512], F16, tag="o16")
                        if th == 0:
                            nc.vector.tensor_scalar_add(
                                o16[:], po[:], bo_sb[:, j : j + 1]
                            )
                        else:
                            nc.scalar.activation(
                                o16[:], po[:],
                                mybir.ActivationFunctionType.Identity,
                                bias=bo_sb[:, j : j + 1],
                            )
                        nc.sync.dma_start(
                            outT[j * 128 : (j + 1) * 128, tsl], o16[:]
                        )

    nc.finalize()
    return nc


_NC_CACHE = {}


def get_nc():
    if "nc" not in _NC_CACHE:
        _NC_CACHE["nc"] = build_nc()
    return _NC_CACHE["nc"]


def _host_prep(hidden_states, reader_token, Wq, bq, Wk, bk, Wv, bv, Wo, bo,
               RWq, Rbq, RWk, Rbk, RWv, Rbv):
    f = np.float32
    np16 = mybir.dt.np(F16)
    np8 = mybir.dt.np(F8)
    hs = np.asarray(hidden_states, f)
    tok = np.asarray(reader_token).astype(np.int64)

    WqT = np.asarray(Wq, f).T  # [e, o]
    WkT = np.asarray(Wk, f).T
    WvT = np.asarray(Wv, f).T
    WoT = np.asarray(Wo, f).T
    bq = np.asarray(bq, f); bk = np.asarray(bk, f)
    bv = np.asarray(bv, f); bo_ = np.asarray(bo, f)
    Rbq = np.asarray(Rbq, f); Rbk = np.asarray(Rbk, f)

    # gen weights [NP, 128, 2, EO, 128]
    wg_arr = np.empty((NP, 128, 2, EO, 128), np16)
    for qk, WT in enumerate((WqT, WkT)):
        r = WT.reshape(EO, 128, NP, 128)  # (eo, p, pair, m)
        wg_arr[:, :, qk] = r.transpose(2, 1, 0, 3).astype(np16)
    # gen biases [128, 2, NP]
    gb = np.empty((128, 2, NP), f)
    for qk, bb in enumerate((bq, bk)):
        gb[:, qk, :] = bb.reshape(NP, 128).T

    # v-bias folds into output bias (probs rows sum to 0.5)
    bo_eff = bo_ + 0.5 * (np.asarray(Wo, f) @ bv)
    bo_t = np.ascontiguousarray(bo_eff.reshape(TB, 128).T)

    # wv [128, EO, E]
    wv_arr = np.ascontiguousarray(
        WvT.reshape(EO, 128, E).transpose(1, 0, 2)
    ).astype(np16)
    # wo [TB, 128, NP, 128]
    wo_arr = np.ascontiguousarray(
        WoT.reshape(NP, 128, TB, 128).transpose(2, 1, 0, 3)
    ).astype(np16)

    percore = {}
    in_maps = []
    for b in range(B):
        g = int(tok[b])
        if g not in percore:
            RWqT = np.asarray(RWq[g], f).T  # [e, o]
            RWkT = np.asarray(RWk[g], f).T
            w8_arr = np.empty((NQ, 128, 2, 2, EO, 128), np8)
            for qk, WT in enumerate((RWqT, RWkT)):
                # o = (quad*4 + m//32)*64 + ab*32 + m%32
                r = WT.reshape(EO, 128, NQ, 4, 2, 32)  # (eo,p,quad,hin,ab,dd)
                # -> (quad, p, qk, ab, eo, m=(hin,dd))
                w8_arr[:, :, qk] = r.transpose(2, 4, 1, 0, 3, 5).reshape(
                    NQ, 2, 128, EO, 128
                ).astype(np8).transpose(0, 2, 1, 3, 4).reshape(NQ, 128, 2, EO, 128)
            rb = np.empty((128, 2, NQ, 2), f)
            for qk, bb in enumerate((Rbq[g], Rbk[g])):
                # p = hin*32 + dd ; value = b[(quad*4+hin)*64 + ab*32 + dd]
                r = bb.reshape(NQ, 4, 2, 32)  # (quad, hin, ab, dd)
                rb[:, qk, :, :] = r.transpose(1, 3, 0, 2).reshape(128, NQ, 2)
            percore[g] = (w8_arr, rb)
        w8_arr, rb = percore[g]
        hsT = np.ascontiguousarray(hs[b].T)
        in_maps.append(
            {
                "hsT16": hsT.astype(np16),
                "hsT8": hsT.astype(np8),
                "wg": wg_arr,
                "w8": w8_arr,
                "wv": wv_arr,
                "wo": wo_arr,
                "gbias": gb,
                "rbias": rb,
                "bo": bo_t,
            }
        )
    return in_maps


def kernel(**inputs) -> np.ndarray:
    in_maps = _host_prep(**inputs)
    nc = get_nc()
    res = run_bass_kernel_spmd(nc, in_maps, list(range(B)))
    out = np.stack(
        [np.asarray(res.results[c]["outT"]).astype(np.float32).T for c in range(B)],
        axis=0,
    )
    return np.ascontiguousarray(out)


# revision 5
# speedup vs baseline: 1.1027x; 1.0010x over previous
"""AuthorGroupAttention Trainium2 kernel.

Data-parallel over batch: 8 samples -> 8 NeuronCores, one sample per core.
Routing resolved on host (per-core reader-group weights gathered, cast and
laid out per-engine-friendly in _host_prep).

Precision (validated vs reference: rel err ~5e-3 against the 2e-2 gate):
  - generic path: fp16 matmul operands everywhere, fp32 PSUM accumulation
  - reader path (weight 0.1 in the prob combine): fp8e4 operands with
    DoubleRow matmuls (0.5 cyc/row); its exp is a Schraudolph bit-trick on
    DVE (scores*1/ln2 + 55.2 rounded to uint8 = e4m3 bits of exp(scores/8)),
    consistent numerator/denominator so the approximation bias cancels in
    the softmax normalization.

Structure:
  - q/k produced in [d, t] layout (d on partitions): per head-pair (gen,
    fp16) / head-quad (rdr, fp8 DR with 32-row subtiles at tile_position)
    projection chains, interleaved into the attention loop as PE filler
    ("pump") with per-boundary forced drains.
  - scores per (head, s-block): gen [128,1024] psum tiles (2-deep pool),
    rdr th-split [128,512] tiles in a shared 4-deep "small" pool that also
    carries av accumulators, projection chains, and deferred v-proj.
  - attention transposed: stationary = exp tile [s,t], moving = [v | 1/w]
    so attn lands [t, d] with the softmax denominator Z/w in the extra
    column; one ACT copy bounces the accumulator to SBUF, gpsimd
    normalize_recip applies w/Z per path, gpsimd add combines gen+rdr.
  - attn [t, e] tiles are DMA-transposed (xbar) per (pair, t-block) into
    [e, t] for the fp16 output projection; v-bias is folded into the output
    bias on host (combined prob rows sum to 0.5).
  - AV/combine of head h-1 is software-pipelined into head h's score loop;
    v-projection for pairs 6-7 is deferred into the pump as late filler.
"""

import os
import sys

for _p in ("/opt/trn_rl_repo",):
    if os.path.isdir(_p) and _p not in sys.path:
        sys.path.insert(0, _p)

import numpy as np

import concourse.bass as bass
import concourse.mybir as mybir
from concourse import bacc
from concourse.tile import TileContext
from concourse.bass_utils import run_bass_kernel_spmd

B, T, E, H, G = 8, 1024, 1024, 16, 4
D = E // H  # 64
SCALING = float(D) ** -0.5
W_G = 0.9 / 2.0
W_R = 0.1 / 2.0
EO = 8
SO = 8
TB = 8
NP = 8  # head pairs
NQ = 4  # head quads

F32 = mybir.dt.float32
F16 = mybir.dt.float16
F8 = mybir.dt.float8e4
U8 = mybir.dt.uint8
DRM = mybir.MatmulPerfMode.DoubleRow
EXP = mybir.ActivationFunctionType.Exp
MULT = mybir.AluOpType.mult
ADD = mybir.AluOpType.add

# Schraudolph constants for e4m3 bits: bits = round(score*A + Bc)
SCH_A = SCALING * 8.0 / float(np.log(2.0))
SCH_B = 56.0 - 0.8

USE_RDR_SCORE_DR = True  # DoubleRow with 32-row subtiles for reader scores


def build_nc():
    nc = bacc.Bacc(name="author_group_attention_v2")

    hsT16 = nc.dram_tensor("hsT16", [E, T], F16, kind="ExternalInput")
    hsT8 = nc.dram_tensor("hsT8", [E, T], F8, kind="ExternalInput")
    wg = nc.dram_tensor("wg", [NP, 128, 2, EO, 128], F16, kind="ExternalInput")
    w8 = nc.dram_tensor("w8", [NQ, 128, 2, 2, EO, 128], F8, kind="ExternalInput")
    wv = nc.dram_tensor("wv", [128, EO, E], F16, kind="ExternalInput")
    wo = nc.dram_tensor("wo", [TB, 128, NP, 128], F16, kind="ExternalInput")
    gbias = nc.dram_tensor("gbias", [128, 2, NP], F32, kind="ExternalInput")
    rbias = nc.dram_tensor("rbias", [128, 2, NQ, 2], F32, kind="ExternalInput")
    bo = nc.dram_tensor("bo", [128, TB], F32, kind="ExternalInput")
    outT = nc.dram_tensor("outT", [E, T], F16, kind="ExternalOutput")

    hsT16_r = hsT16.rearrange("(eo p) t -> p eo t", p=128)
    hsT8_r = hsT8.rearrange("(eo p) t -> p eo t", p=128)


    with TileContext(nc) as tc:
        from contextlib import ExitStack

        with ExitStack() as stack:
            const = stack.enter_context(tc.tile_pool(name="const", bufs=1))

            hsT16_sb = const.tile([128, EO, T], F16, tag="hsT16")
            hs8_sb = const.tile([128, EO, T], F8, tag="hs8")
            v16_sb = const.tile([128, SO, H, 66], F16, tag="v16")
            wv_sb = const.tile([128, EO, E], F16, tag="wv")
            v8_sb = const.tile([128, SO, H, 66], F8, tag="v8")
            gbias_sb = const.tile([128, 2, NP], F32, tag="gbias")
            rbias_sb = const.tile([128, 2, NQ, 2], F32, tag="rbias")
            bo_sb = const.tile([128, TB], F32, tag="bo")
            comb_tiles = [
                const.tile([128, E], F16, tag=f"comb{tb}", name=f"comb{tb}")
                for tb in range(TB)
            ]
            attnT_tiles = [
                const.tile([128, T], F16, tag=f"attnT{p}", name=f"attnT{p}")
                for p in range(NP)
            ]

            # ones columns pre-scaled by 1/w so reciprocal gives w/Z
            nc.vector.memset(v16_sb[:, :, :, 64:65], 1.0 / W_G)
            nc.vector.memset(v8_sb[:, :, :, 64:65], 1.0 / W_R)


            # persistent pools used across prologue + main
            qk16p = stack.enter_context(tc.tile_pool(name="qk16", bufs=2))
            qk8p = stack.enter_context(tc.tile_pool(name="qk8", bufs=2))
            wgp = stack.enter_context(tc.tile_pool(name="wgp", bufs=2))
            w8p = stack.enter_context(tc.tile_pool(name="w8p", bufs=2))

            def gen_proj_steps(pair, pp2_pool):
                """Generic q/k projection for a head pair -> list of step
                closures (PE matmuls + ACT drains). Weight DMAs fire now."""
                steps = []
                outs = {}
                wt2 = wgp.tile([128, 2, EO, 128], F16, tag="wg")
                nc.sync.dma_start(wt2[:], wg[pair])
                for qk in range(2):
                    wt = wt2[:, qk]
                    dst = qk16p.tile([128, T], F16, tag=f"qk16_{qk}")
                    outs[qk] = dst
                    state = {}

                    def mk(qk, wt, dst, state, th, eo):
                        def step():
                            if eo == 0:
                                state[th] = pp2_pool.tile(
                                    [128, 512], F32, tag="sm", name=f"gp{pair}{qk}{th}"
                                )
                            nc.tensor.matmul(
                                state[th][:],
                                wt[:, eo, :],
                                hsT16_sb[:, eo, th * 512 : (th + 1) * 512],
                                start=(eo == 0),
                                stop=(eo == EO - 1),
                            )
                            if eo == EO - 1:
                                if th == 0:
                                    nc.scalar.activation(
                                        dst[:, 0:512],
                                        state[th][:],
                                        mybir.ActivationFunctionType.Identity,
                                        bias=gbias_sb[:, qk, pair : pair + 1],
                                    )
                                else:
                                    nc.vector.tensor_scalar_add(
                                        dst[:, 512:1024],
                                        state[th][:],
                                        gbias_sb[:, qk, pair : pair + 1],
                                    )
                        return step

                    for th in range(2):
                        for eo in range(EO):
                            steps.append(mk(qk, wt, dst, state, th, eo))
                return outs, steps

            def vproj_steps(pair, pool, tag):
                steps = []
                state = {}
                ocols = slice(pair * 128, (pair + 1) * 128)

                def mk(sb, eo):
                    def step():
                        if eo == 0:
                            state[sb] = pool.tile(
                                [128, 512], F32, tag=tag, name=f"pv{pair}_{sb}"
                            )
                        nc.tensor.matmul(
                            state[sb][:, 0:128],
                            hsT16_sb[:, eo, sb * 128 : (sb + 1) * 128],
                            wv_sb[:, eo, ocols],
                            start=(eo == 0),
                            stop=(eo == EO - 1),
                        )
                        if eo == EO - 1:
                            pv_r = state[sb][:, 0:128].rearrange(
                                "p (hh dd) -> p hh dd", dd=64
                            )
                            vsl = v16_sb[:, sb, 2 * pair : 2 * pair + 2, 0:64]
                            nc.vector.tensor_copy(vsl, pv_r)
                            nc.gpsimd.tensor_copy(
                                v8_sb[:, sb, 2 * pair : 2 * pair + 2, 0:64], vsl
                            )
                    return step

                for sb in range(SO):
                    for eo in range(EO):
                        steps.append(mk(sb, eo))
                return steps

            def rdr_proj_steps(quad, pp2_pool):
                """Reader q/k projection for a head quad (fp8 DoubleRow).
                Outputs supertiles [128(=4h x 32d), 2(d-half), T] fp8."""
                steps = []
                outs = {}
                wt4 = w8p.tile([128, 2, 2, EO, 128], F8, tag="w8")
                nc.sync.dma_start(wt4[:], w8[quad])
                for qk in range(2):
                    dst = qk8p.tile([128, 2, T], F8, tag=f"qk8_{qk}")
                    outs[qk] = dst
                    for ab in range(2):
                        wt = wt4[:, qk, ab]
                        state = {}

                        def mk(qk, ab, wt, dst, state, th, a):
                            def step():
                                if a == 0:
                                    state[th] = pp2_pool.tile(
                                        [128, 512], F32, tag="sm",
                                        name=f"rp{quad}{qk}{ab}{th}",
                                    )
                                nc.tensor.matmul(
                                    state[th][:],
                                    wt[:, 2 * a : 2 * a + 2, :],
                                    hs8_sb[:, 2 * a : 2 * a + 2,
                                           th * 512 : (th + 1) * 512],
                                    start=(a == 0),
                                    stop=(a == 3),
                                    perf_mode=DRM,
                                )
                                if a == 3:
                                    if th == 0:
                                        nc.vector.tensor_scalar_add(
                                            dst[:, ab, 0:512],
                                            state[th][:],
                                            rbias_sb[:, qk, quad, ab : ab + 1],
                                        )
                                    else:
                                        nc.scalar.activation(
                                            dst[:, ab, 512:1024],
                                            state[th][:],
                                            mybir.ActivationFunctionType.Identity,
                                            bias=rbias_sb[:, qk, quad, ab : ab + 1],
                                        )
                            return step

                        for th in range(2):
                            for a in range(4):
                                steps.append(mk(qk, ab, wt, dst, state, th, a))
                return outs, steps

            # ---------------- prologue: v proj + first projections ----------
            pump = []
            with tc.tile_pool(
                name="vps", bufs=2, space="PSUM"
            ) as vps, tc.tile_pool(name="pps", bufs=2, space="PSUM") as pps:
                # weight DMAs first: tiny vs the 5MB input stream, and the
                # prologue proj pump stalls without them
                qk0, steps_g0 = gen_proj_steps(0, pps)
                r0, steps_r0 = rdr_proj_steps(0, pps)
                for a, b in ((0, 1), (1, 2), (2, 4), (4, 6), (6, 8)):
                    nc.sync.dma_start(hsT16_sb[:, a:b], hsT16_r[:, a:b])
                    nc.sync.dma_start(wv_sb[:, a:b], wv[:, a:b])
                    if a == 2:
                        nc.sync.dma_start(gbias_sb[:], gbias[:])
                        nc.sync.dma_start(rbias_sb[:], rbias[:])
                        nc.sync.dma_start(bo_sb[:], bo[:])
                for a, b in ((0, 4), (4, 8)):
                    nc.sync.dma_start(hs8_sb[:, a:b], hsT8_r[:, a:b])
                pro_pump = steps_g0 + steps_r0

                for sb in range(SO):
                    pv = vps.tile([128, E], F32, tag="pv", name=f"pv{sb}")
                    for eo in range(EO):
                        for vh, w0, w1 in ((0, 0, 512), (1, 512, 768)):
                            nc.tensor.matmul(
                                pv[:, w0:w1],
                                hsT16_sb[:, eo, sb * 128 : (sb + 1) * 128],
                                wv_sb[:, eo, w0:w1],
                                start=(eo == 0),
                                stop=(eo == EO - 1),
                            )
                        for _ in range(2 if sb >= 4 else 1):
                            if pro_pump:
                                pro_pump.pop(0)()
                    pv_r = pv[:, 0:768].rearrange("p (hh dd) -> p hh dd", dd=64)
                    nc.scalar.copy(v16_sb[:, sb, 0:12, 0:64], pv_r)
                    nc.vector.tensor_copy(v8_sb[:, sb, 0:12, 0:64], pv_r)
                while pro_pump:
                    pro_pump.pop(0)()

            # ---------------- main attention loop ---------------------------
            # Software pipeline: the AV/combine work of head h-1 is emitted
            # interleaved into the scores/exp loop of head h, so the PE has
            # filler work while ACT/DVE drain the score tiles.
            wo_sb = const.tile([128, TB, NP, 128], F16, tag="wo_all")
            for j in range(TB):
                nc.sync.dma_start(wo_sb[:, j], wo[j])

            with ExitStack() as mstack:
                scp = mstack.enter_context(
                    tc.tile_pool(name="scp", bufs=2, space="PSUM")
                )
                smallp = mstack.enter_context(
                    tc.tile_pool(name="smallp", bufs=4, space="PSUM")
                )
                ex16p = mstack.enter_context(tc.tile_pool(name="ex16", bufs=2))
                ex8p = mstack.enter_context(tc.tile_pool(name="ex8", bufs=2))
                tmpp = mstack.enter_context(tc.tile_pool(name="tmpp", bufs=8))
                avsp = mstack.enter_context(tc.tile_pool(name="avsp", bufs=6))

                def av_steps(h, ex16, ex8):
                    """AV + combine for head h as a list of step closures.
                    Each tb yields 3 steps: gen-av mms, rdr-av mms, combine."""
                    pair, hp = h // 2, h % 2
                    vg = v16_sb[:, :, h, :]
                    v8h = v8_sb[:, :, h, :]
                    steps = []
                    state = {}

                    def mk_gen(tb):
                        def step():
                            tsl = slice(tb * 128, (tb + 1) * 128)
                            av = smallp.tile([128, 512], F32, tag="sm",
                                             name=f"av{h}_{tb}")
                            state[tb] = av
                            for a in range(SO):
                                nc.tensor.matmul(
                                    av[:, 0:65],
                                    ex16[:, a, tsl],
                                    vg[:, a, 0:65],
                                    start=(a == 0),
                                    stop=(a == SO - 1),
                                )
                        return step

                    def mk_rdr(tb):
                        def step():
                            tsl = slice(tb * 128, (tb + 1) * 128)
                            av = state[tb]
                            for a in range(4):
                                nc.tensor.matmul(
                                    av[:, 68:133],
                                    ex8[:, 2 * a : 2 * a + 2, tsl],
                                    v8h[:, 2 * a : 2 * a + 2, 0:65],
                                    start=(a == 0),
                                    stop=(a == 3),
                                    perf_mode=DRM,
                                )
                        return step

                    def mk_comb(tb):
                        def step():
                            av = state.pop(tb)
                            avs = avsp.tile([128, 133], F32, tag="avs")
                            nc.scalar.copy(avs[:], av[:, 0:133])
                            csl = slice(h * 64, h * 64 + 64)
                            tmp = tmpp.tile([128, 64], F16, tag="tmp")
                            nc.gpsimd.normalize_recip(
                                comb_tiles[tb][:, csl], avs[:, 0:64],
                                avs[:, 64:65],
                            )
                            nc.gpsimd.normalize_recip(
                                tmp[:], avs[:, 68:132], avs[:, 132:133],
                            )
                            nc.gpsimd.tensor_tensor(
                                comb_tiles[tb][:, csl],
                                comb_tiles[tb][:, csl],
                                tmp[:],
                                ADD,
                            )
                            if hp == 1:
                                nc.sync.dma_start_transpose(
                                    attnT_tiles[pair][
                                        :, tb * 128 : (tb + 1) * 128
                                    ],
                                    comb_tiles[tb][
                                        :, pair * 128 : (pair + 1) * 128
                                    ],
                                )
                        return step

                    for tb in range(TB):
                        steps.append(mk_gen(tb))
                        steps.append(mk_rdr(tb))
                        steps.append(mk_comb(tb))
                    return steps

                Qg = Kg = Q8 = K8 = None
                av_q = []  # pending av steps of the previous head

                def fill(n):
                    """Emit up to n units of filler: av steps take priority
                    (they unblock comb tiles), then proj pump steps."""
                    for _ in range(n):
                        if av_q:
                            av_q.pop(0)()
                        elif pump:
                            pump.pop(0)()

                for h in range(H):
                    pair, quad = h // 2, h // 4
                    hp, hq = h % 2, h % 4
                    if h == 0:
                        Qg, Kg = qk0[0], qk0[1]
                        Q8, K8 = r0[0], r0[1]
                        nxt_g = nxt_r = None
                    if h == 9:
                        pump.extend(vproj_steps(6, smallp, "sm"))
                    if h == 11:
                        pump.extend(vproj_steps(7, smallp, "sm"))
                    if hp == 0 and pair + 1 < NP:
                        nxt_g, s = gen_proj_steps(pair + 1, smallp)
                        pump.extend(s)
                    if hq == 0 and quad + 1 < NQ:
                        nxt_r, s = rdr_proj_steps(quad + 1, smallp)
                        pump.extend(s)

                    ex16 = ex16p.tile([128, SO, T], F16, tag="ex16")
                    ex8 = ex8p.tile([128, SO, T], F8, tag="ex8")

                    grow = slice(64 * hp, 64 * hp + 64)
                    rrow = slice(32 * hq, 32 * hq + 32)

                    for sb in range(SO):
                        ssl = slice(sb * 128, (sb + 1) * 128)
                        sc = scp.tile([128, T], F32, tag="sc", name=f"sc{h}_{sb}")
                        for th in range(2):
                            nc.tensor.matmul(
                                sc[:, th * 512 : (th + 1) * 512],
                                Kg[grow, ssl],
                                Qg[grow, th * 512 : (th + 1) * 512],
                                start=True, stop=True,
                            )
                        nc.scalar.activation(
                            ex16[:, sb, :], sc[:], EXP, scale=SCALING
                        )
                        fill(2)
                        for th in range(2):
                            rc = smallp.tile([128, 512], F32, tag="sm",
                                             name=f"rc{h}_{sb}_{th}")
                            if USE_RDR_SCORE_DR:
                                nc.tensor.matmul(
                                    rc[:],
                                    K8[rrow, :, ssl],
                                    Q8[rrow, :, th * 512 : (th + 1) * 512],
                                    start=True, stop=True,
                                    perf_mode=DRM,
                                    tile_position=(32 * hq, 0),
                                )
                            else:
                                for ab in range(2):
                                    nc.tensor.matmul(
                                        rc[:],
                                        K8[rrow, ab, ssl],
                                        Q8[rrow, ab, th * 512 : (th + 1) * 512],
                                        start=(ab == 0), stop=(ab == 1),
                                        tile_position=(32 * hq, 0),
                                    )
                            nc.vector.tensor_scalar(
                                ex8[:, sb, th * 512 : (th + 1) * 512].bitcast(U8),
                                rc[:],
                                SCH_A, SCH_B, MULT, ADD,
                            )
                            fill(1)
                        fill(1)

                    # queue this head's av work; emitted during later heads'
                    # scores loops via fill()
                    av_q.extend(av_steps(h, ex16, ex8))

                    if hp == 1:
                        while pump:
                            pump.pop(0)()
                        if nxt_g is not None:
                            Qg, Kg = nxt_g[0], nxt_g[1]
                            nxt_g = None
                        if hq == 3 and nxt_r is not None:
                            Q8, K8 = nxt_r[0], nxt_r[1]
                            nxt_r = None

                while av_q:
                    av_q.pop(0)()

            # ---------------- output projection -----------------------------
            with tc.tile_pool(name="ops", bufs=4, space="PSUM") as ops, \
                 tc.tile_pool(name="o16p", bufs=4) as o16p:
                for j in range(TB):
                    for th in range(2):
                        tsl = slice(th * 512, (th + 1) * 512)
                        po = ops.tile([128, 512], F32, tag="po",
                                      name=f"po{j}_{th}")
                        for pr in range(NP):
                            nc.tensor.matmul(
                                po[:],
                                wo_sb[:, j, pr, :],
                                attnT_tiles[pr][:, tsl],
                                start=(pr == 0),
                                stop=(pr == NP - 1),
                            )
                        o16 = o16p.tile([128, 512], F16, tag="o16")
                        if th == 0:
                            nc.vector.tensor_scalar_add(
                                o16[:], po[:], bo_sb[:, j : j + 1]
                            )
                        else:
                            nc.scalar.activation(
                                o16[:], po[:],
                                mybir.ActivationFunctionType.Identity,
                                bias=bo_sb[:, j : j + 1],
                            )
                        nc.sync.dma_start(
                            outT[j * 128 : (j + 1) * 128, tsl], o16[:]
                        )

    nc.finalize()
    return nc


_NC_CACHE = {}


def get_nc():
    if "nc" not in _NC_CACHE:
        _NC_CACHE["nc"] = build_nc()
    return _NC_CACHE["nc"]


def _host_prep(hidden_states, reader_token, Wq, bq, Wk, bk, Wv, bv, Wo, bo,
               RWq, Rbq, RWk, Rbk, RWv, Rbv):
    f = np.float32
    np16 = mybir.dt.np(F16)
    np8 = mybir.dt.np(F8)
    hs = np.asarray(hidden_states, f)
    tok = np.asarray(reader_token).astype(np.int64)

    WqT = np.asarray(Wq, f).T  # [e, o]
    WkT = np.asarray(Wk, f).T
    WvT = np.asarray(Wv, f).T
    WoT = np.asarray(Wo, f).T
    bq = np.asarray(bq, f); bk = np.asarray(bk, f)
    bv = np.asarray(bv, f); bo_ = np.asarray(bo, f)
    Rbq = np.asarray(Rbq, f); Rbk = np.asarray(Rbk, f)

    # gen weights [NP, 128, 2, EO, 128]
    wg_arr = np.empty((NP, 128, 2, EO, 128), np16)
    for qk, WT in enumerate((WqT, WkT)):
        r = WT.reshape(EO, 128, NP, 128)  # (eo, p, pair, m)
        wg_arr[:, :, qk] = r.transpose(2, 1, 0, 3).astype(np16)
    # gen biases [128, 2, NP]
    gb = np.empty((128, 2, NP), f)
    for qk, bb in enumerate((bq, bk)):
        gb[:, qk, :] = bb.reshape(NP, 128).T

    # v-bias folds into output bias (probs rows sum to 0.5)
    bo_eff = bo_ + 0.5 * (np.asarray(Wo, f) @ bv)
    bo_t = np.ascontiguousarray(bo_eff.reshape(TB, 128).T)

    # wv [128, EO, E]
    wv_arr = np.ascontiguousarray(
        WvT.reshape(EO, 128, E).transpose(1, 0, 2)
    ).astype(np16)
    # wo [TB, 128, NP, 128]
    wo_arr = np.ascontiguousarray(
        WoT.reshape(NP, 128, TB, 128).transpose(2, 1, 0, 3)
    ).astype(np16)

    percore = {}
    in_maps = []
    for b in range(B):
        g = int(tok[b])
        if g not in percore:
            RWqT = np.asarray(RWq[g], f).T  # [e, o]
            RWkT = np.asarray(RWk[g], f).T
            w8_arr = np.empty((NQ, 128, 2, 2, EO, 128), np8)
            for qk, WT in enumerate((RWqT, RWkT)):
                # o = (quad*4 + m//32)*64 + ab*32 + m%32
                r = WT.reshape(EO, 128, NQ, 4, 2, 32)  # (eo,p,quad,hin,ab,dd)
                # -> (quad, p, qk, ab, eo, m=(hin,dd))
                w8_arr[:, :, qk] = r.transpose(2, 4, 1, 0, 3, 5).reshape(
                    NQ, 2, 128, EO, 128
                ).astype(np8).transpose(0, 2, 1, 3, 4).reshape(NQ, 128, 2, EO, 128)
            rb = np.empty((128, 2, NQ, 2), f)
            for qk, bb in enumerate((Rbq[g], Rbk[g])):
                # p = hin*32 + dd ; value = b[(quad*4+hin)*64 + ab*32 + dd]
                r = bb.reshape(NQ, 4, 2, 32)  # (quad, hin, ab, dd)
                rb[:, qk, :, :] = r.transpose(1, 3, 0, 2).reshape(128, NQ, 2)
            percore[g] = (w8_arr, rb)
        w8_arr, rb = percore[g]
        hsT = np.ascontiguousarray(hs[b].T)
        in_maps.append(
            {
                "hsT16": hsT.astype(np16),
                "hsT8": hsT.astype(np8),
                "wg": wg_arr,
                "w8": w8_arr,
                "wv": wv_arr,
                "wo": wo_arr,
                "gbias": gb,
                "rbias": rb,
                "bo": bo_t,
            }
        )
    return in_maps


def kernel(**inputs) -> np.ndarray:
    in_maps = _host_prep(**inputs)
    nc = get_nc()
    res = run_bass_kernel_spmd(nc, in_maps, list(range(B)))
    out = np.stack(
        [np.asarray(res.results[c]["outT"]).astype(np.float32).T for c in range(B)],
        axis=0,
    )
    return np.ascontiguousarray(out)


# revision 6
# speedup vs baseline: 1.1047x; 1.0018x over previous
"""AuthorGroupAttention Trainium2 kernel.

Data-parallel over batch: 8 samples -> 8 NeuronCores, one sample per core.
Routing resolved on host (per-core reader-group weights gathered, cast and
laid out per-engine-friendly in _host_prep).

Precision (validated vs reference: rel err ~5e-3 against the 2e-2 gate):
  - generic path: fp16 matmul operands everywhere, fp32 PSUM accumulation
  - reader path (weight 0.1 in the prob combine): fp8e4 operands with
    DoubleRow matmuls (0.5 cyc/row); its exp is a Schraudolph bit-trick on
    DVE (scores*1/ln2 + 55.2 rounded to uint8 = e4m3 bits of exp(scores/8)),
    consistent numerator/denominator so the approximation bias cancels in
    the softmax normalization.

Structure:
  - q/k produced in [d, t] layout (d on partitions): per head-pair (gen,
    fp16) / head-quad (rdr, fp8 DR with 32-row subtiles at tile_position)
    projection chains, interleaved into the attention loop as PE filler
    ("pump") with per-boundary forced drains.
  - scores per (head, s-block): gen [128,1024] psum tiles (2-deep pool),
    rdr th-split [128,512] tiles in a shared 4-deep "small" pool that also
    carries av accumulators, projection chains, and deferred v-proj.
  - attention transposed: stationary = exp tile [s,t], moving = [v | 1/w]
    so attn lands [t, d] with the softmax denominator Z/w in the extra
    column; one ACT copy bounces the accumulator to SBUF, gpsimd
    normalize_recip applies w/Z per path, gpsimd add combines gen+rdr.
  - attn [t, e] tiles are DMA-transposed (xbar) per (pair, t-block) into
    [e, t] for the fp16 output projection; v-bias is folded into the output
    bias on host (combined prob rows sum to 0.5).
  - AV/combine of head h-1 is software-pipelined into head h's score loop;
    v-projection for pairs 6-7 is deferred into the pump as late filler.
"""

import os
import sys

for _p in ("/opt/trn_rl_repo",):
    if os.path.isdir(_p) and _p not in sys.path:
        sys.path.insert(0, _p)

import numpy as np

import concourse.bass as bass
import concourse.mybir as mybir
from concourse import bacc
from concourse.tile import TileContext
from concourse.bass_utils import run_bass_kernel_spmd

B, T, E, H, G = 8, 1024, 1024, 16, 4
D = E // H  # 64
SCALING = float(D) ** -0.5
W_G = 0.9 / 2.0
W_R = 0.1 / 2.0
EO = 8
SO = 8
TB = 8
NP = 8  # head pairs
NQ = 4  # head quads

F32 = mybir.dt.float32
F16 = mybir.dt.float16
F8 = mybir.dt.float8e4
U8 = mybir.dt.uint8
DRM = mybir.MatmulPerfMode.DoubleRow
EXP = mybir.ActivationFunctionType.Exp
MULT = mybir.AluOpType.mult
ADD = mybir.AluOpType.add

# Schraudolph constants for e4m3 bits: bits = round(score*A + Bc)
SCH_A = SCALING * 8.0 / float(np.log(2.0))
SCH_B = 56.0 - 0.8

USE_RDR_SCORE_DR = True  # DoubleRow with 32-row subtiles for reader scores


def build_nc():
    nc = bacc.Bacc(name="author_group_attention_v2")

    hsT16 = nc.dram_tensor("hsT16", [E, T], F16, kind="ExternalInput")
    hsT8 = nc.dram_tensor("hsT8", [E, T], F8, kind="ExternalInput")
    wg = nc.dram_tensor("wg", [NP, 128, 2, EO, 128], F16, kind="ExternalInput")
    w8 = nc.dram_tensor("w8", [NQ, 128, 2, 2, EO, 128], F8, kind="ExternalInput")
    wv = nc.dram_tensor("wv", [128, EO, E], F16, kind="ExternalInput")
    wo = nc.dram_tensor("wo", [TB, 128, NP, 128], F16, kind="ExternalInput")
    gbias = nc.dram_tensor("gbias", [128, 2, NP], F32, kind="ExternalInput")
    rbias = nc.dram_tensor("rbias", [128, 2, NQ, 2], F32, kind="ExternalInput")
    bo = nc.dram_tensor("bo", [128, TB], F32, kind="ExternalInput")
    outT = nc.dram_tensor("outT", [E, T], F16, kind="ExternalOutput")

    hsT16_r = hsT16.rearrange("(eo p) t -> p eo t", p=128)
    hsT8_r = hsT8.rearrange("(eo p) t -> p eo t", p=128)


    with TileContext(nc) as tc:
        from contextlib import ExitStack

        with ExitStack() as stack:
            const = stack.enter_context(tc.tile_pool(name="const", bufs=1))

            hsT16_sb = const.tile([128, EO, T], F16, tag="hsT16")
            hs8_sb = const.tile([128, EO, T], F8, tag="hs8")
            v16_sb = const.tile([128, SO, H, 66], F16, tag="v16")
            wv_sb = const.tile([128, EO, E], F16, tag="wv")
            v8_sb = const.tile([128, SO, H, 66], F8, tag="v8")
            gbias_sb = const.tile([128, 2, NP], F32, tag="gbias")
            rbias_sb = const.tile([128, 2, NQ, 2], F32, tag="rbias")
            bo_sb = const.tile([128, TB], F32, tag="bo")
            comb_tiles = [
                const.tile([128, E], F16, tag=f"comb{tb}", name=f"comb{tb}")
                for tb in range(TB)
            ]
            attnT_tiles = [
                const.tile([128, T], F16, tag=f"attnT{p}", name=f"attnT{p}")
                for p in range(NP)
            ]

            # ones columns pre-scaled by 1/w so reciprocal gives w/Z
            nc.vector.memset(v16_sb[:, :, :, 64:65], 1.0 / W_G)
            nc.vector.memset(v8_sb[:, :, :, 64:65], 1.0 / W_R)


            # persistent pools used across prologue + main
            qk16p = stack.enter_context(tc.tile_pool(name="qk16", bufs=2))
            qk8p = stack.enter_context(tc.tile_pool(name="qk8", bufs=2))
            wgp = stack.enter_context(tc.tile_pool(name="wgp", bufs=2))
            w8p = stack.enter_context(tc.tile_pool(name="w8p", bufs=2))

            def gen_proj_steps(pair, pp2_pool):
                """Generic q/k projection for a head pair -> list of step
                closures (PE matmuls + ACT drains). Weight DMAs fire now."""
                steps = []
                outs = {}
                wt2 = wgp.tile([128, 2, EO, 128], F16, tag="wg")
                nc.sync.dma_start(wt2[:], wg[pair])
                for qk in range(2):
                    wt = wt2[:, qk]
                    dst = qk16p.tile([128, T], F16, tag=f"qk16_{qk}")
                    outs[qk] = dst
                    state = {}

                    def mk(qk, wt, dst, state, th, eo):
                        def step():
                            if eo == 0:
                                state[th] = pp2_pool.tile(
                                    [128, 512], F32, tag="sm", name=f"gp{pair}{qk}{th}"
                                )
                            nc.tensor.matmul(
                                state[th][:],
                                wt[:, eo, :],
                                hsT16_sb[:, eo, th * 512 : (th + 1) * 512],
                                start=(eo == 0),
                                stop=(eo == EO - 1),
                            )
                            if eo == EO - 1:
                                if th == 0:
                                    nc.scalar.activation(
                                        dst[:, 0:512],
                                        state[th][:],
                                        mybir.ActivationFunctionType.Identity,
                                        bias=gbias_sb[:, qk, pair : pair + 1],
                                    )
                                else:
                                    nc.vector.tensor_scalar_add(
                                        dst[:, 512:1024],
                                        state[th][:],
                                        gbias_sb[:, qk, pair : pair + 1],
                                    )
                        return step

                    for th in range(2):
                        for eo in range(EO):
                            steps.append(mk(qk, wt, dst, state, th, eo))
                return outs, steps

            def vproj_steps(pair, pool, tag):
                steps = []
                state = {}
                ocols = slice(pair * 128, (pair + 1) * 128)

                def mk(sb, eo):
                    def step():
                        if eo == 0:
                            state[sb] = pool.tile(
                                [128, 512], F32, tag=tag, name=f"pv{pair}_{sb}"
                            )
                        nc.tensor.matmul(
                            state[sb][:, 0:128],
                            hsT16_sb[:, eo, sb * 128 : (sb + 1) * 128],
                            wv_sb[:, eo, ocols],
                            start=(eo == 0),
                            stop=(eo == EO - 1),
                        )
                        if eo == EO - 1:
                            pv_r = state[sb][:, 0:128].rearrange(
                                "p (hh dd) -> p hh dd", dd=64
                            )
                            vsl = v16_sb[:, sb, 2 * pair : 2 * pair + 2, 0:64]
                            nc.vector.tensor_copy(vsl, pv_r)
                            nc.gpsimd.tensor_copy(
                                v8_sb[:, sb, 2 * pair : 2 * pair + 2, 0:64], vsl
                            )
                    return step

                for sb in range(SO):
                    for eo in range(EO):
                        steps.append(mk(sb, eo))
                return steps

            def rdr_proj_steps(quad, pp2_pool):
                """Reader q/k projection for a head quad (fp8 DoubleRow).
                Outputs supertiles [128(=4h x 32d), 2(d-half), T] fp8."""
                steps = []
                outs = {}
                wt4 = w8p.tile([128, 2, 2, EO, 128], F8, tag="w8")
                nc.sync.dma_start(wt4[:], w8[quad])
                for qk in range(2):
                    dst = qk8p.tile([128, 2, T], F8, tag=f"qk8_{qk}")
                    outs[qk] = dst
                    for ab in range(2):
                        wt = wt4[:, qk, ab]
                        state = {}

                        def mk(qk, ab, wt, dst, state, th, a):
                            def step():
                                if a == 0:
                                    state[th] = pp2_pool.tile(
                                        [128, 512], F32, tag="sm",
                                        name=f"rp{quad}{qk}{ab}{th}",
                                    )
                                nc.tensor.matmul(
                                    state[th][:],
                                    wt[:, 2 * a : 2 * a + 2, :],
                                    hs8_sb[:, 2 * a : 2 * a + 2,
                                           th * 512 : (th + 1) * 512],
                                    start=(a == 0),
                                    stop=(a == 3),
                                    perf_mode=DRM,
                                )
                                if a == 3:
                                    if th == 0:
                                        nc.vector.tensor_scalar_add(
                                            dst[:, ab, 0:512],
                                            state[th][:],
                                            rbias_sb[:, qk, quad, ab : ab + 1],
                                        )
                                    else:
                                        nc.scalar.activation(
                                            dst[:, ab, 512:1024],
                                            state[th][:],
                                            mybir.ActivationFunctionType.Identity,
                                            bias=rbias_sb[:, qk, quad, ab : ab + 1],
                                        )
                            return step

                        for th in range(2):
                            for a in range(4):
                                steps.append(mk(qk, ab, wt, dst, state, th, a))
                return outs, steps

            # ---------------- prologue: v proj + first projections ----------
            pump = []
            with tc.tile_pool(
                name="vps", bufs=2, space="PSUM"
            ) as vps, tc.tile_pool(name="pps", bufs=2, space="PSUM") as pps:
                # weight DMAs first: tiny vs the 5MB input stream, and the
                # prologue proj pump stalls without them
                qk0, steps_g0 = gen_proj_steps(0, pps)
                r0, steps_r0 = rdr_proj_steps(0, pps)
                for a, b in ((0, 1), (1, 2), (2, 4), (4, 6), (6, 8)):
                    nc.sync.dma_start(hsT16_sb[:, a:b], hsT16_r[:, a:b])
                    nc.sync.dma_start(wv_sb[:, a:b], wv[:, a:b])
                    if a == 2:
                        nc.sync.dma_start(gbias_sb[:], gbias[:])
                        nc.sync.dma_start(rbias_sb[:], rbias[:])
                        nc.sync.dma_start(bo_sb[:], bo[:])
                for a, b in ((0, 4), (4, 8)):
                    nc.sync.dma_start(hs8_sb[:, a:b], hsT8_r[:, a:b])
                pro_pump = steps_g0 + steps_r0

                for sb in range(SO):
                    pv = vps.tile([128, E], F32, tag="pv", name=f"pv{sb}")
                    for eo in range(EO):
                        for vh, w0, w1 in ((0, 0, 512), (1, 512, 768)):
                            nc.tensor.matmul(
                                pv[:, w0:w1],
                                hsT16_sb[:, eo, sb * 128 : (sb + 1) * 128],
                                wv_sb[:, eo, w0:w1],
                                start=(eo == 0),
                                stop=(eo == EO - 1),
                            )
                        for _ in range(2 if sb >= 4 else 1):
                            if pro_pump:
                                pro_pump.pop(0)()
                    pv_r = pv[:, 0:768].rearrange("p (hh dd) -> p hh dd", dd=64)
                    nc.scalar.copy(v16_sb[:, sb, 0:12, 0:64], pv_r)
                    nc.vector.tensor_copy(v8_sb[:, sb, 0:12, 0:64], pv_r)
                while pro_pump:
                    pro_pump.pop(0)()

            # ---------------- main attention loop ---------------------------
            # Software pipeline: the AV/combine work of head h-1 is emitted
            # interleaved into the scores/exp loop of head h, so the PE has
            # filler work while ACT/DVE drain the score tiles.
            wo_sb = const.tile([128, TB, NP, 128], F16, tag="wo_all")
            for j in range(TB):
                nc.sync.dma_start(wo_sb[:, j], wo[j])

            with ExitStack() as mstack:
                scp = mstack.enter_context(
                    tc.tile_pool(name="scp", bufs=2, space="PSUM")
                )
                smallp = mstack.enter_context(
                    tc.tile_pool(name="smallp", bufs=4, space="PSUM")
                )
                ex16p = mstack.enter_context(tc.tile_pool(name="ex16", bufs=2))
                ex8p = mstack.enter_context(tc.tile_pool(name="ex8", bufs=2))
                tmpp = mstack.enter_context(tc.tile_pool(name="tmpp", bufs=8))
                avsp = mstack.enter_context(tc.tile_pool(name="avsp", bufs=6))

                def av_steps(h, ex16, ex8):
                    """AV + combine for head h as a list of step closures.
                    Each tb yields 3 steps: gen-av mms, rdr-av mms, combine."""
                    pair, hp = h // 2, h % 2
                    vg = v16_sb[:, :, h, :]
                    v8h = v8_sb[:, :, h, :]
                    steps = []
                    state = {}

                    def mk_gen(tb):
                        def step():
                            tsl = slice(tb * 128, (tb + 1) * 128)
                            av = smallp.tile([128, 512], F32, tag="sm",
                                             name=f"av{h}_{tb}")
                            state[tb] = av
                            for a in range(SO):
                                nc.tensor.matmul(
                                    av[:, 0:65],
                                    ex16[:, a, tsl],
                                    vg[:, a, 0:65],
                                    start=(a == 0),
                                    stop=(a == SO - 1),
                                )
                        return step

                    def mk_rdr(tb):
                        def step():
                            tsl = slice(tb * 128, (tb + 1) * 128)
                            av = state[tb]
                            for a in range(4):
                                nc.tensor.matmul(
                                    av[:, 68:133],
                                    ex8[:, 2 * a : 2 * a + 2, tsl],
                                    v8h[:, 2 * a : 2 * a + 2, 0:65],
                                    start=(a == 0),
                                    stop=(a == 3),
                                    perf_mode=DRM,
                                )
                        return step

                    def mk_comb(tb):
                        def step():
                            av = state.pop(tb)
                            avs = avsp.tile([128, 133], F32, tag="avs")
                            nc.scalar.copy(avs[:], av[:, 0:133])
                            csl = slice(h * 64, h * 64 + 64)
                            tmp = tmpp.tile([128, 64], F16, tag="tmp")
                            nc.gpsimd.normalize_recip(
                                comb_tiles[tb][:, csl], avs[:, 0:64],
                                avs[:, 64:65],
                            )
                            nc.gpsimd.normalize_recip(
                                tmp[:], avs[:, 68:132], avs[:, 132:133],
                            )
                            nc.gpsimd.tensor_tensor(
                                comb_tiles[tb][:, csl],
                                comb_tiles[tb][:, csl],
                                tmp[:],
                                ADD,
                            )
                            if hp == 1:
                                nc.sync.dma_start_transpose(
                                    attnT_tiles[pair][
                                        :, tb * 128 : (tb + 1) * 128
                                    ],
                                    comb_tiles[tb][
                                        :, pair * 128 : (pair + 1) * 128
                                    ],
                                )
                        return step

                    for tb in range(TB):
                        steps.append(mk_gen(tb))
                        steps.append(mk_rdr(tb))
                        steps.append(mk_comb(tb))
                    return steps

                Qg = Kg = Q8 = K8 = None
                av_q = []  # pending av steps of the previous head

                def fill(n):
                    """Emit up to n units of filler: av steps take priority
                    (they unblock comb tiles), then proj pump steps."""
                    for _ in range(n):
                        if av_q:
                            av_q.pop(0)()
                        elif pump:
                            pump.pop(0)()

                for h in range(H):
                    pair, quad = h // 2, h // 4
                    hp, hq = h % 2, h % 4
                    if h == 0:
                        Qg, Kg = qk0[0], qk0[1]
                        Q8, K8 = r0[0], r0[1]
                        nxt_g = nxt_r = None
                    if h == 9:
                        pump.extend(vproj_steps(6, smallp, "sm"))
                    if h == 11:
                        pump.extend(vproj_steps(7, smallp, "sm"))
                    if hp == 0 and pair + 1 < NP:
                        nxt_g, s = gen_proj_steps(pair + 1, smallp)
                        pump.extend(s)
                    if hq == 0 and quad + 1 < NQ:
                        nxt_r, s = rdr_proj_steps(quad + 1, smallp)
                        pump.extend(s)

                    ex16 = ex16p.tile([128, SO, T], F16, tag="ex16")
                    ex8 = ex8p.tile([128, SO, T], F8, tag="ex8")

                    grow = slice(64 * hp, 64 * hp + 64)
                    rrow = slice(32 * hq, 32 * hq + 32)

                    for sb in range(SO):
                        ssl = slice(sb * 128, (sb + 1) * 128)
                        sc = scp.tile([128, T], F32, tag="sc", name=f"sc{h}_{sb}")
                        for th in range(2):
                            nc.tensor.matmul(
                                sc[:, th * 512 : (th + 1) * 512],
                                Kg[grow, ssl],
                                Qg[grow, th * 512 : (th + 1) * 512],
                                start=True, stop=True,
                            )
                        nc.scalar.activation(
                            ex16[:, sb, :], sc[:], EXP, scale=SCALING
                        )
                        fill(2)
                        for th in range(2):
                            rc = smallp.tile([128, 512], F32, tag="sm",
                                             name=f"rc{h}_{sb}_{th}")
                            if USE_RDR_SCORE_DR:
                                nc.tensor.matmul(
                                    rc[:],
                                    K8[rrow, :, ssl],
                                    Q8[rrow, :, th * 512 : (th + 1) * 512],
                                    start=True, stop=True,
                                    perf_mode=DRM,
                                    tile_position=(32 * hq, 0),
                                )
                            else:
                                for ab in range(2):
                                    nc.tensor.matmul(
                                        rc[:],
                                        K8[rrow, ab, ssl],
                                        Q8[rrow, ab, th * 512 : (th + 1) * 512],
                                        start=(ab == 0), stop=(ab == 1),
                                        tile_position=(32 * hq, 0),
                                    )
                            nc.vector.tensor_scalar(
                                ex8[:, sb, th * 512 : (th + 1) * 512].bitcast(U8),
                                rc[:],
                                SCH_A, SCH_B, MULT, ADD,
                            )
                            fill(1)
                        fill(1)

                    # queue this head's av work; emitted during later heads'
                    # scores loops via fill()
                    av_q.extend(av_steps(h, ex16, ex8))

                    if hp == 1:
                        while pump:
                            pump.pop(0)()
                        if nxt_g is not None:
                            Qg, Kg = nxt_g[0], nxt_g[1]
                            nxt_g = None
                        if hq == 3 and nxt_r is not None:
                            Q8, K8 = nxt_r[0], nxt_r[1]
                            nxt_r = None

                while av_q:
                    av_q.pop(0)()

            # ---------------- output projection -----------------------------
            with tc.tile_pool(name="ops", bufs=4, space="PSUM") as ops, \
                 tc.tile_pool(name="o16p", bufs=4) as o16p:
                # 4 half-chain accumulators live at once; prefix over pairs
                # 0..6 depends only on heads <= 13 so it overlaps the final
                # head's av/combine; the pair-7 step + drain go in wave 2.
                halves = [(j, th) for j in range(TB) for th in range(2)]
                pos = {}
                for w0 in range(0, 16, 4):
                    for j, th in halves[w0 : w0 + 4]:
                        tsl = slice(th * 512, (th + 1) * 512)
                        po = ops.tile([128, 512], F32, tag="po",
                                      name=f"po{j}_{th}")
                        pos[(j, th)] = po
                        for pr in range(NP - 1):
                            nc.tensor.matmul(
                                po[:],
                                wo_sb[:, j, pr, :],
                                attnT_tiles[pr][:, tsl],
                                start=(pr == 0),
                                stop=False,
                            )
                    for j, th in halves[w0 : w0 + 4]:
                        tsl = slice(th * 512, (th + 1) * 512)
                        po = pos.pop((j, th))
                        nc.tensor.matmul(
                            po[:],
                            wo_sb[:, j, NP - 1, :],
                            attnT_tiles[NP - 1][:, tsl],
                            start=False,
                            stop=True,
                        )
                        o16 = o16p.tile([128, 512], F16, tag="o16")
                        if th == 0:
                            nc.vector.tensor_scalar_add(
                                o16[:], po[:], bo_sb[:, j : j + 1]
                            )
                        else:
                            nc.scalar.activation(
                                o16[:], po[:],
                                mybir.ActivationFunctionType.Identity,
                                bias=bo_sb[:, j : j + 1],
                            )
                        nc.sync.dma_start(
                            outT[j * 128 : (j + 1) * 128, tsl], o16[:]
                        )

    nc.finalize()
    return nc


_NC_CACHE = {}


def get_nc():
    if "nc" not in _NC_CACHE:
        _NC_CACHE["nc"] = build_nc()
    return _NC_CACHE["nc"]


def _host_prep(hidden_states, reader_token, Wq, bq, Wk, bk, Wv, bv, Wo, bo,
               RWq, Rbq, RWk, Rbk, RWv, Rbv):
    f = np.float32
    np16 = mybir.dt.np(F16)
    np8 = mybir.dt.np(F8)
    hs = np.asarray(hidden_states, f)
    tok = np.asarray(reader_token).astype(np.int64)

    WqT = np.asarray(Wq, f).T  # [e, o]
    WkT = np.asarray(Wk, f).T
    WvT = np.asarray(Wv, f).T
    WoT = np.asarray(Wo, f).T
    bq = np.asarray(bq, f); bk = np.asarray(bk, f)
    bv = np.asarray(bv, f); bo_ = np.asarray(bo, f)
    Rbq = np.asarray(Rbq, f); Rbk = np.asarray(Rbk, f)

    # gen weights [NP, 128, 2, EO, 128]
    wg_arr = np.empty((NP, 128, 2, EO, 128), np16)
    for qk, WT in enumerate((WqT, WkT)):
        r = WT.reshape(EO, 128, NP, 128)  # (eo, p, pair, m)
        wg_arr[:, :, qk] = r.transpose(2, 1, 0, 3).astype(np16)
    # gen biases [128, 2, NP]
    gb = np.empty((128, 2, NP), f)
    for qk, bb in enumerate((bq, bk)):
        gb[:, qk, :] = bb.reshape(NP, 128).T

    # v-bias folds into output bias (probs rows sum to 0.5)
    bo_eff = bo_ + 0.5 * (np.asarray(Wo, f) @ bv)
    bo_t = np.ascontiguousarray(bo_eff.reshape(TB, 128).T)

    # wv [128, EO, E]
    wv_arr = np.ascontiguousarray(
        WvT.reshape(EO, 128, E).transpose(1, 0, 2)
    ).astype(np16)
    # wo [TB, 128, NP, 128]
    wo_arr = np.ascontiguousarray(
        WoT.reshape(NP, 128, TB, 128).transpose(2, 1, 0, 3)
    ).astype(np16)

    percore = {}
    in_maps = []
    for b in range(B):
        g = int(tok[b])
        if g not in percore:
            RWqT = np.asarray(RWq[g], f).T  # [e, o]
            RWkT = np.asarray(RWk[g], f).T
            w8_arr = np.empty((NQ, 128, 2, 2, EO, 128), np8)
            for qk, WT in enumerate((RWqT, RWkT)):
                # o = (quad*4 + m//32)*64 + ab*32 + m%32
                r = WT.reshape(EO, 128, NQ, 4, 2, 32)  # (eo,p,quad,hin,ab,dd)
                # -> (quad, p, qk, ab, eo, m=(hin,dd))
                w8_arr[:, :, qk] = r.transpose(2, 4, 1, 0, 3, 5).reshape(
                    NQ, 2, 128, EO, 128
                ).astype(np8).transpose(0, 2, 1, 3, 4).reshape(NQ, 128, 2, EO, 128)
            rb = np.empty((128, 2, NQ, 2), f)
            for qk, bb in enumerate((Rbq[g], Rbk[g])):
                # p = hin*32 + dd ; value = b[(quad*4+hin)*64 + ab*32 + dd]
                r = bb.reshape(NQ, 4, 2, 32)  # (quad, hin, ab, dd)
                rb[:, qk, :, :] = r.transpose(1, 3, 0, 2).reshape(128, NQ, 2)
            percore[g] = (w8_arr, rb)
        w8_arr, rb = percore[g]
        hsT = np.ascontiguousarray(hs[b].T)
        in_maps.append(
            {
                "hsT16": hsT.astype(np16),
                "hsT8": hsT.astype(np8),
                "wg": wg_arr,
                "w8": w8_arr,
                "wv": wv_arr,
                "wo": wo_arr,
                "gbias": gb,
                "rbias": rb,
                "bo": bo_t,
            }
        )
    return in_maps


def kernel(**inputs) -> np.ndarray:
    in_maps = _host_prep(**inputs)
    nc = get_nc()
    res = run_bass_kernel_spmd(nc, in_maps, list(range(B)))
    out = np.stack(
        [np.asarray(res.results[c]["outT"]).astype(np.float32).T for c in range(B)],
        axis=0,
    )
    return np.ascontiguousarray(out)


# revision 7
# speedup vs baseline: 1.1104x; 1.0052x over previous
"""AuthorGroupAttention Trainium2 kernel.

Data-parallel over batch: 8 samples -> 8 NeuronCores, one sample per core.
Routing resolved on host (per-core reader-group weights gathered, cast and
laid out per-engine-friendly in _host_prep).

Precision (validated vs reference: rel err ~5e-3 against the 2e-2 gate):
  - generic path: fp16 matmul operands everywhere, fp32 PSUM accumulation
  - reader path (weight 0.1 in the prob combine): fp8e4 operands with
    DoubleRow matmuls (0.5 cyc/row); its exp is a Schraudolph bit-trick on
    DVE (scores*1/ln2 + 55.2 rounded to uint8 = e4m3 bits of exp(scores/8)),
    consistent numerator/denominator so the approximation bias cancels in
    the softmax normalization.

Structure:
  - q/k produced in [d, t] layout (d on partitions): per head-pair (gen,
    fp16) / head-quad (rdr, fp8 DR with 32-row subtiles at tile_position)
    projection chains, interleaved into the attention loop as PE filler
    ("pump") with per-boundary forced drains.
  - scores per (head, s-block): gen [128,1024] psum tiles (2-deep pool),
    rdr th-split [128,512] tiles in a shared 4-deep "small" pool that also
    carries av accumulators, projection chains, and deferred v-proj.
  - attention transposed: stationary = exp tile [s,t], moving = [v | 1/w]
    so attn lands [t, d] with the softmax denominator Z/w in the extra
    column; one ACT copy bounces the accumulator to SBUF, gpsimd
    normalize_recip applies w/Z per path, gpsimd add combines gen+rdr.
  - attn [t, e] tiles are DMA-transposed (xbar) per (pair, t-block) into
    [e, t] for the fp16 output projection; v-bias is folded into the output
    bias on host (combined prob rows sum to 0.5).
  - AV/combine of head h-1 is software-pipelined into head h's score loop;
    v-projection for pairs 6-7 is deferred into the pump as late filler.
"""

import os
import sys

for _p in ("/opt/trn_rl_repo",):
    if os.path.isdir(_p) and _p not in sys.path:
        sys.path.insert(0, _p)

import numpy as np

import concourse.bass as bass
import concourse.mybir as mybir
from concourse import bacc
from concourse.tile import TileContext
from concourse.bass_utils import run_bass_kernel_spmd

B, T, E, H, G = 8, 1024, 1024, 16, 4
D = E // H  # 64
SCALING = float(D) ** -0.5
W_G = 0.9 / 2.0
W_R = 0.1 / 2.0
EO = 8
SO = 8
TB = 8
NP = 8  # head pairs
NQ = 4  # head quads

F32 = mybir.dt.float32
F16 = mybir.dt.float16
F8 = mybir.dt.float8e4
U8 = mybir.dt.uint8
DRM = mybir.MatmulPerfMode.DoubleRow
EXP = mybir.ActivationFunctionType.Exp
MULT = mybir.AluOpType.mult
ADD = mybir.AluOpType.add

# Schraudolph constants for e4m3 bits: bits = round(score*A + Bc)
SCH_A = SCALING * 8.0 / float(np.log(2.0))
SCH_B = 56.0 - 0.8

USE_RDR_SCORE_DR = True  # DoubleRow with 32-row subtiles for reader scores


def build_nc():
    nc = bacc.Bacc(name="author_group_attention_v2")

    hsT16 = nc.dram_tensor("hsT16", [E, T], F16, kind="ExternalInput")
    hsT8 = nc.dram_tensor("hsT8", [E, T], F8, kind="ExternalInput")
    wg = nc.dram_tensor("wg", [NP, 128, 2, EO, 128], F16, kind="ExternalInput")
    w8 = nc.dram_tensor("w8", [NQ, 128, 2, 2, EO, 128], F8, kind="ExternalInput")
    wv = nc.dram_tensor("wv", [128, EO, E], F16, kind="ExternalInput")
    wo = nc.dram_tensor("wo", [TB, 128, NP, 128], F16, kind="ExternalInput")
    gbias = nc.dram_tensor("gbias", [128, 2, NP], F32, kind="ExternalInput")
    rbias = nc.dram_tensor("rbias", [128, 2, NQ, 2], F32, kind="ExternalInput")
    bo = nc.dram_tensor("bo", [128, TB], F32, kind="ExternalInput")
    outT = nc.dram_tensor("outT", [E, T], F16, kind="ExternalOutput")

    hsT16_r = hsT16.rearrange("(eo p) t -> p eo t", p=128)
    hsT8_r = hsT8.rearrange("(eo p) t -> p eo t", p=128)


    with TileContext(nc) as tc:
        from contextlib import ExitStack

        with ExitStack() as stack:
            const = stack.enter_context(tc.tile_pool(name="const", bufs=1))

            hsT16_sb = const.tile([128, EO, T], F16, tag="hsT16")
            hs8_sb = const.tile([128, EO, T], F8, tag="hs8")
            v16_sb = const.tile([128, SO, H, 66], F16, tag="v16")
            wv_sb = const.tile([128, EO, E], F16, tag="wv")
            v8_sb = const.tile([128, SO, H, 66], F8, tag="v8")
            gbias_sb = const.tile([128, 2, NP], F32, tag="gbias")
            rbias_sb = const.tile([128, 2, NQ, 2], F32, tag="rbias")
            bo_sb = const.tile([128, TB], F32, tag="bo")
            comb_tiles = [
                const.tile([128, E], F16, tag=f"comb{tb}", name=f"comb{tb}")
                for tb in range(TB)
            ]
            attnT_tiles = [
                const.tile([128, T], F16, tag=f"attnT{p}", name=f"attnT{p}")
                for p in range(NP)
            ]

            # ones columns pre-scaled by 1/w so reciprocal gives w/Z
            nc.vector.memset(v16_sb[:, :, :, 64:65], 1.0 / W_G)
            nc.vector.memset(v8_sb[:, :, :, 64:65], 1.0 / W_R)


            # persistent pools used across prologue + main
            qk16p = stack.enter_context(tc.tile_pool(name="qk16", bufs=2))
            qk8p = stack.enter_context(tc.tile_pool(name="qk8", bufs=2))
            wgp = stack.enter_context(tc.tile_pool(name="wgp", bufs=2))
            w8p = stack.enter_context(tc.tile_pool(name="w8p", bufs=2))

            def gen_proj_steps(pair, pp2_pool):
                """Generic q/k projection for a head pair -> list of step
                closures (PE matmuls + ACT drains). Weight DMAs fire now."""
                steps = []
                outs = {}
                wt2 = wgp.tile([128, 2, EO, 128], F16, tag="wg")
                nc.sync.dma_start(wt2[:], wg[pair])
                for qk in range(2):
                    wt = wt2[:, qk]
                    dst = qk16p.tile([128, T], F16, tag=f"qk16_{qk}")
                    outs[qk] = dst
                    state = {}

                    def mk(qk, wt, dst, state, th, eo):
                        def step():
                            if eo == 0:
                                state[th] = pp2_pool.tile(
                                    [128, 512], F32, tag="sm", name=f"gp{pair}{qk}{th}"
                                )
                            nc.tensor.matmul(
                                state[th][:],
                                wt[:, eo, :],
                                hsT16_sb[:, eo, th * 512 : (th + 1) * 512],
                                start=(eo == 0),
                                stop=(eo == EO - 1),
                            )
                            if eo == EO - 1:
                                if th == 0:
                                    nc.scalar.activation(
                                        dst[:, 0:512],
                                        state[th][:],
                                        mybir.ActivationFunctionType.Identity,
                                        bias=gbias_sb[:, qk, pair : pair + 1],
                                    )
                                else:
                                    nc.vector.tensor_scalar_add(
                                        dst[:, 512:1024],
                                        state[th][:],
                                        gbias_sb[:, qk, pair : pair + 1],
                                    )
                        return step

                    for th in range(2):
                        for eo in range(EO):
                            steps.append(mk(qk, wt, dst, state, th, eo))
                return outs, steps

            def vproj_steps(pair, pool, tag):
                steps = []
                state = {}
                ocols = slice(pair * 128, (pair + 1) * 128)

                def mk(sb, eo):
                    def step():
                        if eo == 0:
                            state[sb] = pool.tile(
                                [128, 512], F32, tag=tag, name=f"pv{pair}_{sb}"
                            )
                        nc.tensor.matmul(
                            state[sb][:, 0:128],
                            hsT16_sb[:, eo, sb * 128 : (sb + 1) * 128],
                            wv_sb[:, eo, ocols],
                            start=(eo == 0),
                            stop=(eo == EO - 1),
                        )
                        if eo == EO - 1:
                            pv_r = state[sb][:, 0:128].rearrange(
                                "p (hh dd) -> p hh dd", dd=64
                            )
                            vsl = v16_sb[:, sb, 2 * pair : 2 * pair + 2, 0:64]
                            nc.vector.tensor_copy(vsl, pv_r)
                            nc.gpsimd.tensor_copy(
                                v8_sb[:, sb, 2 * pair : 2 * pair + 2, 0:64], vsl
                            )
                    return step

                for sb in range(SO):
                    for eo in range(EO):
                        steps.append(mk(sb, eo))
                return steps

            def rdr_proj_steps(quad, pp2_pool):
                """Reader q/k projection for a head quad (fp8 DoubleRow).
                Outputs supertiles [128(=4h x 32d), 2(d-half), T] fp8."""
                steps = []
                outs = {}
                wt4 = w8p.tile([128, 2, 2, EO, 128], F8, tag="w8")
                nc.sync.dma_start(wt4[:], w8[quad])
                for qk in range(2):
                    dst = qk8p.tile([128, 2, T], F8, tag=f"qk8_{qk}")
                    outs[qk] = dst
                    for ab in range(2):
                        wt = wt4[:, qk, ab]
                        state = {}

                        def mk(qk, ab, wt, dst, state, th, a):
                            def step():
                                if a == 0:
                                    state[th] = pp2_pool.tile(
                                        [128, 512], F32, tag="sm",
                                        name=f"rp{quad}{qk}{ab}{th}",
                                    )
                                nc.tensor.matmul(
                                    state[th][:],
                                    wt[:, 2 * a : 2 * a + 2, :],
                                    hs8_sb[:, 2 * a : 2 * a + 2,
                                           th * 512 : (th + 1) * 512],
                                    start=(a == 0),
                                    stop=(a == 3),
                                    perf_mode=DRM,
                                )
                                if a == 3:
                                    if th == 0:
                                        nc.vector.tensor_scalar_add(
                                            dst[:, ab, 0:512],
                                            state[th][:],
                                            rbias_sb[:, qk, quad, ab : ab + 1],
                                        )
                                    else:
                                        nc.scalar.activation(
                                            dst[:, ab, 512:1024],
                                            state[th][:],
                                            mybir.ActivationFunctionType.Identity,
                                            bias=rbias_sb[:, qk, quad, ab : ab + 1],
                                        )
                            return step

                        for th in range(2):
                            for a in range(4):
                                steps.append(mk(qk, ab, wt, dst, state, th, a))
                return outs, steps

            # ---------------- prologue: v proj + first projections ----------
            pump = []
            with tc.tile_pool(
                name="vps", bufs=2, space="PSUM"
            ) as vps, tc.tile_pool(name="pps", bufs=2, space="PSUM") as pps:
                # weight DMAs first: tiny vs the 5MB input stream, and the
                # prologue proj pump stalls without them
                qk0, steps_g0 = gen_proj_steps(0, pps)
                r0, steps_r0 = rdr_proj_steps(0, pps)
                for a, b in ((0, 1), (1, 2), (2, 4), (4, 6), (6, 8)):
                    nc.sync.dma_start(hsT16_sb[:, a:b], hsT16_r[:, a:b])
                    nc.sync.dma_start(wv_sb[:, a:b], wv[:, a:b])
                    if a == 2:
                        nc.sync.dma_start(gbias_sb[:], gbias[:])
                        nc.sync.dma_start(rbias_sb[:], rbias[:])
                        nc.sync.dma_start(bo_sb[:], bo[:])
                for a, b in ((0, 4), (4, 8)):
                    nc.sync.dma_start(hs8_sb[:, a:b], hsT8_r[:, a:b])
                pro_pump = steps_g0 + steps_r0

                for sb in range(SO):
                    pv = vps.tile([128, E], F32, tag="pv", name=f"pv{sb}")
                    for eo in range(EO):
                        for vh, w0, w1 in ((0, 0, 512), (1, 512, 768)):
                            nc.tensor.matmul(
                                pv[:, w0:w1],
                                hsT16_sb[:, eo, sb * 128 : (sb + 1) * 128],
                                wv_sb[:, eo, w0:w1],
                                start=(eo == 0),
                                stop=(eo == EO - 1),
                            )
                        for _ in range(2 if sb >= 4 else 1):
                            if pro_pump:
                                pro_pump.pop(0)()
                    pv_r = pv[:, 0:768].rearrange("p (hh dd) -> p hh dd", dd=64)
                    nc.scalar.copy(v16_sb[:, sb, 0:12, 0:64], pv_r)
                    nc.vector.tensor_copy(v8_sb[:, sb, 0:12, 0:64], pv_r)
                while pro_pump:
                    pro_pump.pop(0)()

            # ---------------- main attention loop ---------------------------
            # Software pipeline: the AV/combine work of head h-1 is emitted
            # interleaved into the scores/exp loop of head h, so the PE has
            # filler work while ACT/DVE drain the score tiles.
            wo_sb = const.tile([128, TB, NP, 128], F16, tag="wo_all")
            for j in range(TB):
                nc.sync.dma_start(wo_sb[:, j], wo[j])

            with ExitStack() as mstack:
                scp = mstack.enter_context(
                    tc.tile_pool(name="scp", bufs=2, space="PSUM")
                )
                smallp = mstack.enter_context(
                    tc.tile_pool(name="smallp", bufs=4, space="PSUM")
                )
                ex16p = mstack.enter_context(tc.tile_pool(name="ex16", bufs=2))
                ex8p = mstack.enter_context(tc.tile_pool(name="ex8", bufs=2))
                tmpp = mstack.enter_context(tc.tile_pool(name="tmpp", bufs=8))
                avsp = mstack.enter_context(tc.tile_pool(name="avsp", bufs=6))

                def av_steps(h, ex16, ex8):
                    """AV + combine for head h as a list of step closures.
                    Each tb yields 3 steps: gen-av mms, rdr-av mms, combine."""
                    pair, hp = h // 2, h % 2
                    vg = v16_sb[:, :, h, :]
                    v8h = v8_sb[:, :, h, :]
                    steps = []
                    state = {}

                    def mk_gen(tb):
                        def step():
                            tsl = slice(tb * 128, (tb + 1) * 128)
                            av = smallp.tile([128, 512], F32, tag="sm",
                                             name=f"av{h}_{tb}")
                            state[tb] = av
                            for a in range(SO):
                                nc.tensor.matmul(
                                    av[:, 0:65],
                                    ex16[:, a, tsl],
                                    vg[:, a, 0:65],
                                    start=(a == 0),
                                    stop=(a == SO - 1),
                                )
                        return step

                    def mk_rdr(tb):
                        def step():
                            tsl = slice(tb * 128, (tb + 1) * 128)
                            av = state[tb]
                            for a in range(4):
                                nc.tensor.matmul(
                                    av[:, 68:133],
                                    ex8[:, 2 * a : 2 * a + 2, tsl],
                                    v8h[:, 2 * a : 2 * a + 2, 0:65],
                                    start=(a == 0),
                                    stop=(a == 3),
                                    perf_mode=DRM,
                                )
                        return step

                    def mk_comb(tb):
                        def step():
                            av = state.pop(tb)
                            avs = avsp.tile([128, 133], F32, tag="avs")
                            nc.scalar.copy(avs[:], av[:, 0:133])
                            csl = slice(h * 64, h * 64 + 64)
                            tmp = tmpp.tile([128, 64], F16, tag="tmp")
                            nc.gpsimd.normalize_recip(
                                comb_tiles[tb][:, csl], avs[:, 0:64],
                                avs[:, 64:65],
                            )
                            nc.gpsimd.normalize_recip(
                                tmp[:], avs[:, 68:132], avs[:, 132:133],
                            )
                            nc.gpsimd.tensor_tensor(
                                comb_tiles[tb][:, csl],
                                comb_tiles[tb][:, csl],
                                tmp[:],
                                ADD,
                            )
                            if hp == 1:
                                nc.sync.dma_start_transpose(
                                    attnT_tiles[pair][
                                        :, tb * 128 : (tb + 1) * 128
                                    ],
                                    comb_tiles[tb][
                                        :, pair * 128 : (pair + 1) * 128
                                    ],
                                )
                        return step

                    for tb in range(TB):
                        steps.append(mk_gen(tb))
                        steps.append(mk_rdr(tb))
                        steps.append(mk_comb(tb))
                    return steps

                Qg = Kg = Q8 = K8 = None
                av_q = []  # pending av steps of the previous head

                def fill(n):
                    """Emit up to n units of filler: av steps take priority
                    (they unblock comb tiles), then proj pump steps."""
                    for _ in range(n):
                        if av_q:
                            av_q.pop(0)()
                        elif pump:
                            pump.pop(0)()

                for h in range(H):
                    pair, quad = h // 2, h // 4
                    hp, hq = h % 2, h % 4
                    if h == 0:
                        Qg, Kg = qk0[0], qk0[1]
                        Q8, K8 = r0[0], r0[1]
                        nxt_g = nxt_r = None
                    if h == 9:
                        pump.extend(vproj_steps(6, smallp, "sm"))
                    if h == 11:
                        pump.extend(vproj_steps(7, smallp, "sm"))
                    if hp == 0 and pair + 1 < NP:
                        nxt_g, s = gen_proj_steps(pair + 1, smallp)
                        pump.extend(s)
                    if hq == 0 and quad + 1 < NQ:
                        nxt_r, s = rdr_proj_steps(quad + 1, smallp)
                        pump.extend(s)

                    ex16 = ex16p.tile([128, SO, T], F16, tag="ex16")
                    ex8 = ex8p.tile([128, SO, T], F8, tag="ex8")

                    grow = slice(64 * hp, 64 * hp + 64)
                    rrow = slice(32 * hq, 32 * hq + 32)

                    for sb in range(SO):
                        ssl = slice(sb * 128, (sb + 1) * 128)
                        sc = scp.tile([128, T], F32, tag="sc", name=f"sc{h}_{sb}")
                        for th in range(2):
                            nc.tensor.matmul(
                                sc[:, th * 512 : (th + 1) * 512],
                                Kg[grow, ssl],
                                Qg[grow, th * 512 : (th + 1) * 512],
                                start=True, stop=True,
                            )
                        nc.scalar.activation(
                            ex16[:, sb, :], sc[:], EXP, scale=SCALING
                        )
                        fill(2)
                        for th in range(2):
                            rc = smallp.tile([128, 512], F32, tag="sm",
                                             name=f"rc{h}_{sb}_{th}")
                            if USE_RDR_SCORE_DR:
                                nc.tensor.matmul(
                                    rc[:],
                                    K8[rrow, :, ssl],
                                    Q8[rrow, :, th * 512 : (th + 1) * 512],
                                    start=True, stop=True,
                                    perf_mode=DRM,
                                    tile_position=(32 * hq, 0),
                                )
                            else:
                                for ab in range(2):
                                    nc.tensor.matmul(
                                        rc[:],
                                        K8[rrow, ab, ssl],
                                        Q8[rrow, ab, th * 512 : (th + 1) * 512],
                                        start=(ab == 0), stop=(ab == 1),
                                        tile_position=(32 * hq, 0),
                                    )
                            nc.vector.tensor_scalar(
                                ex8[:, sb, th * 512 : (th + 1) * 512].bitcast(U8),
                                rc[:],
                                SCH_A, SCH_B, MULT, ADD,
                            )
                            fill(1)
                        fill(1)

                    # queue this head's av work; emitted during later heads'
                    # scores loops via fill()
                    av_q.extend(av_steps(h, ex16, ex8))

                    if hp == 1:
                        while pump:
                            pump.pop(0)()
                        if nxt_g is not None:
                            Qg, Kg = nxt_g[0], nxt_g[1]
                            nxt_g = None
                        if hq == 3 and nxt_r is not None:
                            Q8, K8 = nxt_r[0], nxt_r[1]
                            nxt_r = None

                while av_q:
                    av_q.pop(0)()

            # ---------------- output projection -----------------------------
            with tc.tile_pool(name="ops", bufs=6, space="PSUM") as ops, \
                 tc.tile_pool(name="o16p", bufs=4) as o16p:
                # 4 half-chain accumulators live at once; prefix over pairs
                # 0..6 depends only on heads <= 13 so it overlaps the final
                # head's av/combine; the pair-7 step + drain go in wave 2.
                halves = [(j, th) for j in range(TB) for th in range(2)]
                pos = {}
                for w0 in range(0, 16, 4):
                    for j, th in halves[w0 : w0 + 4]:
                        tsl = slice(th * 512, (th + 1) * 512)
                        po = ops.tile([128, 512], F32, tag="po",
                                      name=f"po{j}_{th}")
                        pos[(j, th)] = po
                        for pr in range(NP - 1):
                            nc.tensor.matmul(
                                po[:],
                                wo_sb[:, j, pr, :],
                                attnT_tiles[pr][:, tsl],
                                start=(pr == 0),
                                stop=False,
                            )
                    for j, th in halves[w0 : w0 + 4]:
                        tsl = slice(th * 512, (th + 1) * 512)
                        po = pos.pop((j, th))
                        nc.tensor.matmul(
                            po[:],
                            wo_sb[:, j, NP - 1, :],
                            attnT_tiles[NP - 1][:, tsl],
                            start=False,
                            stop=True,
                        )
                        o16 = o16p.tile([128, 512], F16, tag="o16")
                        if th == 0:
                            nc.vector.tensor_scalar_add(
                                o16[:], po[:], bo_sb[:, j : j + 1]
                            )
                        else:
                            nc.scalar.activation(
                                o16[:], po[:],
                                mybir.ActivationFunctionType.Identity,
                                bias=bo_sb[:, j : j + 1],
                            )
                        nc.sync.dma_start(
                            outT[j * 128 : (j + 1) * 128, tsl], o16[:]
                        )

    nc.finalize()
    return nc


_NC_CACHE = {}


def get_nc():
    if "nc" not in _NC_CACHE:
        _NC_CACHE["nc"] = build_nc()
    return _NC_CACHE["nc"]


def _host_prep(hidden_states, reader_token, Wq, bq, Wk, bk, Wv, bv, Wo, bo,
               RWq, Rbq, RWk, Rbk, RWv, Rbv):
    f = np.float32
    np16 = mybir.dt.np(F16)
    np8 = mybir.dt.np(F8)
    hs = np.asarray(hidden_states, f)
    tok = np.asarray(reader_token).astype(np.int64)

    WqT = np.asarray(Wq, f).T  # [e, o]
    WkT = np.asarray(Wk, f).T
    WvT = np.asarray(Wv, f).T
    WoT = np.asarray(Wo, f).T
    bq = np.asarray(bq, f); bk = np.asarray(bk, f)
    bv = np.asarray(bv, f); bo_ = np.asarray(bo, f)
    Rbq = np.asarray(Rbq, f); Rbk = np.asarray(Rbk, f)

    # gen weights [NP, 128, 2, EO, 128]
    wg_arr = np.empty((NP, 128, 2, EO, 128), np16)
    for qk, WT in enumerate((WqT, WkT)):
        r = WT.reshape(EO, 128, NP, 128)  # (eo, p, pair, m)
        wg_arr[:, :, qk] = r.transpose(2, 1, 0, 3).astype(np16)
    # gen biases [128, 2, NP]
    gb = np.empty((128, 2, NP), f)
    for qk, bb in enumerate((bq, bk)):
        gb[:, qk, :] = bb.reshape(NP, 128).T

    # v-bias folds into output bias (probs rows sum to 0.5)
    bo_eff = bo_ + 0.5 * (np.asarray(Wo, f) @ bv)
    bo_t = np.ascontiguousarray(bo_eff.reshape(TB, 128).T)

    # wv [128, EO, E]
    wv_arr = np.ascontiguousarray(
        WvT.reshape(EO, 128, E).transpose(1, 0, 2)
    ).astype(np16)
    # wo [TB, 128, NP, 128]
    wo_arr = np.ascontiguousarray(
        WoT.reshape(NP, 128, TB, 128).transpose(2, 1, 0, 3)
    ).astype(np16)

    percore = {}
    in_maps = []
    for b in range(B):
        g = int(tok[b])
        if g not in percore:
            RWqT = np.asarray(RWq[g], f).T  # [e, o]
            RWkT = np.asarray(RWk[g], f).T
            w8_arr = np.empty((NQ, 128, 2, 2, EO, 128), np8)
            for qk, WT in enumerate((RWqT, RWkT)):
                # o = (quad*4 + m//32)*64 + ab*32 + m%32
                r = WT.reshape(EO, 128, NQ, 4, 2, 32)  # (eo,p,quad,hin,ab,dd)
                # -> (quad, p, qk, ab, eo, m=(hin,dd))
                w8_arr[:, :, qk] = r.transpose(2, 4, 1, 0, 3, 5).reshape(
                    NQ, 2, 128, EO, 128
                ).astype(np8).transpose(0, 2, 1, 3, 4).reshape(NQ, 128, 2, EO, 128)
            rb = np.empty((128, 2, NQ, 2), f)
            for qk, bb in enumerate((Rbq[g], Rbk[g])):
                # p = hin*32 + dd ; value = b[(quad*4+hin)*64 + ab*32 + dd]
                r = bb.reshape(NQ, 4, 2, 32)  # (quad, hin, ab, dd)
                rb[:, qk, :, :] = r.transpose(1, 3, 0, 2).reshape(128, NQ, 2)
            percore[g] = (w8_arr, rb)
        w8_arr, rb = percore[g]
        hsT = np.ascontiguousarray(hs[b].T)
        in_maps.append(
            {
                "hsT16": hsT.astype(np16),
                "hsT8": hsT.astype(np8),
                "wg": wg_arr,
                "w8": w8_arr,
                "wv": wv_arr,
                "wo": wo_arr,
                "gbias": gb,
                "rbias": rb,
                "bo": bo_t,
            }
        )
    return in_maps


def kernel(**inputs) -> np.ndarray:
    in_maps = _host_prep(**inputs)
    nc = get_nc()
    res = run_bass_kernel_spmd(nc, in_maps, list(range(B)))
    out = np.stack(
        [np.asarray(res.results[c]["outT"]).astype(np.float32).T for c in range(B)],
        axis=0,
    )
    return np.ascontiguousarray(out)


# revision 8
# speedup vs baseline: 1.1140x; 1.0032x over previous
"""AuthorGroupAttention Trainium2 kernel.

Data-parallel over batch: 8 samples -> 8 NeuronCores, one sample per core.
Routing resolved on host (per-core reader-group weights gathered, cast and
laid out per-engine-friendly in _host_prep).

Precision (validated vs reference: rel err ~5e-3 against the 2e-2 gate):
  - generic path: fp16 matmul operands everywhere, fp32 PSUM accumulation
  - reader path (weight 0.1 in the prob combine): fp8e4 operands with
    DoubleRow matmuls (0.5 cyc/row); its exp is a Schraudolph bit-trick on
    DVE (scores*1/ln2 + 55.2 rounded to uint8 = e4m3 bits of exp(scores/8)),
    consistent numerator/denominator so the approximation bias cancels in
    the softmax normalization.

Structure:
  - q/k produced in [d, t] layout (d on partitions): per head-pair (gen,
    fp16) / head-quad (rdr, fp8 DR with 32-row subtiles at tile_position)
    projection chains, interleaved into the attention loop as PE filler
    ("pump") with per-boundary forced drains.
  - scores per (head, s-block): gen [128,1024] psum tiles (2-deep pool),
    rdr th-split [128,512] tiles in a shared 4-deep "small" pool that also
    carries av accumulators, projection chains, and deferred v-proj.
  - attention transposed: stationary = exp tile [s,t], moving = [v | 1/w]
    so attn lands [t, d] with the softmax denominator Z/w in the extra
    column; one ACT copy bounces the accumulator to SBUF, gpsimd
    normalize_recip applies w/Z per path, gpsimd add combines gen+rdr.
  - attn [t, e] tiles are DMA-transposed (xbar) per (pair, t-block) into
    [e, t] for the fp16 output projection; v-bias is folded into the output
    bias on host (combined prob rows sum to 0.5).
  - AV/combine of head h-1 is software-pipelined into head h's score loop;
    v-projection for pairs 6-7 is deferred into the pump as late filler.
"""

import os
import sys

for _p in ("/opt/trn_rl_repo",):
    if os.path.isdir(_p) and _p not in sys.path:
        sys.path.insert(0, _p)

import numpy as np

import concourse.bass as bass
import concourse.mybir as mybir
from concourse import bacc
from concourse.tile import TileContext
from concourse.bass_utils import run_bass_kernel_spmd

B, T, E, H, G = 8, 1024, 1024, 16, 4
D = E // H  # 64
SCALING = float(D) ** -0.5
W_G = 0.9 / 2.0
W_R = 0.1 / 2.0
EO = 8
SO = 8
TB = 8
NP = 8  # head pairs
NQ = 4  # head quads

F32 = mybir.dt.float32
F16 = mybir.dt.float16
F8 = mybir.dt.float8e4
U8 = mybir.dt.uint8
DRM = mybir.MatmulPerfMode.DoubleRow
EXP = mybir.ActivationFunctionType.Exp
MULT = mybir.AluOpType.mult
ADD = mybir.AluOpType.add

# Schraudolph constants for e4m3 bits: bits = round(score*A + Bc)
SCH_A = SCALING * 8.0 / float(np.log(2.0))
SCH_B = 56.0 - 0.8

USE_RDR_SCORE_DR = True  # DoubleRow with 32-row subtiles for reader scores


def build_nc():
    nc = bacc.Bacc(name="author_group_attention_v2")

    hsT16 = nc.dram_tensor("hsT16", [E, T], F16, kind="ExternalInput")
    hsT8 = nc.dram_tensor("hsT8", [E, T], F8, kind="ExternalInput")
    wg = nc.dram_tensor("wg", [NP, 128, 2, EO, 128], F16, kind="ExternalInput")
    w8 = nc.dram_tensor("w8", [NQ, 128, 2, 2, EO, 128], F8, kind="ExternalInput")
    wv = nc.dram_tensor("wv", [128, EO, E], F16, kind="ExternalInput")
    wo = nc.dram_tensor("wo", [TB, 128, NP, 128], F16, kind="ExternalInput")
    gbias = nc.dram_tensor("gbias", [128, 2, NP], F32, kind="ExternalInput")
    rbias = nc.dram_tensor("rbias", [128, 2, NQ, 2], F32, kind="ExternalInput")
    bo = nc.dram_tensor("bo", [128, TB], F32, kind="ExternalInput")
    outT = nc.dram_tensor("outT", [E, T], F16, kind="ExternalOutput")

    hsT16_r = hsT16.rearrange("(eo p) t -> p eo t", p=128)
    hsT8_r = hsT8.rearrange("(eo p) t -> p eo t", p=128)


    with TileContext(nc) as tc:
        from contextlib import ExitStack

        with ExitStack() as stack:
            const = stack.enter_context(tc.tile_pool(name="const", bufs=1))

            hsT16_sb = const.tile([128, EO, T], F16, tag="hsT16")
            hs8_sb = const.tile([128, EO, T], F8, tag="hs8")
            v16_sb = const.tile([128, SO, H, 66], F16, tag="v16")
            wv_sb = const.tile([128, EO, E], F16, tag="wv")
            v8_sb = const.tile([128, SO, H, 66], F8, tag="v8")
            gbias_sb = const.tile([128, 2, NP], F32, tag="gbias")
            rbias_sb = const.tile([128, 2, NQ, 2], F32, tag="rbias")
            bo_sb = const.tile([128, TB], F32, tag="bo")
            comb_tiles = [
                const.tile([128, E], F16, tag=f"comb{tb}", name=f"comb{tb}")
                for tb in range(TB)
            ]
            attnT_tiles = [
                const.tile([128, T], F16, tag=f"attnT{p}", name=f"attnT{p}")
                for p in range(NP)
            ]

            # ones columns pre-scaled by 1/w so reciprocal gives w/Z
            nc.vector.memset(v16_sb[:, :, :, 64:65], 1.0 / W_G)
            nc.vector.memset(v8_sb[:, :, :, 64:65], 1.0 / W_R)


            # persistent pools used across prologue + main
            qk16p = stack.enter_context(tc.tile_pool(name="qk16", bufs=2))
            qk8p = stack.enter_context(tc.tile_pool(name="qk8", bufs=2))
            wgp = stack.enter_context(tc.tile_pool(name="wgp", bufs=2))
            w8p = stack.enter_context(tc.tile_pool(name="w8p", bufs=2))

            def gen_proj_steps(pair, pp2_pool):
                """Generic q/k projection for a head pair -> list of step
                closures (PE matmuls + ACT drains). Weight DMAs fire now."""
                steps = []
                outs = {}
                wt2 = wgp.tile([128, 2, EO, 128], F16, tag="wg")
                nc.sync.dma_start(wt2[:], wg[pair])
                for qk in range(2):
                    wt = wt2[:, qk]
                    dst = qk16p.tile([128, T], F16, tag=f"qk16_{qk}")
                    outs[qk] = dst
                    state = {}

                    def mk(qk, wt, dst, state, th, eo):
                        def step():
                            if eo == 0:
                                state[th] = pp2_pool.tile(
                                    [128, 512], F32, tag="sm", name=f"gp{pair}{qk}{th}"
                                )
                            nc.tensor.matmul(
                                state[th][:],
                                wt[:, eo, :],
                                hsT16_sb[:, eo, th * 512 : (th + 1) * 512],
                                start=(eo == 0),
                                stop=(eo == EO - 1),
                            )
                            if eo == EO - 1:
                                if th == 0:
                                    nc.scalar.activation(
                                        dst[:, 0:512],
                                        state[th][:],
                                        mybir.ActivationFunctionType.Identity,
                                        bias=gbias_sb[:, qk, pair : pair + 1],
                                    )
                                else:
                                    nc.vector.tensor_scalar_add(
                                        dst[:, 512:1024],
                                        state[th][:],
                                        gbias_sb[:, qk, pair : pair + 1],
                                    )
                        return step

                    for th in range(2):
                        for eo in range(EO):
                            steps.append(mk(qk, wt, dst, state, th, eo))
                return outs, steps

            def vproj_steps(pair, pool, tag):
                steps = []
                state = {}
                ocols = slice(pair * 128, (pair + 1) * 128)

                def mk(sb, eo):
                    def step():
                        if eo == 0:
                            state[sb] = pool.tile(
                                [128, 512], F32, tag=tag, name=f"pv{pair}_{sb}"
                            )
                        nc.tensor.matmul(
                            state[sb][:, 0:128],
                            hsT16_sb[:, eo, sb * 128 : (sb + 1) * 128],
                            wv_sb[:, eo, ocols],
                            start=(eo == 0),
                            stop=(eo == EO - 1),
                        )
                        if eo == EO - 1:
                            pv_r = state[sb][:, 0:128].rearrange(
                                "p (hh dd) -> p hh dd", dd=64
                            )
                            vsl = v16_sb[:, sb, 2 * pair : 2 * pair + 2, 0:64]
                            nc.vector.tensor_copy(vsl, pv_r)
                            nc.gpsimd.tensor_copy(
                                v8_sb[:, sb, 2 * pair : 2 * pair + 2, 0:64], vsl
                            )
                    return step

                for sb in range(SO):
                    for eo in range(EO):
                        steps.append(mk(sb, eo))
                return steps

            def rdr_proj_steps(quad, pp2_pool):
                """Reader q/k projection for a head quad (fp8 DoubleRow).
                Outputs supertiles [128(=4h x 32d), 2(d-half), T] fp8."""
                steps = []
                outs = {}
                wt4 = w8p.tile([128, 2, 2, EO, 128], F8, tag="w8")
                nc.sync.dma_start(wt4[:], w8[quad])
                for qk in range(2):
                    dst = qk8p.tile([128, 2, T], F8, tag=f"qk8_{qk}")
                    outs[qk] = dst
                    for ab in range(2):
                        wt = wt4[:, qk, ab]
                        state = {}

                        def mk(qk, ab, wt, dst, state, th, a):
                            def step():
                                if a == 0:
                                    state[th] = pp2_pool.tile(
                                        [128, 512], F32, tag="sm",
                                        name=f"rp{quad}{qk}{ab}{th}",
                                    )
                                nc.tensor.matmul(
                                    state[th][:],
                                    wt[:, 2 * a : 2 * a + 2, :],
                                    hs8_sb[:, 2 * a : 2 * a + 2,
                                           th * 512 : (th + 1) * 512],
                                    start=(a == 0),
                                    stop=(a == 3),
                                    perf_mode=DRM,
                                )
                                if a == 3:
                                    if th == 0:
                                        nc.vector.tensor_scalar_add(
                                            dst[:, ab, 0:512],
                                            state[th][:],
                                            rbias_sb[:, qk, quad, ab : ab + 1],
                                        )
                                    else:
                                        nc.scalar.activation(
                                            dst[:, ab, 512:1024],
                                            state[th][:],
                                            mybir.ActivationFunctionType.Identity,
                                            bias=rbias_sb[:, qk, quad, ab : ab + 1],
                                        )
                            return step

                        for th in range(2):
                            for a in range(4):
                                steps.append(mk(qk, ab, wt, dst, state, th, a))
                return outs, steps

            # ---------------- prologue: v proj + first projections ----------
            pump = []
            with tc.tile_pool(
                name="vps", bufs=2, space="PSUM"
            ) as vps, tc.tile_pool(name="pps", bufs=2, space="PSUM") as pps:
                # weight DMAs first: tiny vs the 5MB input stream, and the
                # prologue proj pump stalls without them
                qk0, steps_g0 = gen_proj_steps(0, pps)
                r0, steps_r0 = rdr_proj_steps(0, pps)
                for a, b in ((0, 1), (1, 2), (2, 4), (4, 6), (6, 8)):
                    nc.sync.dma_start(hsT16_sb[:, a:b], hsT16_r[:, a:b])
                    nc.sync.dma_start(wv_sb[:, a:b, 0:768], wv[:, a:b, 0:768])
                    if a == 2:
                        nc.sync.dma_start(gbias_sb[:], gbias[:])
                        nc.sync.dma_start(rbias_sb[:], rbias[:])
                        nc.sync.dma_start(bo_sb[:], bo[:])
                for a, b in ((0, 4), (4, 8)):
                    nc.sync.dma_start(hs8_sb[:, a:b], hsT8_r[:, a:b])
                # wv columns for the deferred v-proj pairs 6-7 (needed h>=9)
                nc.sync.dma_start(wv_sb[:, :, 768:1024], wv[:, :, 768:1024])
                pro_pump = steps_g0 + steps_r0

                for sb in range(SO):
                    pv = vps.tile([128, E], F32, tag="pv", name=f"pv{sb}")
                    for eo in range(EO):
                        for vh, w0, w1 in ((0, 0, 512), (1, 512, 768)):
                            nc.tensor.matmul(
                                pv[:, w0:w1],
                                hsT16_sb[:, eo, sb * 128 : (sb + 1) * 128],
                                wv_sb[:, eo, w0:w1],
                                start=(eo == 0),
                                stop=(eo == EO - 1),
                            )
                        for _ in range(2 if sb >= 4 else 1):
                            if pro_pump:
                                pro_pump.pop(0)()
                    pv_r = pv[:, 0:768].rearrange("p (hh dd) -> p hh dd", dd=64)
                    nc.scalar.copy(v16_sb[:, sb, 0:12, 0:64], pv_r)
                    nc.vector.tensor_copy(v8_sb[:, sb, 0:12, 0:64], pv_r)
                while pro_pump:
                    pro_pump.pop(0)()

            # ---------------- main attention loop ---------------------------
            # Software pipeline: the AV/combine work of head h-1 is emitted
            # interleaved into the scores/exp loop of head h, so the PE has
            # filler work while ACT/DVE drain the score tiles.
            wo_sb = const.tile([128, TB, NP, 128], F16, tag="wo_all")
            for j in range(TB):
                nc.sync.dma_start(wo_sb[:, j], wo[j])

            with ExitStack() as mstack:
                scp = mstack.enter_context(
                    tc.tile_pool(name="scp", bufs=2, space="PSUM")
                )
                smallp = mstack.enter_context(
                    tc.tile_pool(name="smallp", bufs=4, space="PSUM")
                )
                ex16p = mstack.enter_context(tc.tile_pool(name="ex16", bufs=2))
                ex8p = mstack.enter_context(tc.tile_pool(name="ex8", bufs=2))
                tmpp = mstack.enter_context(tc.tile_pool(name="tmpp", bufs=8))
                avsp = mstack.enter_context(tc.tile_pool(name="avsp", bufs=6))

                def av_steps(h, ex16, ex8):
                    """AV + combine for head h as a list of step closures.
                    Each tb yields 3 steps: gen-av mms, rdr-av mms, combine."""
                    pair, hp = h // 2, h % 2
                    vg = v16_sb[:, :, h, :]
                    v8h = v8_sb[:, :, h, :]
                    steps = []
                    state = {}

                    def mk_gen(tb):
                        def step():
                            tsl = slice(tb * 128, (tb + 1) * 128)
                            av = smallp.tile([128, 512], F32, tag="sm",
                                             name=f"av{h}_{tb}")
                            state[tb] = av
                            for a in range(SO):
                                nc.tensor.matmul(
                                    av[:, 0:65],
                                    ex16[:, a, tsl],
                                    vg[:, a, 0:65],
                                    start=(a == 0),
                                    stop=(a == SO - 1),
                                )
                        return step

                    def mk_rdr(tb):
                        def step():
                            tsl = slice(tb * 128, (tb + 1) * 128)
                            av = state[tb]
                            for a in range(4):
                                nc.tensor.matmul(
                                    av[:, 68:133],
                                    ex8[:, 2 * a : 2 * a + 2, tsl],
                                    v8h[:, 2 * a : 2 * a + 2, 0:65],
                                    start=(a == 0),
                                    stop=(a == 3),
                                    perf_mode=DRM,
                                )
                        return step

                    def mk_comb(tb):
                        def step():
                            av = state.pop(tb)
                            avs = avsp.tile([128, 133], F32, tag="avs")
                            nc.scalar.copy(avs[:], av[:, 0:133])
                            csl = slice(h * 64, h * 64 + 64)
                            tmp = tmpp.tile([128, 64], F16, tag="tmp")
                            nc.gpsimd.normalize_recip(
                                comb_tiles[tb][:, csl], avs[:, 0:64],
                                avs[:, 64:65],
                            )
                            nc.gpsimd.normalize_recip(
                                tmp[:], avs[:, 68:132], avs[:, 132:133],
                            )
                            nc.gpsimd.tensor_tensor(
                                comb_tiles[tb][:, csl],
                                comb_tiles[tb][:, csl],
                                tmp[:],
                                ADD,
                            )
                            if hp == 1:
                                nc.sync.dma_start_transpose(
                                    attnT_tiles[pair][
                                        :, tb * 128 : (tb + 1) * 128
                                    ],
                                    comb_tiles[tb][
                                        :, pair * 128 : (pair + 1) * 128
                                    ],
                                )
                        return step

                    for tb in range(TB):
                        steps.append(mk_gen(tb))
                        steps.append(mk_rdr(tb))
                        steps.append(mk_comb(tb))
                    return steps

                Qg = Kg = Q8 = K8 = None
                av_q = []  # pending av steps of the previous head

                def fill(n):
                    """Emit up to n units of filler: av steps take priority
                    (they unblock comb tiles), then proj pump steps."""
                    for _ in range(n):
                        if av_q:
                            av_q.pop(0)()
                        elif pump:
                            pump.pop(0)()

                for h in range(H):
                    pair, quad = h // 2, h // 4
                    hp, hq = h % 2, h % 4
                    if h == 0:
                        Qg, Kg = qk0[0], qk0[1]
                        Q8, K8 = r0[0], r0[1]
                        nxt_g = nxt_r = None
                    if h == 9:
                        pump.extend(vproj_steps(6, smallp, "sm"))
                    if h == 11:
                        pump.extend(vproj_steps(7, smallp, "sm"))
                    if hp == 0 and pair + 1 < NP:
                        nxt_g, s = gen_proj_steps(pair + 1, smallp)
                        pump.extend(s)
                    if hq == 0 and quad + 1 < NQ:
                        nxt_r, s = rdr_proj_steps(quad + 1, smallp)
                        pump.extend(s)

                    ex16 = ex16p.tile([128, SO, T], F16, tag="ex16")
                    ex8 = ex8p.tile([128, SO, T], F8, tag="ex8")

                    grow = slice(64 * hp, 64 * hp + 64)
                    rrow = slice(32 * hq, 32 * hq + 32)

                    for sb in range(SO):
                        ssl = slice(sb * 128, (sb + 1) * 128)
                        sc = scp.tile([128, T], F32, tag="sc", name=f"sc{h}_{sb}")
                        for th in range(2):
                            nc.tensor.matmul(
                                sc[:, th * 512 : (th + 1) * 512],
                                Kg[grow, ssl],
                                Qg[grow, th * 512 : (th + 1) * 512],
                                start=True, stop=True,
                            )
                        nc.scalar.activation(
                            ex16[:, sb, :], sc[:], EXP, scale=SCALING
                        )
                        fill(2)
                        for th in range(2):
                            rc = smallp.tile([128, 512], F32, tag="sm",
                                             name=f"rc{h}_{sb}_{th}")
                            if USE_RDR_SCORE_DR:
                                nc.tensor.matmul(
                                    rc[:],
                                    K8[rrow, :, ssl],
                                    Q8[rrow, :, th * 512 : (th + 1) * 512],
                                    start=True, stop=True,
                                    perf_mode=DRM,
                                    tile_position=(32 * hq, 0),
                                )
                            else:
                                for ab in range(2):
                                    nc.tensor.matmul(
                                        rc[:],
                                        K8[rrow, ab, ssl],
                                        Q8[rrow, ab, th * 512 : (th + 1) * 512],
                                        start=(ab == 0), stop=(ab == 1),
                                        tile_position=(32 * hq, 0),
                                    )
                            nc.vector.tensor_scalar(
                                ex8[:, sb, th * 512 : (th + 1) * 512].bitcast(U8),
                                rc[:],
                                SCH_A, SCH_B, MULT, ADD,
                            )
                            fill(1)
                        fill(1)

                    # queue this head's av work; emitted during later heads'
                    # scores loops via fill()
                    av_q.extend(av_steps(h, ex16, ex8))

                    if hp == 1:
                        while pump:
                            pump.pop(0)()
                        if nxt_g is not None:
                            Qg, Kg = nxt_g[0], nxt_g[1]
                            nxt_g = None
                        if hq == 3 and nxt_r is not None:
                            Q8, K8 = nxt_r[0], nxt_r[1]
                            nxt_r = None

                while av_q:
                    av_q.pop(0)()

            # ---------------- output projection -----------------------------
            with tc.tile_pool(name="ops", bufs=6, space="PSUM") as ops, \
                 tc.tile_pool(name="o16p", bufs=4) as o16p:
                # 4 half-chain accumulators live at once; prefix over pairs
                # 0..6 depends only on heads <= 13 so it overlaps the final
                # head's av/combine; the pair-7 step + drain go in wave 2.
                halves = [(j, th) for j in range(TB) for th in range(2)]
                pos = {}
                for w0 in range(0, 16, 4):
                    for j, th in halves[w0 : w0 + 4]:
                        tsl = slice(th * 512, (th + 1) * 512)
                        po = ops.tile([128, 512], F32, tag="po",
                                      name=f"po{j}_{th}")
                        pos[(j, th)] = po
                        for pr in range(NP - 1):
                            nc.tensor.matmul(
                                po[:],
                                wo_sb[:, j, pr, :],
                                attnT_tiles[pr][:, tsl],
                                start=(pr == 0),
                                stop=False,
                            )
                    for j, th in halves[w0 : w0 + 4]:
                        tsl = slice(th * 512, (th + 1) * 512)
                        po = pos.pop((j, th))
                        nc.tensor.matmul(
                            po[:],
                            wo_sb[:, j, NP - 1, :],
                            attnT_tiles[NP - 1][:, tsl],
                            start=False,
                            stop=True,
                        )
                        o16 = o16p.tile([128, 512], F16, tag="o16")
                        if th == 0:
                            nc.vector.tensor_scalar_add(
                                o16[:], po[:], bo_sb[:, j : j + 1]
                            )
                        else:
                            nc.scalar.activation(
                                o16[:], po[:],
                                mybir.ActivationFunctionType.Identity,
                                bias=bo_sb[:, j : j + 1],
                            )
                        nc.sync.dma_start(
                            outT[j * 128 : (j + 1) * 128, tsl], o16[:]
                        )

    nc.finalize()
    return nc


_NC_CACHE = {}


def get_nc():
    if "nc" not in _NC_CACHE:
        _NC_CACHE["nc"] = build_nc()
    return _NC_CACHE["nc"]


def _host_prep(hidden_states, reader_token, Wq, bq, Wk, bk, Wv, bv, Wo, bo,
               RWq, Rbq, RWk, Rbk, RWv, Rbv):
    f = np.float32
    np16 = mybir.dt.np(F16)
    np8 = mybir.dt.np(F8)
    hs = np.asarray(hidden_states, f)
    tok = np.asarray(reader_token).astype(np.int64)

    WqT = np.asarray(Wq, f).T  # [e, o]
    WkT = np.asarray(Wk, f).T
    WvT = np.asarray(Wv, f).T
    WoT = np.asarray(Wo, f).T
    bq = np.asarray(bq, f); bk = np.asarray(bk, f)
    bv = np.asarray(bv, f); bo_ = np.asarray(bo, f)
    Rbq = np.asarray(Rbq, f); Rbk = np.asarray(Rbk, f)

    # gen weights [NP, 128, 2, EO, 128]
    wg_arr = np.empty((NP, 128, 2, EO, 128), np16)
    for qk, WT in enumerate((WqT, WkT)):
        r = WT.reshape(EO, 128, NP, 128)  # (eo, p, pair, m)
        wg_arr[:, :, qk] = r.transpose(2, 1, 0, 3).astype(np16)
    # gen biases [128, 2, NP]
    gb = np.empty((128, 2, NP), f)
    for qk, bb in enumerate((bq, bk)):
        gb[:, qk, :] = bb.reshape(NP, 128).T

    # v-bias folds into output bias (probs rows sum to 0.5)
    bo_eff = bo_ + 0.5 * (np.asarray(Wo, f) @ bv)
    bo_t = np.ascontiguousarray(bo_eff.reshape(TB, 128).T)

    # wv [128, EO, E]
    wv_arr = np.ascontiguousarray(
        WvT.reshape(EO, 128, E).transpose(1, 0, 2)
    ).astype(np16)
    # wo [TB, 128, NP, 128]
    wo_arr = np.ascontiguousarray(
        WoT.reshape(NP, 128, TB, 128).transpose(2, 1, 0, 3)
    ).astype(np16)

    percore = {}
    in_maps = []
    for b in range(B):
        g = int(tok[b])
        if g not in percore:
            RWqT = np.asarray(RWq[g], f).T  # [e, o]
            RWkT = np.asarray(RWk[g], f).T
            w8_arr = np.empty((NQ, 128, 2, 2, EO, 128), np8)
            for qk, WT in enumerate((RWqT, RWkT)):
                # o = (quad*4 + m//32)*64 + ab*32 + m%32
                r = WT.reshape(EO, 128, NQ, 4, 2, 32)  # (eo,p,quad,hin,ab,dd)
                # -> (quad, p, qk, ab, eo, m=(hin,dd))
                w8_arr[:, :, qk] = r.transpose(2, 4, 1, 0, 3, 5).reshape(
                    NQ, 2, 128, EO, 128
                ).astype(np8).transpose(0, 2, 1, 3, 4).reshape(NQ, 128, 2, EO, 128)
            rb = np.empty((128, 2, NQ, 2), f)
            for qk, bb in enumerate((Rbq[g], Rbk[g])):
                # p = hin*32 + dd ; value = b[(quad*4+hin)*64 + ab*32 + dd]
                r = bb.reshape(NQ, 4, 2, 32)  # (quad, hin, ab, dd)
                rb[:, qk, :, :] = r.transpose(1, 3, 0, 2).reshape(128, NQ, 2)
            percore[g] = (w8_arr, rb)
        w8_arr, rb = percore[g]
        hsT = np.ascontiguousarray(hs[b].T)
        in_maps.append(
            {
                "hsT16": hsT.astype(np16),
                "hsT8": hsT.astype(np8),
                "wg": wg_arr,
                "w8": w8_arr,
                "wv": wv_arr,
                "wo": wo_arr,
                "gbias": gb,
                "rbias": rb,
                "bo": bo_t,
            }
        )
    return in_maps


def kernel(**inputs) -> np.ndarray:
    in_maps = _host_prep(**inputs)
    nc = get_nc()
    res = run_bass_kernel_spmd(nc, in_maps, list(range(B)))
    out = np.stack(
        [np.asarray(res.results[c]["outT"]).astype(np.float32).T for c in range(B)],
        axis=0,
    )
    return np.ascontiguousarray(out)


# revision 9
# speedup vs baseline: 1.1143x; 1.0002x over previous
"""AuthorGroupAttention Trainium2 kernel.

Data-parallel over batch: 8 samples -> 8 NeuronCores, one sample per core.
Routing resolved on host (per-core reader-group weights gathered, cast and
laid out per-engine-friendly in _host_prep).

Precision (validated vs reference: rel err ~5e-3 against the 2e-2 gate):
  - generic path: fp16 matmul operands everywhere, fp32 PSUM accumulation
  - reader path (weight 0.1 in the prob combine): fp8e4 operands with
    DoubleRow matmuls (0.5 cyc/row); its exp is a Schraudolph bit-trick on
    DVE (scores*1/ln2 + 55.2 rounded to uint8 = e4m3 bits of exp(scores/8)),
    consistent numerator/denominator so the approximation bias cancels in
    the softmax normalization.

Structure:
  - q/k produced in [d, t] layout (d on partitions): per head-pair (gen,
    fp16) / head-quad (rdr, fp8 DR with 32-row subtiles at tile_position)
    projection chains, interleaved into the attention loop as PE filler
    ("pump") with per-boundary forced drains.
  - scores per (head, s-block): gen [128,1024] psum tiles (2-deep pool),
    rdr th-split [128,512] tiles in a shared 4-deep "small" pool that also
    carries av accumulators, projection chains, and deferred v-proj.
  - attention transposed: stationary = exp tile [s,t], moving = [v | 1/w]
    so attn lands [t, d] with the softmax denominator Z/w in the extra
    column; one ACT copy bounces the accumulator to SBUF, gpsimd
    normalize_recip applies w/Z per path, gpsimd add combines gen+rdr.
  - attn [t, e] tiles are DMA-transposed (xbar) per (pair, t-block) into
    [e, t] for the fp16 output projection; v-bias is folded into the output
    bias on host (combined prob rows sum to 0.5).
  - AV/combine of head h-1 is software-pipelined into head h's score loop;
    v-projection for pairs 6-7 is deferred into the pump as late filler.
"""

import os
import sys

for _p in ("/opt/trn_rl_repo",):
    if os.path.isdir(_p) and _p not in sys.path:
        sys.path.insert(0, _p)

import numpy as np

import concourse.bass as bass
import concourse.mybir as mybir
from concourse import bacc
from concourse.tile import TileContext
from concourse.bass_utils import run_bass_kernel_spmd

B, T, E, H, G = 8, 1024, 1024, 16, 4
D = E // H  # 64
SCALING = float(D) ** -0.5
W_G = 0.9 / 2.0
W_R = 0.1 / 2.0
EO = 8
SO = 8
TB = 8
NP = 8  # head pairs
NQ = 4  # head quads

F32 = mybir.dt.float32
F16 = mybir.dt.float16
F8 = mybir.dt.float8e4
U8 = mybir.dt.uint8
DRM = mybir.MatmulPerfMode.DoubleRow
EXP = mybir.ActivationFunctionType.Exp
MULT = mybir.AluOpType.mult
ADD = mybir.AluOpType.add

# Schraudolph constants for e4m3 bits: bits = round(score*A + Bc)
SCH_A = SCALING * 8.0 / float(np.log(2.0))
SCH_B = 56.0 - 0.8

USE_RDR_SCORE_DR = True  # DoubleRow with 32-row subtiles for reader scores


def build_nc():
    nc = bacc.Bacc(name="author_group_attention_v2")

    hsT16 = nc.dram_tensor("hsT16", [E, T], F16, kind="ExternalInput")
    hsT8 = nc.dram_tensor("hsT8", [E, T], F8, kind="ExternalInput")
    wg = nc.dram_tensor("wg", [NP, 128, 2, EO, 128], F16, kind="ExternalInput")
    w8 = nc.dram_tensor("w8", [NQ, 128, 2, 2, EO, 128], F8, kind="ExternalInput")
    wv = nc.dram_tensor("wv", [128, EO, E], F16, kind="ExternalInput")
    wo = nc.dram_tensor("wo", [TB, 128, NP, 128], F16, kind="ExternalInput")
    gbias = nc.dram_tensor("gbias", [128, 2, NP], F32, kind="ExternalInput")
    rbias = nc.dram_tensor("rbias", [128, 2, NQ, 2], F32, kind="ExternalInput")
    bo = nc.dram_tensor("bo", [128, TB], F32, kind="ExternalInput")
    outT = nc.dram_tensor("outT", [E, T], F16, kind="ExternalOutput")

    hsT16_r = hsT16.rearrange("(eo p) t -> p eo t", p=128)
    hsT8_r = hsT8.rearrange("(eo p) t -> p eo t", p=128)


    with TileContext(nc) as tc:
        from contextlib import ExitStack

        with ExitStack() as stack:
            const = stack.enter_context(tc.tile_pool(name="const", bufs=1))

            hsT16_sb = const.tile([128, EO, T], F16, tag="hsT16")
            hs8_sb = const.tile([128, EO, T], F8, tag="hs8")
            v16_sb = const.tile([128, SO, H, 66], F16, tag="v16")
            wv_sb = const.tile([128, EO, E], F16, tag="wv")
            v8_sb = const.tile([128, SO, H, 66], F8, tag="v8")
            gbias_sb = const.tile([128, 2, NP], F32, tag="gbias")
            rbias_sb = const.tile([128, 2, NQ, 2], F32, tag="rbias")
            bo_sb = const.tile([128, TB], F32, tag="bo")
            comb_tiles = [
                const.tile([128, E], F16, tag=f"comb{tb}", name=f"comb{tb}")
                for tb in range(TB)
            ]
            attnT_tiles = [
                const.tile([128, T], F16, tag=f"attnT{p}", name=f"attnT{p}")
                for p in range(NP)
            ]

            # ones columns pre-scaled by 1/w so reciprocal gives w/Z
            nc.vector.memset(v16_sb[:, :, :, 64:65], 1.0 / W_G)
            nc.vector.memset(v8_sb[:, :, :, 64:65], 1.0 / W_R)


            # persistent pools used across prologue + main
            qk16p = stack.enter_context(tc.tile_pool(name="qk16", bufs=2))
            qk8p = stack.enter_context(tc.tile_pool(name="qk8", bufs=2))
            wgp = stack.enter_context(tc.tile_pool(name="wgp", bufs=2))
            w8p = stack.enter_context(tc.tile_pool(name="w8p", bufs=2))

            def gen_proj_steps(pair, pp2_pool):
                """Generic q/k projection for a head pair -> list of step
                closures (PE matmuls + ACT drains). Weight DMAs fire now."""
                steps = []
                outs = {}
                wt2 = wgp.tile([128, 2, EO, 128], F16, tag="wg")
                nc.sync.dma_start(wt2[:], wg[pair])
                for qk in range(2):
                    wt = wt2[:, qk]
                    dst = qk16p.tile([128, T], F16, tag=f"qk16_{qk}")
                    outs[qk] = dst
                    state = {}

                    def mk(qk, wt, dst, state, th, eo):
                        def step():
                            if eo == 0:
                                state[th] = pp2_pool.tile(
                                    [128, 512], F32, tag="sm", name=f"gp{pair}{qk}{th}"
                                )
                            nc.tensor.matmul(
                                state[th][:],
                                wt[:, eo, :],
                                hsT16_sb[:, eo, th * 512 : (th + 1) * 512],
                                start=(eo == 0),
                                stop=(eo == EO - 1),
                            )
                            if eo == EO - 1:
                                if th == 0:
                                    nc.scalar.activation(
                                        dst[:, 0:512],
                                        state[th][:],
                                        mybir.ActivationFunctionType.Identity,
                                        bias=gbias_sb[:, qk, pair : pair + 1],
                                    )
                                else:
                                    nc.vector.tensor_scalar_add(
                                        dst[:, 512:1024],
                                        state[th][:],
                                        gbias_sb[:, qk, pair : pair + 1],
                                    )
                        return step

                    for th in range(2):
                        for eo in range(EO):
                            steps.append(mk(qk, wt, dst, state, th, eo))
                return outs, steps

            def vproj_steps(pair, pool, tag):
                steps = []
                state = {}
                ocols = slice(pair * 128, (pair + 1) * 128)

                def mk(sb, eo):
                    def step():
                        if eo == 0:
                            state[sb] = pool.tile(
                                [128, 512], F32, tag=tag, name=f"pv{pair}_{sb}"
                            )
                        nc.tensor.matmul(
                            state[sb][:, 0:128],
                            hsT16_sb[:, eo, sb * 128 : (sb + 1) * 128],
                            wv_sb[:, eo, ocols],
                            start=(eo == 0),
                            stop=(eo == EO - 1),
                        )
                        if eo == EO - 1:
                            pv_r = state[sb][:, 0:128].rearrange(
                                "p (hh dd) -> p hh dd", dd=64
                            )
                            vsl = v16_sb[:, sb, 2 * pair : 2 * pair + 2, 0:64]
                            nc.vector.tensor_copy(vsl, pv_r)
                            nc.gpsimd.tensor_copy(
                                v8_sb[:, sb, 2 * pair : 2 * pair + 2, 0:64], vsl
                            )
                    return step

                for sb in range(SO):
                    for eo in range(EO):
                        steps.append(mk(sb, eo))
                return steps

            def rdr_proj_steps(quad, pp2_pool):
                """Reader q/k projection for a head quad (fp8 DoubleRow).
                Outputs supertiles [128(=4h x 32d), 2(d-half), T] fp8."""
                steps = []
                outs = {}
                wt4 = w8p.tile([128, 2, 2, EO, 128], F8, tag="w8")
                nc.sync.dma_start(wt4[:], w8[quad])
                for qk in range(2):
                    dst = qk8p.tile([128, 2, T], F8, tag=f"qk8_{qk}")
                    outs[qk] = dst
                    for ab in range(2):
                        wt = wt4[:, qk, ab]
                        state = {}

                        def mk(qk, ab, wt, dst, state, th, a):
                            def step():
                                if a == 0:
                                    state[th] = pp2_pool.tile(
                                        [128, 512], F32, tag="sm",
                                        name=f"rp{quad}{qk}{ab}{th}",
                                    )
                                nc.tensor.matmul(
                                    state[th][:],
                                    wt[:, 2 * a : 2 * a + 2, :],
                                    hs8_sb[:, 2 * a : 2 * a + 2,
                                           th * 512 : (th + 1) * 512],
                                    start=(a == 0),
                                    stop=(a == 3),
                                    perf_mode=DRM,
                                )
                                if a == 3:
                                    if th == 0:
                                        nc.vector.tensor_scalar_add(
                                            dst[:, ab, 0:512],
                                            state[th][:],
                                            rbias_sb[:, qk, quad, ab : ab + 1],
                                        )
                                    else:
                                        nc.scalar.activation(
                                            dst[:, ab, 512:1024],
                                            state[th][:],
                                            mybir.ActivationFunctionType.Identity,
                                            bias=rbias_sb[:, qk, quad, ab : ab + 1],
                                        )
                            return step

                        for th in range(2):
                            for a in range(4):
                                steps.append(mk(qk, ab, wt, dst, state, th, a))
                return outs, steps

            # ---------------- prologue: v proj + first projections ----------
            pump = []
            with tc.tile_pool(
                name="vps", bufs=2, space="PSUM"
            ) as vps, tc.tile_pool(name="pps", bufs=2, space="PSUM") as pps:
                # weight DMAs first: tiny vs the 5MB input stream, and the
                # prologue proj pump stalls without them
                qk0, steps_g0 = gen_proj_steps(0, pps)
                r0, steps_r0 = rdr_proj_steps(0, pps)
                for a, b in ((0, 1), (1, 2), (2, 4), (4, 6), (6, 8)):
                    nc.sync.dma_start(hsT16_sb[:, a:b], hsT16_r[:, a:b])
                    nc.sync.dma_start(wv_sb[:, a:b, 0:512], wv[:, a:b, 0:512])
                    if a == 2:
                        nc.sync.dma_start(gbias_sb[:], gbias[:])
                        nc.sync.dma_start(rbias_sb[:], rbias[:])
                        nc.sync.dma_start(bo_sb[:], bo[:])
                for a, b in ((0, 4), (4, 8)):
                    nc.sync.dma_start(hs8_sb[:, a:b], hsT8_r[:, a:b])
                # wv columns for the deferred v-proj pairs 6-7 (needed h>=9)
                nc.sync.dma_start(wv_sb[:, :, 512:1024], wv[:, :, 512:1024])
                pro_pump = steps_g0 + steps_r0

                for sb in range(SO):
                    pv = vps.tile([128, E], F32, tag="pv", name=f"pv{sb}")
                    for eo in range(EO):
                        nc.tensor.matmul(
                            pv[:, 0:512],
                            hsT16_sb[:, eo, sb * 128 : (sb + 1) * 128],
                            wv_sb[:, eo, 0:512],
                            start=(eo == 0),
                            stop=(eo == EO - 1),
                        )
                        for _ in range(2 if sb >= 4 else 1):
                            if pro_pump:
                                pro_pump.pop(0)()
                    pv_r = pv[:, 0:512].rearrange("p (hh dd) -> p hh dd", dd=64)
                    nc.scalar.copy(v16_sb[:, sb, 0:8, 0:64], pv_r)
                    nc.vector.tensor_copy(v8_sb[:, sb, 0:8, 0:64], pv_r)
                while pro_pump:
                    pro_pump.pop(0)()

            # ---------------- main attention loop ---------------------------
            # Software pipeline: the AV/combine work of head h-1 is emitted
            # interleaved into the scores/exp loop of head h, so the PE has
            # filler work while ACT/DVE drain the score tiles.
            wo_sb = const.tile([128, TB, NP, 128], F16, tag="wo_all")
            for j in range(TB):
                nc.sync.dma_start(wo_sb[:, j], wo[j])

            with ExitStack() as mstack:
                scp = mstack.enter_context(
                    tc.tile_pool(name="scp", bufs=2, space="PSUM")
                )
                smallp = mstack.enter_context(
                    tc.tile_pool(name="smallp", bufs=4, space="PSUM")
                )
                ex16p = mstack.enter_context(tc.tile_pool(name="ex16", bufs=2))
                ex8p = mstack.enter_context(tc.tile_pool(name="ex8", bufs=2))
                tmpp = mstack.enter_context(tc.tile_pool(name="tmpp", bufs=8))
                avsp = mstack.enter_context(tc.tile_pool(name="avsp", bufs=6))

                def av_steps(h, ex16, ex8):
                    """AV + combine for head h as a list of step closures.
                    Each tb yields 3 steps: gen-av mms, rdr-av mms, combine."""
                    pair, hp = h // 2, h % 2
                    vg = v16_sb[:, :, h, :]
                    v8h = v8_sb[:, :, h, :]
                    steps = []
                    state = {}

                    def mk_gen(tb):
                        def step():
                            tsl = slice(tb * 128, (tb + 1) * 128)
                            av = smallp.tile([128, 512], F32, tag="sm",
                                             name=f"av{h}_{tb}")
                            state[tb] = av
                            for a in range(SO):
                                nc.tensor.matmul(
                                    av[:, 0:65],
                                    ex16[:, a, tsl],
                                    vg[:, a, 0:65],
                                    start=(a == 0),
                                    stop=(a == SO - 1),
                                )
                        return step

                    def mk_rdr(tb):
                        def step():
                            tsl = slice(tb * 128, (tb + 1) * 128)
                            av = state[tb]
                            for a in range(4):
                                nc.tensor.matmul(
                                    av[:, 68:133],
                                    ex8[:, 2 * a : 2 * a + 2, tsl],
                                    v8h[:, 2 * a : 2 * a + 2, 0:65],
                                    start=(a == 0),
                                    stop=(a == 3),
                                    perf_mode=DRM,
                                )
                        return step

                    def mk_comb(tb):
                        def step():
                            av = state.pop(tb)
                            avs = avsp.tile([128, 133], F32, tag="avs")
                            nc.scalar.copy(avs[:], av[:, 0:133])
                            csl = slice(h * 64, h * 64 + 64)
                            tmp = tmpp.tile([128, 64], F16, tag="tmp")
                            nc.gpsimd.normalize_recip(
                                comb_tiles[tb][:, csl], avs[:, 0:64],
                                avs[:, 64:65],
                            )
                            nc.gpsimd.normalize_recip(
                                tmp[:], avs[:, 68:132], avs[:, 132:133],
                            )
                            nc.gpsimd.tensor_tensor(
                                comb_tiles[tb][:, csl],
                                comb_tiles[tb][:, csl],
                                tmp[:],
                                ADD,
                            )
                            if hp == 1:
                                nc.sync.dma_start_transpose(
                                    attnT_tiles[pair][
                                        :, tb * 128 : (tb + 1) * 128
                                    ],
                                    comb_tiles[tb][
                                        :, pair * 128 : (pair + 1) * 128
                                    ],
                                )
                        return step

                    for tb in range(TB):
                        steps.append(mk_gen(tb))
                        steps.append(mk_rdr(tb))
                        steps.append(mk_comb(tb))
                    return steps

                Qg = Kg = Q8 = K8 = None
                av_q = []  # pending av steps of the previous head

                def fill(n):
                    """Emit up to n units of filler: av steps take priority
                    (they unblock comb tiles), then proj pump steps."""
                    for _ in range(n):
                        if av_q:
                            av_q.pop(0)()
                        elif pump:
                            pump.pop(0)()

                for h in range(H):
                    pair, quad = h // 2, h // 4
                    hp, hq = h % 2, h % 4
                    if h == 0:
                        Qg, Kg = qk0[0], qk0[1]
                        Q8, K8 = r0[0], r0[1]
                        nxt_g = nxt_r = None
                    if h in (5, 7, 9, 11):
                        pump.extend(vproj_steps((h - 5) // 2 + 4, smallp, "sm"))
                    if hp == 0 and pair + 1 < NP:
                        nxt_g, s = gen_proj_steps(pair + 1, smallp)
                        pump.extend(s)
                    if hq == 0 and quad + 1 < NQ:
                        nxt_r, s = rdr_proj_steps(quad + 1, smallp)
                        pump.extend(s)

                    ex16 = ex16p.tile([128, SO, T], F16, tag="ex16")
                    ex8 = ex8p.tile([128, SO, T], F8, tag="ex8")

                    grow = slice(64 * hp, 64 * hp + 64)
                    rrow = slice(32 * hq, 32 * hq + 32)

                    for sb in range(SO):
                        ssl = slice(sb * 128, (sb + 1) * 128)
                        sc = scp.tile([128, T], F32, tag="sc", name=f"sc{h}_{sb}")
                        for th in range(2):
                            nc.tensor.matmul(
                                sc[:, th * 512 : (th + 1) * 512],
                                Kg[grow, ssl],
                                Qg[grow, th * 512 : (th + 1) * 512],
                                start=True, stop=True,
                            )
                        nc.scalar.activation(
                            ex16[:, sb, :], sc[:], EXP, scale=SCALING
                        )
                        fill(2)
                        for th in range(2):
                            rc = smallp.tile([128, 512], F32, tag="sm",
                                             name=f"rc{h}_{sb}_{th}")
                            if USE_RDR_SCORE_DR:
                                nc.tensor.matmul(
                                    rc[:],
                                    K8[rrow, :, ssl],
                                    Q8[rrow, :, th * 512 : (th + 1) * 512],
                                    start=True, stop=True,
                                    perf_mode=DRM,
                                    tile_position=(32 * hq, 0),
                                )
                            else:
                                for ab in range(2):
                                    nc.tensor.matmul(
                                        rc[:],
                                        K8[rrow, ab, ssl],
                                        Q8[rrow, ab, th * 512 : (th + 1) * 512],
                                        start=(ab == 0), stop=(ab == 1),
                                        tile_position=(32 * hq, 0),
                                    )
                            nc.vector.tensor_scalar(
                                ex8[:, sb, th * 512 : (th + 1) * 512].bitcast(U8),
                                rc[:],
                                SCH_A, SCH_B, MULT, ADD,
                            )
                            fill(1)
                        fill(1)

                    # queue this head's av work; emitted during later heads'
                    # scores loops via fill()
                    av_q.extend(av_steps(h, ex16, ex8))

                    if hp == 1:
                        while pump:
                            pump.pop(0)()
                        if nxt_g is not None:
                            Qg, Kg = nxt_g[0], nxt_g[1]
                            nxt_g = None
                        if hq == 3 and nxt_r is not None:
                            Q8, K8 = nxt_r[0], nxt_r[1]
                            nxt_r = None

                while av_q:
                    av_q.pop(0)()

            # ---------------- output projection -----------------------------
            with tc.tile_pool(name="ops", bufs=6, space="PSUM") as ops, \
                 tc.tile_pool(name="o16p", bufs=4) as o16p:
                # 4 half-chain accumulators live at once; prefix over pairs
                # 0..6 depends only on heads <= 13 so it overlaps the final
                # head's av/combine; the pair-7 step + drain go in wave 2.
                halves = [(j, th) for j in range(TB) for th in range(2)]
                pos = {}
                for w0 in range(0, 16, 4):
                    for j, th in halves[w0 : w0 + 4]:
                        tsl = slice(th * 512, (th + 1) * 512)
                        po = ops.tile([128, 512], F32, tag="po",
                                      name=f"po{j}_{th}")
                        pos[(j, th)] = po
                        for pr in range(NP - 1):
                            nc.tensor.matmul(
                                po[:],
                                wo_sb[:, j, pr, :],
                                attnT_tiles[pr][:, tsl],
                                start=(pr == 0),
                                stop=False,
                            )
                    for j, th in halves[w0 : w0 + 4]:
                        tsl = slice(th * 512, (th + 1) * 512)
                        po = pos.pop((j, th))
                        nc.tensor.matmul(
                            po[:],
                            wo_sb[:, j, NP - 1, :],
                            attnT_tiles[NP - 1][:, tsl],
                            start=False,
                            stop=True,
                        )
                        o16 = o16p.tile([128, 512], F16, tag="o16")
                        if th == 0:
                            nc.vector.tensor_scalar_add(
                                o16[:], po[:], bo_sb[:, j : j + 1]
                            )
                        else:
                            nc.scalar.activation(
                                o16[:], po[:],
                                mybir.ActivationFunctionType.Identity,
                                bias=bo_sb[:, j : j + 1],
                            )
                        nc.sync.dma_start(
                            outT[j * 128 : (j + 1) * 128, tsl], o16[:]
                        )

    nc.finalize()
    return nc


_NC_CACHE = {}


def get_nc():
    if "nc" not in _NC_CACHE:
        _NC_CACHE["nc"] = build_nc()
    return _NC_CACHE["nc"]


def _host_prep(hidden_states, reader_token, Wq, bq, Wk, bk, Wv, bv, Wo, bo,
               RWq, Rbq, RWk, Rbk, RWv, Rbv):
    f = np.float32
    np16 = mybir.dt.np(F16)
    np8 = mybir.dt.np(F8)
    hs = np.asarray(hidden_states, f)
    tok = np.asarray(reader_token).astype(np.int64)

    WqT = np.asarray(Wq, f).T  # [e, o]
    WkT = np.asarray(Wk, f).T
    WvT = np.asarray(Wv, f).T
    WoT = np.asarray(Wo, f).T
    bq = np.asarray(bq, f); bk = np.asarray(bk, f)
    bv = np.asarray(bv, f); bo_ = np.asarray(bo, f)
    Rbq = np.asarray(Rbq, f); Rbk = np.asarray(Rbk, f)

    # gen weights [NP, 128, 2, EO, 128]
    wg_arr = np.empty((NP, 128, 2, EO, 128), np16)
    for qk, WT in enumerate((WqT, WkT)):
        r = WT.reshape(EO, 128, NP, 128)  # (eo, p, pair, m)
        wg_arr[:, :, qk] = r.transpose(2, 1, 0, 3).astype(np16)
    # gen biases [128, 2, NP]
    gb = np.empty((128, 2, NP), f)
    for qk, bb in enumerate((bq, bk)):
        gb[:, qk, :] = bb.reshape(NP, 128).T

    # v-bias folds into output bias (probs rows sum to 0.5)
    bo_eff = bo_ + 0.5 * (np.asarray(Wo, f) @ bv)
    bo_t = np.ascontiguousarray(bo_eff.reshape(TB, 128).T)

    # wv [128, EO, E]
    wv_arr = np.ascontiguousarray(
        WvT.reshape(EO, 128, E).transpose(1, 0, 2)
    ).astype(np16)
    # wo [TB, 128, NP, 128]
    wo_arr = np.ascontiguousarray(
        WoT.reshape(NP, 128, TB, 128).transpose(2, 1, 0, 3)
    ).astype(np16)

    percore = {}
    in_maps = []
    for b in range(B):
        g = int(tok[b])
        if g not in percore:
            RWqT = np.asarray(RWq[g], f).T  # [e, o]
            RWkT = np.asarray(RWk[g], f).T
            w8_arr = np.empty((NQ, 128, 2, 2, EO, 128), np8)
            for qk, WT in enumerate((RWqT, RWkT)):
                # o = (quad*4 + m//32)*64 + ab*32 + m%32
                r = WT.reshape(EO, 128, NQ, 4, 2, 32)  # (eo,p,quad,hin,ab,dd)
                # -> (quad, p, qk, ab, eo, m=(hin,dd))
                w8_arr[:, :, qk] = r.transpose(2, 4, 1, 0, 3, 5).reshape(
                    NQ, 2, 128, EO, 128
                ).astype(np8).transpose(0, 2, 1, 3, 4).reshape(NQ, 128, 2, EO, 128)
            rb = np.empty((128, 2, NQ, 2), f)
            for qk, bb in enumerate((Rbq[g], Rbk[g])):
                # p = hin*32 + dd ; value = b[(quad*4+hin)*64 + ab*32 + dd]
                r = bb.reshape(NQ, 4, 2, 32)  # (quad, hin, ab, dd)
                rb[:, qk, :, :] = r.transpose(1, 3, 0, 2).reshape(128, NQ, 2)
            percore[g] = (w8_arr, rb)
        w8_arr, rb = percore[g]
        hsT = np.ascontiguousarray(hs[b].T)
        in_maps.append(
            {
                "hsT16": hsT.astype(np16),
                "hsT8": hsT.astype(np8),
                "wg": wg_arr,
                "w8": w8_arr,
                "wv": wv_arr,
                "wo": wo_arr,
                "gbias": gb,
                "rbias": rb,
                "bo": bo_t,
            }
        )
    return in_maps


def kernel(**inputs) -> np.ndarray:
    in_maps = _host_prep(**inputs)
    nc = get_nc()
    res = run_bass_kernel_spmd(nc, in_maps, list(range(B)))
    out = np.stack(
        [np.asarray(res.results[c]["outT"]).astype(np.float32).T for c in range(B)],
        axis=0,
    )
    return np.ascontiguousarray(out)


# revision 10
# speedup vs baseline: 1.1173x; 1.0027x over previous
"""AuthorGroupAttention Trainium2 kernel.

Data-parallel over batch: 8 samples -> 8 NeuronCores, one sample per core.
Routing resolved on host (per-core reader-group weights gathered, cast and
laid out per-engine-friendly in _host_prep).

Precision (validated vs reference: rel err ~5e-3 against the 2e-2 gate):
  - generic path: fp16 matmul operands everywhere, fp32 PSUM accumulation
  - reader path (weight 0.1 in the prob combine): fp8e4 operands with
    DoubleRow matmuls (0.5 cyc/row); its exp is a Schraudolph bit-trick on
    DVE (scores*1/ln2 + 55.2 rounded to uint8 = e4m3 bits of exp(scores/8)),
    consistent numerator/denominator so the approximation bias cancels in
    the softmax normalization.

Structure:
  - q/k produced in [d, t] layout (d on partitions): per head-pair (gen,
    fp16) / head-quad (rdr, fp8 DR with 32-row subtiles at tile_position)
    projection chains, interleaved into the attention loop as PE filler
    ("pump") with per-boundary forced drains.
  - scores per (head, s-block): gen [128,1024] psum tiles (2-deep pool),
    rdr th-split [128,512] tiles in a shared 4-deep "small" pool that also
    carries av accumulators, projection chains, and deferred v-proj.
  - attention transposed: stationary = exp tile [s,t], moving = [v | 1/w]
    so attn lands [t, d] with the softmax denominator Z/w in the extra
    column; one ACT copy bounces the accumulator to SBUF, gpsimd
    normalize_recip applies w/Z per path, gpsimd add combines gen+rdr.
  - attn [t, e] tiles are DMA-transposed (xbar) per (pair, t-block) into
    [e, t] for the fp16 output projection; v-bias is folded into the output
    bias on host (combined prob rows sum to 0.5).
  - AV/combine of head h-1 is software-pipelined into head h's score loop;
    v-projection for pairs 6-7 is deferred into the pump as late filler.
"""

import os
import sys

for _p in ("/opt/trn_rl_repo",):
    if os.path.isdir(_p) and _p not in sys.path:
        sys.path.insert(0, _p)

import numpy as np

import concourse.bass as bass
import concourse.mybir as mybir
from concourse import bacc
from concourse.tile import TileContext
from concourse.bass_utils import run_bass_kernel_spmd

B, T, E, H, G = 8, 1024, 1024, 16, 4
D = E // H  # 64
SCALING = float(D) ** -0.5
W_G = 0.9 / 2.0
W_R = 0.1 / 2.0
EO = 8
SO = 8
TB = 8
NP = 8  # head pairs
NQ = 4  # head quads

F32 = mybir.dt.float32
F16 = mybir.dt.float16
F8 = mybir.dt.float8e4
U8 = mybir.dt.uint8
DRM = mybir.MatmulPerfMode.DoubleRow
EXP = mybir.ActivationFunctionType.Exp
MULT = mybir.AluOpType.mult
ADD = mybir.AluOpType.add

# Schraudolph constants for e4m3 bits: bits = round(score*A + Bc)
SCH_A = SCALING * 8.0 / float(np.log(2.0))
SCH_B = 56.0 - 0.8

USE_RDR_SCORE_DR = True  # DoubleRow with 32-row subtiles for reader scores


def build_nc():
    nc = bacc.Bacc(name="author_group_attention_v2")

    hsT16 = nc.dram_tensor("hsT16", [E, T], F16, kind="ExternalInput")
    hsT8 = nc.dram_tensor("hsT8", [E, T], F8, kind="ExternalInput")
    wg = nc.dram_tensor("wg", [NP, 128, 2, EO, 128], F16, kind="ExternalInput")
    w8 = nc.dram_tensor("w8", [NQ, 128, 2, 2, EO, 128], F8, kind="ExternalInput")
    wv = nc.dram_tensor("wv", [128, EO, E], F16, kind="ExternalInput")
    wo = nc.dram_tensor("wo", [TB, 128, NP, 128], F16, kind="ExternalInput")
    gbias = nc.dram_tensor("gbias", [128, 2, NP], F32, kind="ExternalInput")
    rbias = nc.dram_tensor("rbias", [128, 2, NQ, 2], F32, kind="ExternalInput")
    bo = nc.dram_tensor("bo", [128, TB], F32, kind="ExternalInput")
    outT = nc.dram_tensor("outT", [E, T], F16, kind="ExternalOutput")

    hsT16_r = hsT16.rearrange("(eo p) t -> p eo t", p=128)
    hsT8_r = hsT8.rearrange("(eo p) t -> p eo t", p=128)


    with TileContext(nc) as tc:
        from contextlib import ExitStack

        with ExitStack() as stack:
            const = stack.enter_context(tc.tile_pool(name="const", bufs=1))

            hsT16_sb = const.tile([128, EO, T], F16, tag="hsT16")
            hs8_sb = const.tile([128, EO, T], F8, tag="hs8")
            v16_sb = const.tile([128, SO, H, 66], F16, tag="v16")
            wv_sb = const.tile([128, EO, E], F16, tag="wv")
            v8_sb = const.tile([128, SO, H, 66], F8, tag="v8")
            gbias_sb = const.tile([128, 2, NP], F32, tag="gbias")
            rbias_sb = const.tile([128, 2, NQ, 2], F32, tag="rbias")
            bo_sb = const.tile([128, TB], F32, tag="bo")
            comb_tiles = [
                const.tile([128, E], F16, tag=f"comb{tb}", name=f"comb{tb}")
                for tb in range(TB)
            ]
            attnT_tiles = [
                const.tile([128, T], F16, tag=f"attnT{p}", name=f"attnT{p}")
                for p in range(NP)
            ]

            # ones columns pre-scaled by 1/w so reciprocal gives w/Z
            nc.vector.memset(v16_sb[:, :, :, 64:65], 1.0 / W_G)
            nc.vector.memset(v8_sb[:, :, :, 64:65], 1.0 / W_R)


            # persistent pools used across prologue + main
            qk16p = stack.enter_context(tc.tile_pool(name="qk16", bufs=2))
            qk8p = stack.enter_context(tc.tile_pool(name="qk8", bufs=2))
            wgp = stack.enter_context(tc.tile_pool(name="wgp", bufs=2))
            w8p = stack.enter_context(tc.tile_pool(name="w8p", bufs=2))

            def gen_proj_steps(pair, pp2_pool):
                """Generic q/k projection for a head pair -> list of step
                closures (PE matmuls + ACT drains). Weight DMAs fire now."""
                steps = []
                outs = {}
                wt2 = wgp.tile([128, 2, EO, 128], F16, tag="wg")
                nc.sync.dma_start(wt2[:], wg[pair])
                for qk in range(2):
                    wt = wt2[:, qk]
                    dst = qk16p.tile([128, T], F16, tag=f"qk16_{qk}")
                    outs[qk] = dst
                    state = {}

                    def mk(qk, wt, dst, state, th, eo):
                        def step():
                            if eo == 0:
                                state[th] = pp2_pool.tile(
                                    [128, 512], F32, tag="sm", name=f"gp{pair}{qk}{th}"
                                )
                            nc.tensor.matmul(
                                state[th][:],
                                wt[:, eo, :],
                                hsT16_sb[:, eo, th * 512 : (th + 1) * 512],
                                start=(eo == 0),
                                stop=(eo == EO - 1),
                            )
                            if eo == EO - 1:
                                if th == 0:
                                    nc.scalar.activation(
                                        dst[:, 0:512],
                                        state[th][:],
                                        mybir.ActivationFunctionType.Identity,
                                        bias=gbias_sb[:, qk, pair : pair + 1],
                                    )
                                else:
                                    nc.vector.tensor_scalar_add(
                                        dst[:, 512:1024],
                                        state[th][:],
                                        gbias_sb[:, qk, pair : pair + 1],
                                    )
                        return step

                    for th in range(2):
                        for eo in range(EO):
                            steps.append(mk(qk, wt, dst, state, th, eo))
                return outs, steps

            def vproj_steps(pair, pool, tag):
                steps = []
                state = {}
                ocols = slice(pair * 128, (pair + 1) * 128)

                def mk(sb, eo):
                    def step():
                        if eo == 0:
                            state[sb] = pool.tile(
                                [128, 512], F32, tag=tag, name=f"pv{pair}_{sb}"
                            )
                        nc.tensor.matmul(
                            state[sb][:, 0:128],
                            hsT16_sb[:, eo, sb * 128 : (sb + 1) * 128],
                            wv_sb[:, eo, ocols],
                            start=(eo == 0),
                            stop=(eo == EO - 1),
                        )
                        if eo == EO - 1:
                            pv_r = state[sb][:, 0:128].rearrange(
                                "p (hh dd) -> p hh dd", dd=64
                            )
                            vsl = v16_sb[:, sb, 2 * pair : 2 * pair + 2, 0:64]
                            nc.vector.tensor_copy(vsl, pv_r)
                            nc.gpsimd.tensor_copy(
                                v8_sb[:, sb, 2 * pair : 2 * pair + 2, 0:64], vsl
                            )
                    return step

                for sb in range(SO):
                    for eo in range(EO):
                        steps.append(mk(sb, eo))
                return steps

            def rdr_proj_steps(quad, pp2_pool):
                """Reader q/k projection for a head quad (fp8 DoubleRow).
                Outputs supertiles [128(=4h x 32d), 2(d-half), T] fp8."""
                steps = []
                outs = {}
                wt4 = w8p.tile([128, 2, 2, EO, 128], F8, tag="w8")
                nc.sync.dma_start(wt4[:], w8[quad])
                for qk in range(2):
                    dst = qk8p.tile([128, 2, T], F8, tag=f"qk8_{qk}")
                    outs[qk] = dst
                    for ab in range(2):
                        wt = wt4[:, qk, ab]
                        state = {}

                        def mk(qk, ab, wt, dst, state, th, a):
                            def step():
                                if a == 0:
                                    state[th] = pp2_pool.tile(
                                        [128, 512], F32, tag="sm",
                                        name=f"rp{quad}{qk}{ab}{th}",
                                    )
                                nc.tensor.matmul(
                                    state[th][:],
                                    wt[:, 2 * a : 2 * a + 2, :],
                                    hs8_sb[:, 2 * a : 2 * a + 2,
                                           th * 512 : (th + 1) * 512],
                                    start=(a == 0),
                                    stop=(a == 3),
                                    perf_mode=DRM,
                                )
                                if a == 3:
                                    if th == 0:
                                        nc.vector.tensor_scalar_add(
                                            dst[:, ab, 0:512],
                                            state[th][:],
                                            rbias_sb[:, qk, quad, ab : ab + 1],
                                        )
                                    else:
                                        nc.scalar.activation(
                                            dst[:, ab, 512:1024],
                                            state[th][:],
                                            mybir.ActivationFunctionType.Identity,
                                            bias=rbias_sb[:, qk, quad, ab : ab + 1],
                                        )
                            return step

                        for th in range(2):
                            for a in range(4):
                                steps.append(mk(qk, ab, wt, dst, state, th, a))
                return outs, steps

            # ---------------- prologue: v proj + first projections ----------
            pump = []
            with tc.tile_pool(
                name="vps", bufs=2, space="PSUM"
            ) as vps, tc.tile_pool(name="pps", bufs=2, space="PSUM") as pps:
                # first chunk pair ahead of the 0.77MB weight DMAs so the
                # v-chains start immediately; weights next (prologue pump)
                nc.sync.dma_start(hsT16_sb[:, 0:1], hsT16_r[:, 0:1])
                nc.sync.dma_start(wv_sb[:, 0:1, 0:512], wv[:, 0:1, 0:512])
                qk0, steps_g0 = gen_proj_steps(0, pps)
                r0, steps_r0 = rdr_proj_steps(0, pps)
                for a, b in ((1, 2), (2, 4), (4, 6), (6, 8)):
                    nc.sync.dma_start(hsT16_sb[:, a:b], hsT16_r[:, a:b])
                    nc.sync.dma_start(wv_sb[:, a:b, 0:512], wv[:, a:b, 0:512])
                    if a == 2:
                        nc.sync.dma_start(gbias_sb[:], gbias[:])
                        nc.sync.dma_start(rbias_sb[:], rbias[:])
                        nc.sync.dma_start(bo_sb[:], bo[:])
                for a, b in ((0, 4), (4, 8)):
                    nc.sync.dma_start(hs8_sb[:, a:b], hsT8_r[:, a:b])
                # wv columns for the deferred v-proj pairs 6-7 (needed h>=9)
                nc.sync.dma_start(wv_sb[:, :, 512:1024], wv[:, :, 512:1024])
                pro_pump = steps_g0 + steps_r0

                for sb in range(SO):
                    pv = vps.tile([128, E], F32, tag="pv", name=f"pv{sb}")
                    for eo in range(EO):
                        nc.tensor.matmul(
                            pv[:, 0:512],
                            hsT16_sb[:, eo, sb * 128 : (sb + 1) * 128],
                            wv_sb[:, eo, 0:512],
                            start=(eo == 0),
                            stop=(eo == EO - 1),
                        )
                        for _ in range(2 if sb >= 4 else 1):
                            if pro_pump:
                                pro_pump.pop(0)()
                    pv_r = pv[:, 0:512].rearrange("p (hh dd) -> p hh dd", dd=64)
                    nc.scalar.copy(v16_sb[:, sb, 0:8, 0:64], pv_r)
                    nc.vector.tensor_copy(v8_sb[:, sb, 0:8, 0:64], pv_r)
                while pro_pump:
                    pro_pump.pop(0)()

            # ---------------- main attention loop ---------------------------
            # Software pipeline: the AV/combine work of head h-1 is emitted
            # interleaved into the scores/exp loop of head h, so the PE has
            # filler work while ACT/DVE drain the score tiles.
            wo_sb = const.tile([128, TB, NP, 128], F16, tag="wo_all")
            for j in range(TB):
                nc.sync.dma_start(wo_sb[:, j], wo[j])

            with ExitStack() as mstack:
                scp = mstack.enter_context(
                    tc.tile_pool(name="scp", bufs=2, space="PSUM")
                )
                smallp = mstack.enter_context(
                    tc.tile_pool(name="smallp", bufs=4, space="PSUM")
                )
                ex16p = mstack.enter_context(tc.tile_pool(name="ex16", bufs=2))
                ex8p = mstack.enter_context(tc.tile_pool(name="ex8", bufs=2))
                tmpp = mstack.enter_context(tc.tile_pool(name="tmpp", bufs=8))
                avsp = mstack.enter_context(tc.tile_pool(name="avsp", bufs=6))

                def av_steps(h, ex16, ex8):
                    """AV + combine for head h as a list of step closures.
                    Each tb yields 3 steps: gen-av mms, rdr-av mms, combine."""
                    pair, hp = h // 2, h % 2
                    vg = v16_sb[:, :, h, :]
                    v8h = v8_sb[:, :, h, :]
                    steps = []
                    state = {}

                    def mk_gen(tb):
                        def step():
                            tsl = slice(tb * 128, (tb + 1) * 128)
                            av = smallp.tile([128, 512], F32, tag="sm",
                                             name=f"av{h}_{tb}")
                            state[tb] = av
                            for a in range(SO):
                                nc.tensor.matmul(
                                    av[:, 0:65],
                                    ex16[:, a, tsl],
                                    vg[:, a, 0:65],
                                    start=(a == 0),
                                    stop=(a == SO - 1),
                                )
                        return step

                    def mk_rdr(tb):
                        def step():
                            tsl = slice(tb * 128, (tb + 1) * 128)
                            av = state[tb]
                            for a in range(4):
                                nc.tensor.matmul(
                                    av[:, 68:133],
                                    ex8[:, 2 * a : 2 * a + 2, tsl],
                                    v8h[:, 2 * a : 2 * a + 2, 0:65],
                                    start=(a == 0),
                                    stop=(a == 3),
                                    perf_mode=DRM,
                                )
                        return step

                    def mk_comb(tb):
                        def step():
                            av = state.pop(tb)
                            csl = slice(h * 64, h * 64 + 64)
                            tmp = tmpp.tile([128, 64], F16, tag="tmp")
                            avs = avsp.tile([128, 133], F32, tag="avs")
                            nc.scalar.copy(avs[:], av[:, 0:133])
                            nc.gpsimd.normalize_recip(
                                comb_tiles[tb][:, csl], avs[:, 0:64],
                                avs[:, 64:65],
                            )
                            nc.gpsimd.normalize_recip(
                                tmp[:], avs[:, 68:132], avs[:, 132:133],
                            )
                            nc.gpsimd.tensor_tensor(
                                comb_tiles[tb][:, csl],
                                comb_tiles[tb][:, csl],
                                tmp[:],
                                ADD,
                            )
                            if hp == 1:
                                nc.sync.dma_start_transpose(
                                    attnT_tiles[pair][
                                        :, tb * 128 : (tb + 1) * 128
                                    ],
                                    comb_tiles[tb][
                                        :, pair * 128 : (pair + 1) * 128
                                    ],
                                )
                        return step

                    for tb in range(TB):
                        steps.append(mk_gen(tb))
                        steps.append(mk_rdr(tb))
                        steps.append(mk_comb(tb))
                    return steps

                Qg = Kg = Q8 = K8 = None
                av_q = []  # pending av steps of the previous head

                def fill(n):
                    """Emit up to n units of filler: av steps take priority
                    (they unblock comb tiles), then proj pump steps."""
                    for _ in range(n):
                        if av_q:
                            av_q.pop(0)()
                        elif pump:
                            pump.pop(0)()

                for h in range(H):
                    pair, quad = h // 2, h // 4
                    hp, hq = h % 2, h % 4
                    if h == 0:
                        Qg, Kg = qk0[0], qk0[1]
                        Q8, K8 = r0[0], r0[1]
                        nxt_g = nxt_r = None
                    if h in (7, 9, 11, 13):
                        pump.extend(vproj_steps((h - 7) // 2 + 4, smallp, "sm"))
                    if hp == 0 and pair + 1 < NP:
                        nxt_g, s = gen_proj_steps(pair + 1, smallp)
                        pump.extend(s)
                    if hq == 0 and quad + 1 < NQ:
                        nxt_r, s = rdr_proj_steps(quad + 1, smallp)
                        pump.extend(s)

                    ex16 = ex16p.tile([128, SO, T], F16, tag="ex16")
                    ex8 = ex8p.tile([128, SO, T], F8, tag="ex8")

                    grow = slice(64 * hp, 64 * hp + 64)
                    rrow = slice(32 * hq, 32 * hq + 32)

                    for sb in range(SO):
                        ssl = slice(sb * 128, (sb + 1) * 128)
                        sc = scp.tile([128, T], F32, tag="sc", name=f"sc{h}_{sb}")
                        for th in range(2):
                            nc.tensor.matmul(
                                sc[:, th * 512 : (th + 1) * 512],
                                Kg[grow, ssl],
                                Qg[grow, th * 512 : (th + 1) * 512],
                                start=True, stop=True,
                            )
                        nc.scalar.activation(
                            ex16[:, sb, :], sc[:], EXP, scale=SCALING
                        )
                        fill(2)
                        for th in range(2):
                            rc = smallp.tile([128, 512], F32, tag="sm",
                                             name=f"rc{h}_{sb}_{th}")
                            if USE_RDR_SCORE_DR:
                                nc.tensor.matmul(
                                    rc[:],
                                    K8[rrow, :, ssl],
                                    Q8[rrow, :, th * 512 : (th + 1) * 512],
                                    start=True, stop=True,
                                    perf_mode=DRM,
                                    tile_position=(32 * hq, 0),
                                )
                            else:
                                for ab in range(2):
                                    nc.tensor.matmul(
                                        rc[:],
                                        K8[rrow, ab, ssl],
                                        Q8[rrow, ab, th * 512 : (th + 1) * 512],
                                        start=(ab == 0), stop=(ab == 1),
                                        tile_position=(32 * hq, 0),
                                    )
                            nc.vector.tensor_scalar(
                                ex8[:, sb, th * 512 : (th + 1) * 512].bitcast(U8),
                                rc[:],
                                SCH_A, SCH_B, MULT, ADD,
                            )
                            fill(1)
                        fill(1)

                    # queue this head's av work; emitted during later heads'
                    # scores loops via fill()
                    av_q.extend(av_steps(h, ex16, ex8))

                    if hp == 1:
                        while pump:
                            pump.pop(0)()
                        if nxt_g is not None:
                            Qg, Kg = nxt_g[0], nxt_g[1]
                            nxt_g = None
                        if hq == 3 and nxt_r is not None:
                            Q8, K8 = nxt_r[0], nxt_r[1]
                            nxt_r = None

                while av_q:
                    av_q.pop(0)()

            # ---------------- output projection -----------------------------
            with tc.tile_pool(name="ops", bufs=6, space="PSUM") as ops, \
                 tc.tile_pool(name="o16p", bufs=4) as o16p:
                # 4 half-chain accumulators live at once; prefix over pairs
                # 0..6 depends only on heads <= 13 so it overlaps the final
                # head's av/combine; the pair-7 step + drain go in wave 2.
                halves = [(j, th) for j in range(TB) for th in range(2)]
                pos = {}
                for w0 in range(0, 16, 4):
                    for j, th in halves[w0 : w0 + 4]:
                        tsl = slice(th * 512, (th + 1) * 512)
                        po = ops.tile([128, 512], F32, tag="po",
                                      name=f"po{j}_{th}")
                        pos[(j, th)] = po
                        for pr in range(NP - 1):
                            nc.tensor.matmul(
                                po[:],
                                wo_sb[:, j, pr, :],
                                attnT_tiles[pr][:, tsl],
                                start=(pr == 0),
                                stop=False,
                            )
                    for j, th in halves[w0 : w0 + 4]:
                        tsl = slice(th * 512, (th + 1) * 512)
                        po = pos.pop((j, th))
                        nc.tensor.matmul(
                            po[:],
                            wo_sb[:, j, NP - 1, :],
                            attnT_tiles[NP - 1][:, tsl],
                            start=False,
                            stop=True,
                        )
                        o16 = o16p.tile([128, 512], F16, tag="o16")
                        if th == 0:
                            nc.vector.tensor_scalar_add(
                                o16[:], po[:], bo_sb[:, j : j + 1]
                            )
                        else:
                            nc.scalar.activation(
                                o16[:], po[:],
                                mybir.ActivationFunctionType.Identity,
                                bias=bo_sb[:, j : j + 1],
                            )
                        nc.sync.dma_start(
                            outT[j * 128 : (j + 1) * 128, tsl], o16[:]
                        )

    nc.finalize()
    return nc


_NC_CACHE = {}


def get_nc():
    if "nc" not in _NC_CACHE:
        _NC_CACHE["nc"] = build_nc()
    return _NC_CACHE["nc"]


def _host_prep(hidden_states, reader_token, Wq, bq, Wk, bk, Wv, bv, Wo, bo,
               RWq, Rbq, RWk, Rbk, RWv, Rbv):
    f = np.float32
    np16 = mybir.dt.np(F16)
    np8 = mybir.dt.np(F8)
    hs = np.asarray(hidden_states, f)
    tok = np.asarray(reader_token).astype(np.int64)

    WqT = np.asarray(Wq, f).T  # [e, o]
    WkT = np.asarray(Wk, f).T
    WvT = np.asarray(Wv, f).T
    WoT = np.asarray(Wo, f).T
    bq = np.asarray(bq, f); bk = np.asarray(bk, f)
    bv = np.asarray(bv, f); bo_ = np.asarray(bo, f)
    Rbq = np.asarray(Rbq, f); Rbk = np.asarray(Rbk, f)

    # gen weights [NP, 128, 2, EO, 128]
    wg_arr = np.empty((NP, 128, 2, EO, 128), np16)
    for qk, WT in enumerate((WqT, WkT)):
        r = WT.reshape(EO, 128, NP, 128)  # (eo, p, pair, m)
        wg_arr[:, :, qk] = r.transpose(2, 1, 0, 3).astype(np16)
    # gen biases [128, 2, NP]
    gb = np.empty((128, 2, NP), f)
    for qk, bb in enumerate((bq, bk)):
        gb[:, qk, :] = bb.reshape(NP, 128).T

    # v-bias folds into output bias (probs rows sum to 0.5)
    bo_eff = bo_ + 0.5 * (np.asarray(Wo, f) @ bv)
    bo_t = np.ascontiguousarray(bo_eff.reshape(TB, 128).T)

    # wv [128, EO, E]
    wv_arr = np.ascontiguousarray(
        WvT.reshape(EO, 128, E).transpose(1, 0, 2)
    ).astype(np16)
    # wo [TB, 128, NP, 128]
    wo_arr = np.ascontiguousarray(
        WoT.reshape(NP, 128, TB, 128).transpose(2, 1, 0, 3)
    ).astype(np16)

    percore = {}
    in_maps = []
    for b in range(B):
        g = int(tok[b])
        if g not in percore:
            RWqT = np.asarray(RWq[g], f).T  # [e, o]
            RWkT = np.asarray(RWk[g], f).T
            w8_arr = np.empty((NQ, 128, 2, 2, EO, 128), np8)
            for qk, WT in enumerate((RWqT, RWkT)):
                # o = (quad*4 + m//32)*64 + ab*32 + m%32
                r = WT.reshape(EO, 128, NQ, 4, 2, 32)  # (eo,p,quad,hin,ab,dd)
                # -> (quad, p, qk, ab, eo, m=(hin,dd))
                w8_arr[:, :, qk] = r.transpose(2, 4, 1, 0, 3, 5).reshape(
                    NQ, 2, 128, EO, 128
                ).astype(np8).transpose(0, 2, 1, 3, 4).reshape(NQ, 128, 2, EO, 128)
            rb = np.empty((128, 2, NQ, 2), f)
            for qk, bb in enumerate((Rbq[g], Rbk[g])):
                # p = hin*32 + dd ; value = b[(quad*4+hin)*64 + ab*32 + dd]
                r = bb.reshape(NQ, 4, 2, 32)  # (quad, hin, ab, dd)
                rb[:, qk, :, :] = r.transpose(1, 3, 0, 2).reshape(128, NQ, 2)
            percore[g] = (w8_arr, rb)
        w8_arr, rb = percore[g]
        hsT = np.ascontiguousarray(hs[b].T)
        in_maps.append(
            {
                "hsT16": hsT.astype(np16),
                "hsT8": hsT.astype(np8),
                "wg": wg_arr,
                "w8": w8_arr,
                "wv": wv_arr,
                "wo": wo_arr,
                "gbias": gb,
                "rbias": rb,
                "bo": bo_t,
            }
        )
    return in_maps


def kernel(**inputs) -> np.ndarray:
    in_maps = _host_prep(**inputs)
    nc = get_nc()
    res = run_bass_kernel_spmd(nc, in_maps, list(range(B)))
    out = np.stack(
        [np.asarray(res.results[c]["outT"]).astype(np.float32).T for c in range(B)],
        axis=0,
    )
    return np.ascontiguousarray(out)


# revision 11
# speedup vs baseline: 1.1179x; 1.0006x over previous
"""AuthorGroupAttention Trainium2 kernel.

Data-parallel over batch: 8 samples -> 8 NeuronCores, one sample per core.
Routing resolved on host (per-core reader-group weights gathered, cast and
laid out per-engine-friendly in _host_prep).

Precision (validated vs reference: rel err ~5e-3 against the 2e-2 gate):
  - generic path: fp16 matmul operands everywhere, fp32 PSUM accumulation
  - reader path (weight 0.1 in the prob combine): fp8e4 operands with
    DoubleRow matmuls (0.5 cyc/row); its exp is a Schraudolph bit-trick on
    DVE (scores*1/ln2 + 55.2 rounded to uint8 = e4m3 bits of exp(scores/8)),
    consistent numerator/denominator so the approximation bias cancels in
    the softmax normalization.

Structure:
  - q/k produced in [d, t] layout (d on partitions): per head-pair (gen,
    fp16) / head-quad (rdr, fp8 DR with 32-row subtiles at tile_position)
    projection chains, interleaved into the attention loop as PE filler
    ("pump") with per-boundary forced drains.
  - scores per (head, s-block): gen [128,1024] psum tiles (2-deep pool),
    rdr th-split [128,512] tiles in a shared 4-deep "small" pool that also
    carries av accumulators, projection chains, and deferred v-proj.
  - attention transposed: stationary = exp tile [s,t], moving = [v | 1/w]
    so attn lands [t, d] with the softmax denominator Z/w in the extra
    column; one ACT copy bounces the accumulator to SBUF, gpsimd
    normalize_recip applies w/Z per path, gpsimd add combines gen+rdr.
  - attn [t, e] tiles are DMA-transposed (xbar) per (pair, t-block) into
    [e, t] for the fp16 output projection; v-bias is folded into the output
    bias on host (combined prob rows sum to 0.5).
  - AV/combine of head h-1 is software-pipelined into head h's score loop;
    v-projection for pairs 6-7 is deferred into the pump as late filler.
"""

import os
import sys

for _p in ("/opt/trn_rl_repo",):
    if os.path.isdir(_p) and _p not in sys.path:
        sys.path.insert(0, _p)

import numpy as np

import concourse.bass as bass
import concourse.mybir as mybir
from concourse import bacc
from concourse.tile import TileContext
from concourse.bass_utils import run_bass_kernel_spmd

B, T, E, H, G = 8, 1024, 1024, 16, 4
D = E // H  # 64
SCALING = float(D) ** -0.5
W_G = 0.9 / 2.0
W_R = 0.1 / 2.0
EO = 8
SO = 8
TB = 8
NP = 8  # head pairs
NQ = 4  # head quads

F32 = mybir.dt.float32
F16 = mybir.dt.float16
F8 = mybir.dt.float8e4
U8 = mybir.dt.uint8
DRM = mybir.MatmulPerfMode.DoubleRow
EXP = mybir.ActivationFunctionType.Exp
MULT = mybir.AluOpType.mult
ADD = mybir.AluOpType.add

# Schraudolph constants for e4m3 bits: bits = round(score*A + Bc)
SCH_A = SCALING * 8.0 / float(np.log(2.0))
SCH_B = 56.0 - 0.8

USE_RDR_SCORE_DR = True  # DoubleRow with 32-row subtiles for reader scores


def build_nc():
    nc = bacc.Bacc(name="author_group_attention_v2")

    hsT16 = nc.dram_tensor("hsT16", [E, T], F16, kind="ExternalInput")
    hsT8 = nc.dram_tensor("hsT8", [E, T], F8, kind="ExternalInput")
    wg = nc.dram_tensor("wg", [NP, 128, 2, EO, 128], F16, kind="ExternalInput")
    w8 = nc.dram_tensor("w8", [NQ, 128, 2, 2, EO, 128], F8, kind="ExternalInput")
    wv = nc.dram_tensor("wv", [128, EO, E], F16, kind="ExternalInput")
    wo = nc.dram_tensor("wo", [TB, 128, NP, 128], F16, kind="ExternalInput")
    gbias = nc.dram_tensor("gbias", [128, 2, NP], F32, kind="ExternalInput")
    rbias = nc.dram_tensor("rbias", [128, 2, NQ, 2], F32, kind="ExternalInput")
    bo = nc.dram_tensor("bo", [128, TB], F32, kind="ExternalInput")
    outT = nc.dram_tensor("outT", [E, T], F16, kind="ExternalOutput")

    hsT16_r = hsT16.rearrange("(eo p) t -> p eo t", p=128)
    hsT8_r = hsT8.rearrange("(eo p) t -> p eo t", p=128)


    with TileContext(nc) as tc:
        from contextlib import ExitStack

        with ExitStack() as stack:
            const = stack.enter_context(tc.tile_pool(name="const", bufs=1))

            hsT16_sb = const.tile([128, EO, T], F16, tag="hsT16")
            hs8_sb = const.tile([128, EO, T], F8, tag="hs8")
            v16_sb = const.tile([128, SO, H, 66], F16, tag="v16")
            wv_sb = const.tile([128, EO, E], F16, tag="wv")
            v8_sb = const.tile([128, SO, H, 66], F8, tag="v8")
            gbias_sb = const.tile([128, 2, NP], F32, tag="gbias")
            rbias_sb = const.tile([128, 2, NQ, 2], F32, tag="rbias")
            bo_sb = const.tile([128, TB], F32, tag="bo")
            comb_tiles = [
                const.tile([128, E], F16, tag=f"comb{tb}", name=f"comb{tb}")
                for tb in range(TB)
            ]
            attnT_tiles = [
                const.tile([128, T], F16, tag=f"attnT{p}", name=f"attnT{p}")
                for p in range(NP)
            ]

            # ones columns pre-scaled by 1/w so reciprocal gives w/Z
            nc.vector.memset(v16_sb[:, :, :, 64:65], 1.0 / W_G)
            nc.vector.memset(v8_sb[:, :, :, 64:65], 1.0 / W_R)


            # persistent pools used across prologue + main
            qk16p = stack.enter_context(tc.tile_pool(name="qk16", bufs=2))
            qk8p = stack.enter_context(tc.tile_pool(name="qk8", bufs=2))
            wgp = stack.enter_context(tc.tile_pool(name="wgp", bufs=2))
            w8p = stack.enter_context(tc.tile_pool(name="w8p", bufs=2))

            def gen_proj_steps(pair, pp2_pool):
                """Generic q/k projection for a head pair -> list of step
                closures (PE matmuls + ACT drains). Weight DMAs fire now."""
                steps = []
                outs = {}
                wt2 = wgp.tile([128, 2, EO, 128], F16, tag="wg")
                nc.sync.dma_start(wt2[:], wg[pair])
                for qk in range(2):
                    wt = wt2[:, qk]
                    dst = qk16p.tile([128, T], F16, tag=f"qk16_{qk}")
                    outs[qk] = dst
                    state = {}

                    def mk(qk, wt, dst, state, th, eo):
                        def step():
                            if eo == 0:
                                state[th] = pp2_pool.tile(
                                    [128, 512], F32, tag="sm", name=f"gp{pair}{qk}{th}"
                                )
                            nc.tensor.matmul(
                                state[th][:],
                                wt[:, eo, :],
                                hsT16_sb[:, eo, th * 512 : (th + 1) * 512],
                                start=(eo == 0),
                                stop=(eo == EO - 1),
                            )
                            if eo == EO - 1:
                                if th == 0:
                                    nc.scalar.activation(
                                        dst[:, 0:512],
                                        state[th][:],
                                        mybir.ActivationFunctionType.Identity,
                                        bias=gbias_sb[:, qk, pair : pair + 1],
                                    )
                                else:
                                    nc.vector.tensor_scalar_add(
                                        dst[:, 512:1024],
                                        state[th][:],
                                        gbias_sb[:, qk, pair : pair + 1],
                                    )
                        return step

                    for th in range(2):
                        for eo in range(EO):
                            steps.append(mk(qk, wt, dst, state, th, eo))
                return outs, steps

            def vproj_steps(pair, pool, tag):
                """One closure per s-block: a full 8-matmul chain + drains,
                so each pump pop contributes ~0.4us of PE work instead of
                a single 53ns micro-matmul."""
                steps = []
                ocols = slice(pair * 128, (pair + 1) * 128)

                def mk(sb):
                    def step():
                        pv = pool.tile(
                            [128, 512], F32, tag=tag, name=f"pv{pair}_{sb}"
                        )
                        for eo in range(EO):
                            nc.tensor.matmul(
                                pv[:, 0:128],
                                hsT16_sb[:, eo, sb * 128 : (sb + 1) * 128],
                                wv_sb[:, eo, ocols],
                                start=(eo == 0),
                                stop=(eo == EO - 1),
                            )
                        pv_r = pv[:, 0:128].rearrange(
                            "p (hh dd) -> p hh dd", dd=64
                        )
                        vsl = v16_sb[:, sb, 2 * pair : 2 * pair + 2, 0:64]
                        nc.vector.tensor_copy(vsl, pv_r)
                        nc.gpsimd.tensor_copy(
                            v8_sb[:, sb, 2 * pair : 2 * pair + 2, 0:64], vsl
                        )
                    return step

                for sb in range(SO):
                    steps.append(mk(sb))
                return steps

            def rdr_proj_steps(quad, pp2_pool):
                """Reader q/k projection for a head quad (fp8 DoubleRow).
                Outputs supertiles [128(=4h x 32d), 2(d-half), T] fp8."""
                steps = []
                outs = {}
                wt4 = w8p.tile([128, 2, 2, EO, 128], F8, tag="w8")
                nc.sync.dma_start(wt4[:], w8[quad])
                for qk in range(2):
                    dst = qk8p.tile([128, 2, T], F8, tag=f"qk8_{qk}")
                    outs[qk] = dst
                    for ab in range(2):
                        wt = wt4[:, qk, ab]
                        state = {}

                        def mk(qk, ab, wt, dst, state, th, a):
                            def step():
                                if a == 0:
                                    state[th] = pp2_pool.tile(
                                        [128, 512], F32, tag="sm",
                                        name=f"rp{quad}{qk}{ab}{th}",
                                    )
                                nc.tensor.matmul(
                                    state[th][:],
                                    wt[:, 2 * a : 2 * a + 2, :],
                                    hs8_sb[:, 2 * a : 2 * a + 2,
                                           th * 512 : (th + 1) * 512],
                                    start=(a == 0),
                                    stop=(a == 3),
                                    perf_mode=DRM,
                                )
                                if a == 3:
                                    if th == 0:
                                        nc.vector.tensor_scalar_add(
                                            dst[:, ab, 0:512],
                                            state[th][:],
                                            rbias_sb[:, qk, quad, ab : ab + 1],
                                        )
                                    else:
                                        nc.scalar.activation(
                                            dst[:, ab, 512:1024],
                                            state[th][:],
                                            mybir.ActivationFunctionType.Identity,
                                            bias=rbias_sb[:, qk, quad, ab : ab + 1],
                                        )
                            return step

                        for th in range(2):
                            for a in range(4):
                                steps.append(mk(qk, ab, wt, dst, state, th, a))
                return outs, steps

            # ---------------- prologue: v proj + first projections ----------
            pump = []
            with tc.tile_pool(
                name="vps", bufs=2, space="PSUM"
            ) as vps, tc.tile_pool(name="pps", bufs=2, space="PSUM") as pps:
                # first chunk pair ahead of the 0.77MB weight DMAs so the
                # v-chains start immediately; weights next (prologue pump)
                nc.sync.dma_start(hsT16_sb[:, 0:1], hsT16_r[:, 0:1])
                nc.sync.dma_start(wv_sb[:, 0:1, 0:512], wv[:, 0:1, 0:512])
                qk0, steps_g0 = gen_proj_steps(0, pps)
                r0, steps_r0 = rdr_proj_steps(0, pps)
                for a, b in ((1, 2), (2, 4), (4, 6), (6, 8)):
                    nc.sync.dma_start(hsT16_sb[:, a:b], hsT16_r[:, a:b])
                    nc.sync.dma_start(wv_sb[:, a:b, 0:512], wv[:, a:b, 0:512])
                    if a == 2:
                        nc.sync.dma_start(gbias_sb[:], gbias[:])
                        nc.sync.dma_start(rbias_sb[:], rbias[:])
                        nc.sync.dma_start(bo_sb[:], bo[:])
                for a, b in ((0, 4), (4, 8)):
                    nc.sync.dma_start(hs8_sb[:, a:b], hsT8_r[:, a:b])
                # wv columns for the deferred v-proj pairs 6-7 (needed h>=9)
                nc.sync.dma_start(wv_sb[:, :, 512:1024], wv[:, :, 512:1024])
                pro_pump = steps_g0 + steps_r0

                for sb in range(SO):
                    pv = vps.tile([128, E], F32, tag="pv", name=f"pv{sb}")
                    for eo in range(EO):
                        nc.tensor.matmul(
                            pv[:, 0:512],
                            hsT16_sb[:, eo, sb * 128 : (sb + 1) * 128],
                            wv_sb[:, eo, 0:512],
                            start=(eo == 0),
                            stop=(eo == EO - 1),
                        )
                        for _ in range(2 if sb >= 4 else 1):
                            if pro_pump:
                                pro_pump.pop(0)()
                    pv_r = pv[:, 0:512].rearrange("p (hh dd) -> p hh dd", dd=64)
                    nc.scalar.copy(v16_sb[:, sb, 0:8, 0:64], pv_r)
                    nc.vector.tensor_copy(v8_sb[:, sb, 0:8, 0:64], pv_r)
                while pro_pump:
                    pro_pump.pop(0)()

            # ---------------- main attention loop ---------------------------
            # Software pipeline: the AV/combine work of head h-1 is emitted
            # interleaved into the scores/exp loop of head h, so the PE has
            # filler work while ACT/DVE drain the score tiles.
            wo_sb = const.tile([128, TB, NP, 128], F16, tag="wo_all")
            for j in range(TB):
                nc.sync.dma_start(wo_sb[:, j], wo[j])

            with ExitStack() as mstack:
                scp = mstack.enter_context(
                    tc.tile_pool(name="scp", bufs=2, space="PSUM")
                )
                smallp = mstack.enter_context(
                    tc.tile_pool(name="smallp", bufs=4, space="PSUM")
                )
                ex16p = mstack.enter_context(tc.tile_pool(name="ex16", bufs=2))
                ex8p = mstack.enter_context(tc.tile_pool(name="ex8", bufs=2))
                tmpp = mstack.enter_context(tc.tile_pool(name="tmpp", bufs=8))
                avsp = mstack.enter_context(tc.tile_pool(name="avsp", bufs=6))

                def av_steps(h, ex16, ex8):
                    """AV + combine for head h as a list of step closures.
                    Each tb yields 3 steps: gen-av mms, rdr-av mms, combine."""
                    pair, hp = h // 2, h % 2
                    vg = v16_sb[:, :, h, :]
                    v8h = v8_sb[:, :, h, :]
                    steps = []
                    state = {}

                    def mk_gen(tb):
                        def step():
                            tsl = slice(tb * 128, (tb + 1) * 128)
                            av = smallp.tile([128, 512], F32, tag="sm",
                                             name=f"av{h}_{tb}")
                            state[tb] = av
                            for a in range(SO):
                                nc.tensor.matmul(
                                    av[:, 0:65],
                                    ex16[:, a, tsl],
                                    vg[:, a, 0:65],
                                    start=(a == 0),
                                    stop=(a == SO - 1),
                                )
                        return step

                    def mk_rdr(tb):
                        def step():
                            tsl = slice(tb * 128, (tb + 1) * 128)
                            av = state[tb]
                            for a in range(4):
                                nc.tensor.matmul(
                                    av[:, 68:133],
                                    ex8[:, 2 * a : 2 * a + 2, tsl],
                                    v8h[:, 2 * a : 2 * a + 2, 0:65],
                                    start=(a == 0),
                                    stop=(a == 3),
                                    perf_mode=DRM,
                                )
                        return step

                    def mk_comb(tb):
                        def step():
                            av = state.pop(tb)
                            csl = slice(h * 64, h * 64 + 64)
                            tmp = tmpp.tile([128, 64], F16, tag="tmp")
                            avs = avsp.tile([128, 133], F32, tag="avs")
                            nc.scalar.copy(avs[:], av[:, 0:133])
                            nc.gpsimd.normalize_recip(
                                comb_tiles[tb][:, csl], avs[:, 0:64],
                                avs[:, 64:65],
                            )
                            nc.gpsimd.normalize_recip(
                                tmp[:], avs[:, 68:132], avs[:, 132:133],
                            )
                            nc.gpsimd.tensor_tensor(
                                comb_tiles[tb][:, csl],
                                comb_tiles[tb][:, csl],
                                tmp[:],
                                ADD,
                            )
                            if hp == 1:
                                nc.sync.dma_start_transpose(
                                    attnT_tiles[pair][
                                        :, tb * 128 : (tb + 1) * 128
                                    ],
                                    comb_tiles[tb][
                                        :, pair * 128 : (pair + 1) * 128
                                    ],
                                )
                        return step

                    for tb in range(TB):
                        steps.append(mk_gen(tb))
                        steps.append(mk_rdr(tb))
                        steps.append(mk_comb(tb))
                    return steps

                Qg = Kg = Q8 = K8 = None
                av_q = []  # pending av steps of the previous head

                def fill(n):
                    """Emit up to n units of filler: av steps take priority
                    (they unblock comb tiles), then proj pump steps."""
                    for _ in range(n):
                        if av_q:
                            av_q.pop(0)()
                        elif pump:
                            pump.pop(0)()

                for h in range(H):
                    pair, quad = h // 2, h // 4
                    hp, hq = h % 2, h % 4
                    if h == 0:
                        Qg, Kg = qk0[0], qk0[1]
                        Q8, K8 = r0[0], r0[1]
                        nxt_g = nxt_r = None
                    if h in (7, 9, 11, 13):
                        pump.extend(vproj_steps((h - 7) // 2 + 4, smallp, "sm"))
                    if hp == 0 and pair + 1 < NP:
                        nxt_g, s = gen_proj_steps(pair + 1, smallp)
                        pump.extend(s)
                    if hq == 0 and quad + 1 < NQ:
                        nxt_r, s = rdr_proj_steps(quad + 1, smallp)
                        pump.extend(s)

                    ex16 = ex16p.tile([128, SO, T], F16, tag="ex16")
                    ex8 = ex8p.tile([128, SO, T], F8, tag="ex8")

                    grow = slice(64 * hp, 64 * hp + 64)
                    rrow = slice(32 * hq, 32 * hq + 32)

                    for sb in range(SO):
                        ssl = slice(sb * 128, (sb + 1) * 128)
                        sc = scp.tile([128, T], F32, tag="sc", name=f"sc{h}_{sb}")
                        for th in range(2):
                            nc.tensor.matmul(
                                sc[:, th * 512 : (th + 1) * 512],
                                Kg[grow, ssl],
                                Qg[grow, th * 512 : (th + 1) * 512],
                                start=True, stop=True,
                            )
                        nc.scalar.activation(
                            ex16[:, sb, :], sc[:], EXP, scale=SCALING
                        )
                        fill(2)
                        for th in range(2):
                            rc = smallp.tile([128, 512], F32, tag="sm",
                                             name=f"rc{h}_{sb}_{th}")
                            if USE_RDR_SCORE_DR:
                                nc.tensor.matmul(
                                    rc[:],
                                    K8[rrow, :, ssl],
                                    Q8[rrow, :, th * 512 : (th + 1) * 512],
                                    start=True, stop=True,
                                    perf_mode=DRM,
                                    tile_position=(32 * hq, 0),
                                )
                            else:
                                for ab in range(2):
                                    nc.tensor.matmul(
                                        rc[:],
                                        K8[rrow, ab, ssl],
                                        Q8[rrow, ab, th * 512 : (th + 1) * 512],
                                        start=(ab == 0), stop=(ab == 1),
                                        tile_position=(32 * hq, 0),
                                    )
                            nc.vector.tensor_scalar(
                                ex8[:, sb, th * 512 : (th + 1) * 512].bitcast(U8),
                                rc[:],
                                SCH_A, SCH_B, MULT, ADD,
                            )
                            fill(1)
                        fill(1)

                    # queue this head's av work; emitted during later heads'
                    # scores loops via fill()
                    av_q.extend(av_steps(h, ex16, ex8))

                    if hp == 1:
                        while pump:
                            pump.pop(0)()
                        if nxt_g is not None:
                            Qg, Kg = nxt_g[0], nxt_g[1]
                            nxt_g = None
                        if hq == 3 and nxt_r is not None:
                            Q8, K8 = nxt_r[0], nxt_r[1]
                            nxt_r = None

                while av_q:
                    av_q.pop(0)()

            # ---------------- output projection -----------------------------
            with tc.tile_pool(name="ops", bufs=6, space="PSUM") as ops, \
                 tc.tile_pool(name="o16p", bufs=4) as o16p:
                # 4 half-chain accumulators live at once; prefix over pairs
                # 0..6 depends only on heads <= 13 so it overlaps the final
                # head's av/combine; the pair-7 step + drain go in wave 2.
                halves = [(j, th) for j in range(TB) for th in range(2)]
                pos = {}
                for w0 in range(0, 16, 4):
                    for j, th in halves[w0 : w0 + 4]:
                        tsl = slice(th * 512, (th + 1) * 512)
                        po = ops.tile([128, 512], F32, tag="po",
                                      name=f"po{j}_{th}")
                        pos[(j, th)] = po
                        for pr in range(NP - 1):
                            nc.tensor.matmul(
                                po[:],
                                wo_sb[:, j, pr, :],
                                attnT_tiles[pr][:, tsl],
                                start=(pr == 0),
                                stop=False,
                            )
                    for j, th in halves[w0 : w0 + 4]:
                        tsl = slice(th * 512, (th + 1) * 512)
                        po = pos.pop((j, th))
                        nc.tensor.matmul(
                            po[:],
                            wo_sb[:, j, NP - 1, :],
                            attnT_tiles[NP - 1][:, tsl],
                            start=False,
                            stop=True,
                        )
                        o16 = o16p.tile([128, 512], F16, tag="o16")
                        if th == 0:
                            nc.vector.tensor_scalar_add(
                                o16[:], po[:], bo_sb[:, j : j + 1]
                            )
                        else:
                            nc.scalar.activation(
                                o16[:], po[:],
                                mybir.ActivationFunctionType.Identity,
                                bias=bo_sb[:, j : j + 1],
                            )
                        nc.sync.dma_start(
                            outT[j * 128 : (j + 1) * 128, tsl], o16[:]
                        )

    nc.finalize()
    return nc


_NC_CACHE = {}


def get_nc():
    if "nc" not in _NC_CACHE:
        _NC_CACHE["nc"] = build_nc()
    return _NC_CACHE["nc"]


def _host_prep(hidden_states, reader_token, Wq, bq, Wk, bk, Wv, bv, Wo, bo,
               RWq, Rbq, RWk, Rbk, RWv, Rbv):
    f = np.float32
    np16 = mybir.dt.np(F16)
    np8 = mybir.dt.np(F8)
    hs = np.asarray(hidden_states, f)
    tok = np.asarray(reader_token).astype(np.int64)

    WqT = np.asarray(Wq, f).T  # [e, o]
    WkT = np.asarray(Wk, f).T
    WvT = np.asarray(Wv, f).T
    WoT = np.asarray(Wo, f).T
    bq = np.asarray(bq, f); bk = np.asarray(bk, f)
    bv = np.asarray(bv, f); bo_ = np.asarray(bo, f)
    Rbq = np.asarray(Rbq, f); Rbk = np.asarray(Rbk, f)

    # gen weights [NP, 128, 2, EO, 128]
    wg_arr = np.empty((NP, 128, 2, EO, 128), np16)
    for qk, WT in enumerate((WqT, WkT)):
        r = WT.reshape(EO, 128, NP, 128)  # (eo, p, pair, m)
        wg_arr[:, :, qk] = r.transpose(2, 1, 0, 3).astype(np16)
    # gen biases [128, 2, NP]
    gb = np.empty((128, 2, NP), f)
    for qk, bb in enumerate((bq, bk)):
        gb[:, qk, :] = bb.reshape(NP, 128).T

    # v-bias folds into output bias (probs rows sum to 0.5)
    bo_eff = bo_ + 0.5 * (np.asarray(Wo, f) @ bv)
    bo_t = np.ascontiguousarray(bo_eff.reshape(TB, 128).T)

    # wv [128, EO, E]
    wv_arr = np.ascontiguousarray(
        WvT.reshape(EO, 128, E).transpose(1, 0, 2)
    ).astype(np16)
    # wo [TB, 128, NP, 128]
    wo_arr = np.ascontiguousarray(
        WoT.reshape(NP, 128, TB, 128).transpose(2, 1, 0, 3)
    ).astype(np16)

    percore = {}
    in_maps = []
    for b in range(B):
        g = int(tok[b])
        if g not in percore:
            RWqT = np.asarray(RWq[g], f).T  # [e, o]
            RWkT = np.asarray(RWk[g], f).T
            w8_arr = np.empty((NQ, 128, 2, 2, EO, 128), np8)
            for qk, WT in enumerate((RWqT, RWkT)):
                # o = (quad*4 + m//32)*64 + ab*32 + m%32
                r = WT.reshape(EO, 128, NQ, 4, 2, 32)  # (eo,p,quad,hin,ab,dd)
                # -> (quad, p, qk, ab, eo, m=(hin,dd))
                w8_arr[:, :, qk] = r.transpose(2, 4, 1, 0, 3, 5).reshape(
                    NQ, 2, 128, EO, 128
                ).astype(np8).transpose(0, 2, 1, 3, 4).reshape(NQ, 128, 2, EO, 128)
            rb = np.empty((128, 2, NQ, 2), f)
            for qk, bb in enumerate((Rbq[g], Rbk[g])):
                # p = hin*32 + dd ; value = b[(quad*4+hin)*64 + ab*32 + dd]
                r = bb.reshape(NQ, 4, 2, 32)  # (quad, hin, ab, dd)
                rb[:, qk, :, :] = r.transpose(1, 3, 0, 2).reshape(128, NQ, 2)
            percore[g] = (w8_arr, rb)
        w8_arr, rb = percore[g]
        hsT = np.ascontiguousarray(hs[b].T)
        in_maps.append(
            {
                "hsT16": hsT.astype(np16),
                "hsT8": hsT.astype(np8),
                "wg": wg_arr,
                "w8": w8_arr,
                "wv": wv_arr,
                "wo": wo_arr,
                "gbias": gb,
                "rbias": rb,
                "bo": bo_t,
            }
        )
    return in_maps


def kernel(**inputs) -> np.ndarray:
    in_maps = _host_prep(**inputs)
    nc = get_nc()
    res = run_bass_kernel_spmd(nc, in_maps, list(range(B)))
    out = np.stack(
        [np.asarray(res.results[c]["outT"]).astype(np.float32).T for c in range(B)],
        axis=0,
    )
    return np.ascontiguousarray(out)


# revision 12
# speedup vs baseline: 1.1181x; 1.0002x over previous
"""AuthorGroupAttention Trainium2 kernel.

Data-parallel over batch: 8 samples -> 8 NeuronCores, one sample per core.
Routing resolved on host (per-core reader-group weights gathered, cast and
laid out per-engine-friendly in _host_prep).

Precision (validated vs reference: rel err ~5e-3 against the 2e-2 gate):
  - generic path: fp16 matmul operands everywhere, fp32 PSUM accumulation
  - reader path (weight 0.1 in the prob combine): fp8e4 operands with
    DoubleRow matmuls (0.5 cyc/row); its exp is a Schraudolph bit-trick on
    DVE (scores*1/ln2 + 55.2 rounded to uint8 = e4m3 bits of exp(scores/8)),
    consistent numerator/denominator so the approximation bias cancels in
    the softmax normalization.

Structure:
  - q/k produced in [d, t] layout (d on partitions): per head-pair (gen,
    fp16) / head-quad (rdr, fp8 DR with 32-row subtiles at tile_position)
    projection chains, interleaved into the attention loop as PE filler
    ("pump") with per-boundary forced drains.
  - scores per (head, s-block): gen [128,1024] psum tiles (2-deep pool),
    rdr th-split [128,512] tiles in a shared 4-deep "small" pool that also
    carries av accumulators, projection chains, and deferred v-proj.
  - attention transposed: stationary = exp tile [s,t], moving = [v | 1/w]
    so attn lands [t, d] with the softmax denominator Z/w in the extra
    column; one ACT copy bounces the accumulator to SBUF, gpsimd
    normalize_recip applies w/Z per path, gpsimd add combines gen+rdr.
  - attn [t, e] tiles are DMA-transposed (xbar) per (pair, t-block) into
    [e, t] for the fp16 output projection; v-bias is folded into the output
    bias on host (combined prob rows sum to 0.5).
  - AV/combine of head h-1 is software-pipelined into head h's score loop;
    v-projection for pairs 6-7 is deferred into the pump as late filler.
"""

import os
import sys

for _p in ("/opt/trn_rl_repo",):
    if os.path.isdir(_p) and _p not in sys.path:
        sys.path.insert(0, _p)

import numpy as np

import concourse.bass as bass
import concourse.mybir as mybir
from concourse import bacc
from concourse.tile import TileContext
from concourse.bass_utils import run_bass_kernel_spmd

B, T, E, H, G = 8, 1024, 1024, 16, 4
D = E // H  # 64
SCALING = float(D) ** -0.5
W_G = 0.9 / 2.0
W_R = 0.1 / 2.0
EO = 8
SO = 8
TB = 8
NP = 8  # head pairs
NQ = 4  # head quads

F32 = mybir.dt.float32
F16 = mybir.dt.float16
F8 = mybir.dt.float8e4
U8 = mybir.dt.uint8
DRM = mybir.MatmulPerfMode.DoubleRow
EXP = mybir.ActivationFunctionType.Exp
MULT = mybir.AluOpType.mult
ADD = mybir.AluOpType.add

# Schraudolph constants for e4m3 bits: bits = round(score*A + Bc)
SCH_A = SCALING * 8.0 / float(np.log(2.0))
SCH_B = 56.0 - 0.8

USE_RDR_SCORE_DR = True  # DoubleRow with 32-row subtiles for reader scores


def build_nc():
    nc = bacc.Bacc(name="author_group_attention_v2")

    hsT16 = nc.dram_tensor("hsT16", [E, T], F16, kind="ExternalInput")
    hsT8 = nc.dram_tensor("hsT8", [E, T], F8, kind="ExternalInput")
    wg = nc.dram_tensor("wg", [NP, 128, 2, EO, 128], F16, kind="ExternalInput")
    w8 = nc.dram_tensor("w8", [NQ, 128, 2, 2, EO, 128], F8, kind="ExternalInput")
    wv = nc.dram_tensor("wv", [128, EO, E], F16, kind="ExternalInput")
    wo = nc.dram_tensor("wo", [TB, 128, NP, 128], F16, kind="ExternalInput")
    gbias = nc.dram_tensor("gbias", [128, 2, NP], F32, kind="ExternalInput")
    rbias = nc.dram_tensor("rbias", [128, 2, NQ, 2], F32, kind="ExternalInput")
    bo = nc.dram_tensor("bo", [128, TB], F32, kind="ExternalInput")
    outT = nc.dram_tensor("outT", [E, T], F16, kind="ExternalOutput")

    hsT16_r = hsT16.rearrange("(eo p) t -> p eo t", p=128)
    hsT8_r = hsT8.rearrange("(eo p) t -> p eo t", p=128)


    with TileContext(nc) as tc:
        from contextlib import ExitStack

        with ExitStack() as stack:
            const = stack.enter_context(tc.tile_pool(name="const", bufs=1))

            hsT16_sb = const.tile([128, EO, T], F16, tag="hsT16")
            hs8_sb = const.tile([128, EO, T], F8, tag="hs8")
            v16_sb = const.tile([128, SO, H, 66], F16, tag="v16")
            wv_sb = const.tile([128, EO, E], F16, tag="wv")
            v8_sb = const.tile([128, SO, H, 66], F8, tag="v8")
            gbias_sb = const.tile([128, 2, NP], F32, tag="gbias")
            rbias_sb = const.tile([128, 2, NQ, 2], F32, tag="rbias")
            bo_sb = const.tile([128, TB], F32, tag="bo")
            comb_tiles = [
                const.tile([128, E], F16, tag=f"comb{tb}", name=f"comb{tb}")
                for tb in range(TB)
            ]
            attnT_tiles = [
                const.tile([128, T], F16, tag=f"attnT{p}", name=f"attnT{p}")
                for p in range(NP)
            ]

            # ones columns pre-scaled by 1/w so reciprocal gives w/Z
            nc.vector.memset(v16_sb[:, :, :, 64:65], 1.0 / W_G)
            nc.vector.memset(v8_sb[:, :, :, 64:65], 1.0 / W_R)


            # persistent pools used across prologue + main
            qk16p = stack.enter_context(tc.tile_pool(name="qk16", bufs=2))
            qk8p = stack.enter_context(tc.tile_pool(name="qk8", bufs=2))
            wgp = stack.enter_context(tc.tile_pool(name="wgp", bufs=2))
            w8p = stack.enter_context(tc.tile_pool(name="w8p", bufs=2))

            def gen_proj_steps(pair, pp2_pool):
                """Generic q/k projection for a head pair -> list of step
                closures (PE matmuls + ACT drains). Weight DMAs fire now."""
                steps = []
                outs = {}
                wt2 = wgp.tile([128, 2, EO, 128], F16, tag="wg")
                nc.sync.dma_start(wt2[:], wg[pair])
                for qk in range(2):
                    wt = wt2[:, qk]
                    dst = qk16p.tile([128, T], F16, tag=f"qk16_{qk}")
                    outs[qk] = dst
                    state = {}

                    def mk(qk, wt, dst, state, th, eo):
                        def step():
                            if eo == 0:
                                state[th] = pp2_pool.tile(
                                    [128, 512], F32, tag="sm", name=f"gp{pair}{qk}{th}"
                                )
                            nc.tensor.matmul(
                                state[th][:],
                                wt[:, eo, :],
                                hsT16_sb[:, eo, th * 512 : (th + 1) * 512],
                                start=(eo == 0),
                                stop=(eo == EO - 1),
                            )
                            if eo == EO - 1:
                                if th == 0:
                                    nc.scalar.activation(
                                        dst[:, 0:512],
                                        state[th][:],
                                        mybir.ActivationFunctionType.Identity,
                                        bias=gbias_sb[:, qk, pair : pair + 1],
                                    )
                                else:
                                    nc.vector.tensor_scalar_add(
                                        dst[:, 512:1024],
                                        state[th][:],
                                        gbias_sb[:, qk, pair : pair + 1],
                                    )
                        return step

                    for th in range(2):
                        for eo in range(EO):
                            steps.append(mk(qk, wt, dst, state, th, eo))
                return outs, steps

            def vproj_steps(pair, pool, tag):
                """One closure per s-block: a full 8-matmul chain + drains,
                so each pump pop contributes ~0.4us of PE work instead of
                a single 53ns micro-matmul."""
                steps = []
                ocols = slice(pair * 128, (pair + 1) * 128)

                def mk(sb):
                    def step():
                        pv = pool.tile(
                            [128, 512], F32, tag=tag, name=f"pv{pair}_{sb}"
                        )
                        for eo in range(EO):
                            nc.tensor.matmul(
                                pv[:, 0:128],
                                hsT16_sb[:, eo, sb * 128 : (sb + 1) * 128],
                                wv_sb[:, eo, ocols],
                                start=(eo == 0),
                                stop=(eo == EO - 1),
                            )
                        pv_r = pv[:, 0:128].rearrange(
                            "p (hh dd) -> p hh dd", dd=64
                        )
                        vsl = v16_sb[:, sb, 2 * pair : 2 * pair + 2, 0:64]
                        nc.vector.tensor_copy(vsl, pv_r)
                        nc.gpsimd.tensor_copy(
                            v8_sb[:, sb, 2 * pair : 2 * pair + 2, 0:64], vsl
                        )
                    return step

                for sb in range(SO):
                    steps.append(mk(sb))
                return steps

            def rdr_proj_steps(quad, pp2_pool):
                """Reader q/k projection for a head quad (fp8 DoubleRow).
                Outputs supertiles [128(=4h x 32d), 2(d-half), T] fp8."""
                steps = []
                outs = {}
                wt4 = w8p.tile([128, 2, 2, EO, 128], F8, tag="w8")
                nc.sync.dma_start(wt4[:], w8[quad])
                for qk in range(2):
                    dst = qk8p.tile([128, 2, T], F8, tag=f"qk8_{qk}")
                    outs[qk] = dst
                    for ab in range(2):
                        wt = wt4[:, qk, ab]
                        state = {}

                        def mk(qk, ab, wt, dst, th):
                            def step():
                                po = pp2_pool.tile(
                                    [128, 512], F32, tag="sm",
                                    name=f"rp{quad}{qk}{ab}{th}",
                                )
                                for a in range(4):
                                    nc.tensor.matmul(
                                        po[:],
                                        wt[:, 2 * a : 2 * a + 2, :],
                                        hs8_sb[:, 2 * a : 2 * a + 2,
                                               th * 512 : (th + 1) * 512],
                                        start=(a == 0),
                                        stop=(a == 3),
                                        perf_mode=DRM,
                                    )
                                if th == 0:
                                    nc.vector.tensor_scalar_add(
                                        dst[:, ab, 0:512], po[:],
                                        rbias_sb[:, qk, quad, ab : ab + 1],
                                    )
                                else:
                                    nc.scalar.activation(
                                        dst[:, ab, 512:1024], po[:],
                                        mybir.ActivationFunctionType.Identity,
                                        bias=rbias_sb[:, qk, quad, ab : ab + 1],
                                    )
                            return step

                        for th in range(2):
                            steps.append(mk(qk, ab, wt, dst, th))
                return outs, steps

            # ---------------- prologue: v proj + first projections ----------
            pump = []
            with tc.tile_pool(
                name="vps", bufs=2, space="PSUM"
            ) as vps, tc.tile_pool(name="pps", bufs=2, space="PSUM") as pps:
                # first chunk pair ahead of the 0.77MB weight DMAs so the
                # v-chains start immediately; weights next (prologue pump)
                nc.sync.dma_start(hsT16_sb[:, 0:1], hsT16_r[:, 0:1])
                nc.sync.dma_start(wv_sb[:, 0:1, 0:512], wv[:, 0:1, 0:512])
                qk0, steps_g0 = gen_proj_steps(0, pps)
                r0, steps_r0 = rdr_proj_steps(0, pps)
                for a, b in ((1, 2), (2, 4), (4, 6), (6, 8)):
                    nc.sync.dma_start(hsT16_sb[:, a:b], hsT16_r[:, a:b])
                    nc.sync.dma_start(wv_sb[:, a:b, 0:512], wv[:, a:b, 0:512])
                    if a == 2:
                        nc.sync.dma_start(gbias_sb[:], gbias[:])
                        nc.sync.dma_start(rbias_sb[:], rbias[:])
                        nc.sync.dma_start(bo_sb[:], bo[:])
                for a, b in ((0, 4), (4, 8)):
                    nc.sync.dma_start(hs8_sb[:, a:b], hsT8_r[:, a:b])
                # wv columns for the deferred v-proj pairs 6-7 (needed h>=9)
                nc.sync.dma_start(wv_sb[:, :, 512:1024], wv[:, :, 512:1024])
                pro_pump = steps_g0 + steps_r0

                for sb in range(SO):
                    pv = vps.tile([128, E], F32, tag="pv", name=f"pv{sb}")
                    for eo in range(EO):
                        nc.tensor.matmul(
                            pv[:, 0:512],
                            hsT16_sb[:, eo, sb * 128 : (sb + 1) * 128],
                            wv_sb[:, eo, 0:512],
                            start=(eo == 0),
                            stop=(eo == EO - 1),
                        )
                        for _ in range(2 if sb >= 4 else 1):
                            if pro_pump:
                                pro_pump.pop(0)()
                    pv_r = pv[:, 0:512].rearrange("p (hh dd) -> p hh dd", dd=64)
                    nc.scalar.copy(v16_sb[:, sb, 0:8, 0:64], pv_r)
                    nc.vector.tensor_copy(v8_sb[:, sb, 0:8, 0:64], pv_r)
                while pro_pump:
                    pro_pump.pop(0)()

            # ---------------- main attention loop ---------------------------
            # Software pipeline: the AV/combine work of head h-1 is emitted
            # interleaved into the scores/exp loop of head h, so the PE has
            # filler work while ACT/DVE drain the score tiles.
            wo_sb = const.tile([128, TB, NP, 128], F16, tag="wo_all")
            for j in range(TB):
                nc.sync.dma_start(wo_sb[:, j], wo[j])

            with ExitStack() as mstack:
                scp = mstack.enter_context(
                    tc.tile_pool(name="scp", bufs=2, space="PSUM")
                )
                smallp = mstack.enter_context(
                    tc.tile_pool(name="smallp", bufs=4, space="PSUM")
                )
                ex16p = mstack.enter_context(tc.tile_pool(name="ex16", bufs=2))
                ex8p = mstack.enter_context(tc.tile_pool(name="ex8", bufs=2))
                tmpp = mstack.enter_context(tc.tile_pool(name="tmpp", bufs=8))
                avsp = mstack.enter_context(tc.tile_pool(name="avsp", bufs=6))

                def av_steps(h, ex16, ex8):
                    """AV + combine for head h as a list of step closures.
                    Each tb yields 3 steps: gen-av mms, rdr-av mms, combine."""
                    pair, hp = h // 2, h % 2
                    vg = v16_sb[:, :, h, :]
                    v8h = v8_sb[:, :, h, :]
                    steps = []
                    state = {}

                    def mk_gen(tb):
                        def step():
                            tsl = slice(tb * 128, (tb + 1) * 128)
                            av = smallp.tile([128, 512], F32, tag="sm",
                                             name=f"av{h}_{tb}")
                            state[tb] = av
                            for a in range(SO):
                                nc.tensor.matmul(
                                    av[:, 0:65],
                                    ex16[:, a, tsl],
                                    vg[:, a, 0:65],
                                    start=(a == 0),
                                    stop=(a == SO - 1),
                                )
                        return step

                    def mk_rdr(tb):
                        def step():
                            tsl = slice(tb * 128, (tb + 1) * 128)
                            av = state[tb]
                            for a in range(4):
                                nc.tensor.matmul(
                                    av[:, 68:133],
                                    ex8[:, 2 * a : 2 * a + 2, tsl],
                                    v8h[:, 2 * a : 2 * a + 2, 0:65],
                                    start=(a == 0),
                                    stop=(a == 3),
                                    perf_mode=DRM,
                                )
                        return step

                    def mk_comb(tb):
                        def step():
                            av = state.pop(tb)
                            csl = slice(h * 64, h * 64 + 64)
                            tmp = tmpp.tile([128, 64], F16, tag="tmp")
                            avs = avsp.tile([128, 133], F32, tag="avs")
                            nc.scalar.copy(avs[:], av[:, 0:133])
                            nc.gpsimd.normalize_recip(
                                comb_tiles[tb][:, csl], avs[:, 0:64],
                                avs[:, 64:65],
                            )
                            nc.gpsimd.normalize_recip(
                                tmp[:], avs[:, 68:132], avs[:, 132:133],
                            )
                            nc.gpsimd.tensor_tensor(
                                comb_tiles[tb][:, csl],
                                comb_tiles[tb][:, csl],
                                tmp[:],
                                ADD,
                            )
                            if hp == 1:
                                nc.sync.dma_start_transpose(
                                    attnT_tiles[pair][
                                        :, tb * 128 : (tb + 1) * 128
                                    ],
                                    comb_tiles[tb][
                                        :, pair * 128 : (pair + 1) * 128
                                    ],
                                )
                        return step

                    for tb in range(TB):
                        steps.append(mk_gen(tb))
                        steps.append(mk_rdr(tb))
                        steps.append(mk_comb(tb))
                    return steps

                Qg = Kg = Q8 = K8 = None
                av_q = []  # pending av steps of the previous head

                def fill(n):
                    """Emit up to n units of filler: av steps take priority
                    (they unblock comb tiles), then proj pump steps."""
                    for _ in range(n):
                        if av_q:
                            av_q.pop(0)()
                        elif pump:
                            pump.pop(0)()

                for h in range(H):
                    pair, quad = h // 2, h // 4
                    hp, hq = h % 2, h % 4
                    if h == 0:
                        Qg, Kg = qk0[0], qk0[1]
                        Q8, K8 = r0[0], r0[1]
                        nxt_g = nxt_r = None
                    if h in (7, 9, 11, 13):
                        pump.extend(vproj_steps((h - 7) // 2 + 4, smallp, "sm"))
                    if hp == 0 and pair + 1 < NP:
                        nxt_g, s = gen_proj_steps(pair + 1, smallp)
                        pump.extend(s)
                    if hq == 0 and quad + 1 < NQ:
                        nxt_r, s = rdr_proj_steps(quad + 1, smallp)
                        pump.extend(s)

                    ex16 = ex16p.tile([128, SO, T], F16, tag="ex16")
                    ex8 = ex8p.tile([128, SO, T], F8, tag="ex8")

                    grow = slice(64 * hp, 64 * hp + 64)
                    rrow = slice(32 * hq, 32 * hq + 32)

                    for sb in range(SO):
                        ssl = slice(sb * 128, (sb + 1) * 128)
                        sc = scp.tile([128, T], F32, tag="sc", name=f"sc{h}_{sb}")
                        for th in range(2):
                            nc.tensor.matmul(
                                sc[:, th * 512 : (th + 1) * 512],
                                Kg[grow, ssl],
                                Qg[grow, th * 512 : (th + 1) * 512],
                                start=True, stop=True,
                            )
                        nc.scalar.activation(
                            ex16[:, sb, :], sc[:], EXP, scale=SCALING
                        )
                        fill(2)
                        for th in range(2):
                            rc = smallp.tile([128, 512], F32, tag="sm",
                                             name=f"rc{h}_{sb}_{th}")
                            if USE_RDR_SCORE_DR:
                                nc.tensor.matmul(
                                    rc[:],
                                    K8[rrow, :, ssl],
                                    Q8[rrow, :, th * 512 : (th + 1) * 512],
                                    start=True, stop=True,
                                    perf_mode=DRM,
                                    tile_position=(32 * hq, 0),
                                )
                            else:
                                for ab in range(2):
                                    nc.tensor.matmul(
                                        rc[:],
                                        K8[rrow, ab, ssl],
                                        Q8[rrow, ab, th * 512 : (th + 1) * 512],
                                        start=(ab == 0), stop=(ab == 1),
                                        tile_position=(32 * hq, 0),
                                    )
                            nc.vector.tensor_scalar(
                                ex8[:, sb, th * 512 : (th + 1) * 512].bitcast(U8),
                                rc[:],
                                SCH_A, SCH_B, MULT, ADD,
                            )
                            fill(1)
                        fill(1)

                    # queue this head's av work; emitted during later heads'
                    # scores loops via fill()
                    av_q.extend(av_steps(h, ex16, ex8))

                    if hp == 1:
                        while pump:
                            pump.pop(0)()
                        if nxt_g is not None:
                            Qg, Kg = nxt_g[0], nxt_g[1]
                            nxt_g = None
                        if hq == 3 and nxt_r is not None:
                            Q8, K8 = nxt_r[0], nxt_r[1]
                            nxt_r = None

                while av_q:
                    av_q.pop(0)()

            # ---------------- output projection -----------------------------
            with tc.tile_pool(name="ops", bufs=6, space="PSUM") as ops, \
                 tc.tile_pool(name="o16p", bufs=4) as o16p:
                # 4 half-chain accumulators live at once; prefix over pairs
                # 0..6 depends only on heads <= 13 so it overlaps the final
                # head's av/combine; the pair-7 step + drain go in wave 2.
                halves = [(j, th) for j in range(TB) for th in range(2)]
                pos = {}
                for w0 in range(0, 16, 4):
                    for j, th in halves[w0 : w0 + 4]:
                        tsl = slice(th * 512, (th + 1) * 512)
                        po = ops.tile([128, 512], F32, tag="po",
                                      name=f"po{j}_{th}")
                        pos[(j, th)] = po
                        for pr in range(NP - 1):
                            nc.tensor.matmul(
                                po[:],
                                wo_sb[:, j, pr, :],
                                attnT_tiles[pr][:, tsl],
                                start=(pr == 0),
                                stop=False,
                            )
                    for j, th in halves[w0 : w0 + 4]:
                        tsl = slice(th * 512, (th + 1) * 512)
                        po = pos.pop((j, th))
                        nc.tensor.matmul(
                            po[:],
                            wo_sb[:, j, NP - 1, :],
                            attnT_tiles[NP - 1][:, tsl],
                            start=False,
                            stop=True,
                        )
                        o16 = o16p.tile([128, 512], F16, tag="o16")
                        if th == 0:
                            nc.vector.tensor_scalar_add(
                                o16[:], po[:], bo_sb[:, j : j + 1]
                            )
                        else:
                            nc.scalar.activation(
                                o16[:], po[:],
                                mybir.ActivationFunctionType.Identity,
                                bias=bo_sb[:, j : j + 1],
                            )
                        nc.sync.dma_start(
                            outT[j * 128 : (j + 1) * 128, tsl], o16[:]
                        )

    nc.finalize()
    return nc


_NC_CACHE = {}


def get_nc():
    if "nc" not in _NC_CACHE:
        _NC_CACHE["nc"] = build_nc()
    return _NC_CACHE["nc"]


def _host_prep(hidden_states, reader_token, Wq, bq, Wk, bk, Wv, bv, Wo, bo,
               RWq, Rbq, RWk, Rbk, RWv, Rbv):
    f = np.float32
    np16 = mybir.dt.np(F16)
    np8 = mybir.dt.np(F8)
    hs = np.asarray(hidden_states, f)
    tok = np.asarray(reader_token).astype(np.int64)

    WqT = np.asarray(Wq, f).T  # [e, o]
    WkT = np.asarray(Wk, f).T
    WvT = np.asarray(Wv, f).T
    WoT = np.asarray(Wo, f).T
    bq = np.asarray(bq, f); bk = np.asarray(bk, f)
    bv = np.asarray(bv, f); bo_ = np.asarray(bo, f)
    Rbq = np.asarray(Rbq, f); Rbk = np.asarray(Rbk, f)

    # gen weights [NP, 128, 2, EO, 128]
    wg_arr = np.empty((NP, 128, 2, EO, 128), np16)
    for qk, WT in enumerate((WqT, WkT)):
        r = WT.reshape(EO, 128, NP, 128)  # (eo, p, pair, m)
        wg_arr[:, :, qk] = r.transpose(2, 1, 0, 3).astype(np16)
    # gen biases [128, 2, NP]
    gb = np.empty((128, 2, NP), f)
    for qk, bb in enumerate((bq, bk)):
        gb[:, qk, :] = bb.reshape(NP, 128).T

    # v-bias folds into output bias (probs rows sum to 0.5)
    bo_eff = bo_ + 0.5 * (np.asarray(Wo, f) @ bv)
    bo_t = np.ascontiguousarray(bo_eff.reshape(TB, 128).T)

    # wv [128, EO, E]
    wv_arr = np.ascontiguousarray(
        WvT.reshape(EO, 128, E).transpose(1, 0, 2)
    ).astype(np16)
    # wo [TB, 128, NP, 128]
    wo_arr = np.ascontiguousarray(
        WoT.reshape(NP, 128, TB, 128).transpose(2, 1, 0, 3)
    ).astype(np16)

    percore = {}
    in_maps = []
    for b in range(B):
        g = int(tok[b])
        if g not in percore:
            RWqT = np.asarray(RWq[g], f).T  # [e, o]
            RWkT = np.asarray(RWk[g], f).T
            w8_arr = np.empty((NQ, 128, 2, 2, EO, 128), np8)
            for qk, WT in enumerate((RWqT, RWkT)):
                # o = (quad*4 + m//32)*64 + ab*32 + m%32
                r = WT.reshape(EO, 128, NQ, 4, 2, 32)  # (eo,p,quad,hin,ab,dd)
                # -> (quad, p, qk, ab, eo, m=(hin,dd))
                w8_arr[:, :, qk] = r.transpose(2, 4, 1, 0, 3, 5).reshape(
                    NQ, 2, 128, EO, 128
                ).astype(np8).transpose(0, 2, 1, 3, 4).reshape(NQ, 128, 2, EO, 128)
            rb = np.empty((128, 2, NQ, 2), f)
            for qk, bb in enumerate((Rbq[g], Rbk[g])):
                # p = hin*32 + dd ; value = b[(quad*4+hin)*64 + ab*32 + dd]
                r = bb.reshape(NQ, 4, 2, 32)  # (quad, hin, ab, dd)
                rb[:, qk, :, :] = r.transpose(1, 3, 0, 2).reshape(128, NQ, 2)
            percore[g] = (w8_arr, rb)
        w8_arr, rb = percore[g]
        hsT = np.ascontiguousarray(hs[b].T)
        in_maps.append(
            {
                "hsT16": hsT.astype(np16),
                "hsT8": hsT.astype(np8),
                "wg": wg_arr,
                "w8": w8_arr,
                "wv": wv_arr,
                "wo": wo_arr,
                "gbias": gb,
                "rbias": rb,
                "bo": bo_t,
            }
        )
    return in_maps


def kernel(**inputs) -> np.ndarray:
    in_maps = _host_prep(**inputs)
    nc = get_nc()
    res = run_bass_kernel_spmd(nc, in_maps, list(range(B)))
    out = np.stack(
        [np.asarray(res.results[c]["outT"]).astype(np.float32).T for c in range(B)],
        axis=0,
    )
    return np.ascontiguousarray(out)
